# revision 9
# baseline (speedup 1.0000x reference)
"""HashEmbedder3D Trainium2 kernel v6.

Key changes vs v2 baseline:
- dma_gather with single_packet=False + indices replicated across all 8
  16-partition groups allows 8192-idx instructions (994ns fixed cost
  amortized 8x).
- Dense levels 0-3: slot-per-voxel-base block tables (one 32B descriptor
  fetches all 8 corners; no select trees). Levels 4-7: Q-packed slots
  (one descriptor + small k-offset select tree).
- Hash levels 8-15: one 64B slot read per corner (8 classes), extraction
  via 4-round select tree, gathers batched 2 classes per instruction.
- Index wrap/transpose done by DRAM-bounce DMA + DVE interleave + SBUF
  broadcast DMAs; Pool engine only runs SWDGE gathers.
"""
import math
import sys

import numpy as np

sys.path.insert(0, "/opt/trn_rl_repo")

from concourse import bacc, bass, mybir
import concourse.tile as tile

N_LEVELS = 16
F = 2
LOG2_T = 19
T = 1 << LOG2_T
BASE, FINEST = 16, 512
B_GROWTH = float(np.exp((np.log(np.float32(FINEST)) - np.log(np.float32(BASE))) / np.float32(N_LEVELS - 1)))
RES = [math.floor(BASE * B_GROWTH**i) for i in range(N_LEVELS)]
SIZES = [(r + 1) ** 3 if r**3 < T else T for r in RES]
OFFS = np.concatenate([[0], np.cumsum(SIZES)]).tolist()
TOTAL_ROWS = OFFS[-1]
PRIMES = [1, 2654435761, 805459861]
N_POINTS = 1048576
N_CORES = 8
P = 128

DT = mybir.dt
AL = mybir.AluOpType

DENSE_LV = list(range(8))
HASH_LV = list(range(8, 16))

# dense level slot geometry: levels 0-3 slot-per-base, 4-7 Q-packed
LEVEL_Q = {0: 1, 1: 1, 2: 1, 3: 1, 4: 2, 5: 4, 6: 8, 7: 16}
LEVEL_M = {}
N_SLOTS, ELEM = {}, {}
for l in DENSE_LV:
    r, Q = RES[l], LEVEL_Q[l]
    M = -(-r // Q)
    LEVEL_M[l] = M
    N_SLOTS[l] = r * r * M
    ELEM[l] = 8 if Q == 1 else 4 * (Q + 1)
ELEM[7] = 2 * (LEVEL_Q[7] + 1)  # level 7 split in two di-halves of 2x17 lanes
for l in HASH_LV:
    N_SLOTS[l], ELEM[l] = T // 16, 16
SLOT_OFF = {}
_a = 0
for l in range(N_LEVELS):
    SLOT_OFF[l] = _a
    _a += N_SLOTS[l]
SLOT_OFF7B = _a  # second (di=1) half of level 7
_a += N_SLOTS[7]
TOT_SLOTS = _a
assert all(N_SLOTS[l] <= 32768 for l in range(N_LEVELS)), N_SLOTS

# class layout: dense levels are classes 0-7; hash level l corner m is
# class 8 + (l-8)*8 + m. Each class contributes S indices per tile.
NCC = 8 + 8 * 8  # 72


def _i32(v):
    return int(np.int32(np.uint32(v)))


MAX_GIDX = 8192


def dma_gather_raw(eng, out_ap, in_ap, idxs_ap, num_idxs, elem_size, elem_step=64):
    stride_bytes = elem_step * 4
    assert stride_bytes % 256 == 0
    _in_ap = eng.lower_ap_dma(in_ap, for_custom_bir_dma=True)
    _idxs_ap = eng.lower_ap(idxs_ap)
    _out_ap = eng.lower_ap(out_ap)
    return eng.add_instruction(
        mybir.InstDMAGatherAnt(
            name=eng.bass.get_next_instruction_name(),
            ins=[*_in_ap, _idxs_ap, eng.lower_val_access(eng.to_reg(num_idxs))],
            outs=[_out_ap],
            transpose=False,
            num_idxs=num_idxs,
            elem_size=elem_size,
            stride_bytes_256=stride_bytes // 256,
            gen_mode=0,
            single_packet=num_idxs <= 1024,
            queue_num=0,
            sbuf_tokens_per_rank=0,
            sbuf_free_dim_per_rank=0,
            sbuf_free_dim_pad_per_rank=0,
            sbuf_byte_offset=0,
        )
    )


def apd(tap, off, dims):
    return bass.AP(tap.tensor, tap.offset + off, [list(d) for d in dims])


def build_kernel(slots_total, S):
    n_outer = slots_total // S
    assert n_outer * S == slots_total

    nc = bacc.Bacc(None, target_bir_lowering=False, debug=False)
    x_in = nc.dram_tensor("x", [P, slots_total, 3], DT.float32, kind="ExternalInput")
    tabx = nc.dram_tensor("tabx", [TOT_SLOTS, 64], DT.int32, kind="ExternalInput")
    cvec_in = nc.dram_tensor("cvec", [1, 128], DT.float32, kind="ExternalInput")
    out = nc.dram_tensor("out", [P, slots_total, 32], DT.float32, kind="ExternalOutput")

    nW = NCC * S  # idx ints per partition per tile

    with tile.TileContext(nc) as tc:
        with (
            tc.tile_pool(name="big", bufs=1) as bigp,
            tc.tile_pool(name="dbl", bufs=1) as dblp,
            tc.tile_pool(name="work", bufs=1) as workp,
        ):
            cv = bigp.tile([P, 128], DT.float32, tag="cv", name="cv")
            nc.sync.dma_start(cv[:], apd(cvec_in[:], 0, [[0, P], [1, 128]]))

            def cvb3(col, n, w=8):
                # [P, n, w] view of per-level const at cv[col:col+w]
                return apd(cv, col, [list(cv.ap[0]), [0, n], [1, w]])

            def cvb4(col, a, b, w=8):
                return apd(cv, col, [list(cv.ap[0]), [0, a], [0, b], [1, w]])

            x_t = bigp.tile([P, S, 3], DT.float32, tag="x_t", name="x_t")
            w_bf = None
            idxb = bigp.tile([P, nW], DT.int16, tag="idxb", name="idxb")
            hm = None
            koffs = None
            scr = bigp.tile([P, nW], DT.int16, tag="scr", name="scr", space="DRAM")
            wt = bigp.tile([P, 2 * nW], DT.int16, tag="wt", name="wt")
            wrp = None
            gdd = [
                bigp.tile([P, 68 * S], DT.int32, tag=f"gdd{i}", name=f"gdd{i}")
                for i in range(2)
            ]
            gdh = [
                bigp.tile([P, 2 * 16 * S], DT.int32, tag=f"gdh{i}", name=f"gdh{i}")
                for i in range(3)
            ]
            hcor = bigp.tile([P, 8 * S], DT.int32, tag="hcor", name="hcor")
            osb = bigp.tile([P, S, 32], DT.float32, tag="osb", name="osb")
            bli_d = bigp.tile([P, S, 3, 8], DT.int32, tag="bli_d", name="bli_d")
            bli_h = bigp.tile([P, S, 3, 8], DT.int32, tag="bli_h", name="bli_h")

            nc.vector.memset(wt[:], 0)

            with tc.For_i(
                0,
                slots_total,
                S,
                hint_engines=(mybir.EngineType.DVE, mybir.EngineType.Pool),
            ) as st:
                nc.sync.dma_start(x_t[:], x_in[:, bass.ds(st, S), :])
                w_bf = dblp.tile([P, S, 48], DT.bfloat16, tag="w_bf", name="w_bf")
                hm = dblp.tile([P, 64 * S], DT.int32, tag="hm", name="hm")
                koffs = dblp.tile([P, S, 8], DT.int32, tag="koffs", name="koffs")
                wrp = dblp.tile([P, 8 * nW], DT.int16, tag="wrp", name="wrp")

                # ================= phase 1: voxel coords + weights ============
                xc = workp.tile([P, S, 3], DT.float32, tag="xc", name="xc")
                nc.vector.tensor_scalar(out=xc[:], in0=x_t[:], op0=AL.max, scalar1=-1.0, op1=AL.min, scalar2=1.0)

                for half, lv0 in ((0, 0), (1, 8)):
                    bli = bli_d if half == 0 else bli_h
                    tf = workp.tile([P, S, 3, 8], DT.float32, tag="tf", name="tf")
                    fi = workp.tile([P, S, 3, 8], DT.int32, tag="fi", name="fi")
                    ff = workp.tile([P, S, 3, 8], DT.float32, tag="ff", name="ff")
                    blf = workp.tile([P, S, 3, 8], DT.float32, tag="blf", name="blf")
                    su = workp.tile([P, S, 3, 8], DT.float32, tag="su", name="su")
                    xb = apd(xc, 0, [list(xc.ap[0]), [3, S], [1, 3], [0, 8]])
                    xbu = apd(x_t, 0, [list(x_t.ap[0]), [3, S], [1, 3], [0, 8]])
                    nc.vector.tensor_scalar(out=tf[:], in0=xb, op0=AL.add, scalar1=1.0, scalar2=None)
                    nc.vector.tensor_tensor(out=tf[:], in0=tf[:], in1=cvb4(lv0, S, 3), op=AL.mult)
                    nc.vector.tensor_copy(out=fi[:], in_=tf[:])
                    nc.vector.tensor_copy(out=ff[:], in_=fi[:])
                    nc.vector.tensor_tensor(out=blf[:], in0=ff[:], in1=tf[:], op=AL.is_gt)
                    nc.vector.tensor_tensor(out=blf[:], in0=ff[:], in1=blf[:], op=AL.subtract)
                    nc.vector.tensor_scalar(out=blf[:], in0=blf[:], op0=AL.max, scalar1=0.0, scalar2=None)
                    nc.vector.tensor_tensor(out=blf[:], in0=blf[:], in1=cvb4(16 + lv0, S, 3), op=AL.min)
                    nc.vector.tensor_copy(out=bli[:], in_=blf[:])
                    nc.vector.tensor_tensor(out=su[:], in0=blf[:], in1=cvb4(32 + lv0, S, 3), op=AL.mult)
                    nc.vector.tensor_scalar(out=su[:], in0=su[:], op0=AL.add, scalar1=-1.0, scalar2=None)
                    nc.vector.tensor_tensor(out=su[:], in0=xbu, in1=su[:], op=AL.subtract)
                    nc.vector.tensor_tensor(out=su[:], in0=su[:], in1=cvb4(lv0, S, 3), op=AL.mult)
                    wdst = apd(w_bf, 3 * lv0, [list(w_bf.ap[0]), [48, S], [1, 3], [3, 8]])
                    nc.vector.tensor_copy(out=wdst, in_=su[:])

                # ================= dense slot ids =============================
                i_ = bli_d[:, :, 0, :]
                j_ = bli_d[:, :, 1, :]
                k_ = bli_d[:, :, 2, :]
                kq = workp.tile([P, S, 8], DT.int32, tag="kq", name="kq")
                sid = workp.tile([P, S, 8], DT.int32, tag="sid", name="sid")
                t1 = workp.tile([P, S, 8], DT.int32, tag="t1d", name="t1d")
                nc.vector.tensor_copy(out=kq[:], in_=k_)
                for l in range(4, 8):
                    q = LEVEL_Q[l].bit_length() - 1
                    nc.vector.tensor_scalar(out=kq[:, :, l], in0=k_[:, :, l], op0=AL.logical_shift_right, scalar1=q, scalar2=None)
                # koff = k - kq*Q  (only levels 4-7 used)
                nc.vector.tensor_tensor(out=koffs[:], in0=kq[:], in1=apd(cv, 72, [list(cv.ap[0]), [0, S], [1, 8]]), op=AL.mult)
                nc.vector.tensor_tensor(out=koffs[:], in0=k_, in1=koffs[:], op=AL.subtract)
                # sid = i*A + j*B + kq
                nc.vector.tensor_tensor(out=t1[:], in0=i_, in1=apd(cv, 48, [list(cv.ap[0]), [0, S], [1, 8]]), op=AL.mult)
                nc.vector.tensor_tensor(out=sid[:], in0=j_, in1=apd(cv, 56, [list(cv.ap[0]), [0, S], [1, 8]]), op=AL.mult)
                nc.vector.tensor_tensor(out=sid[:], in0=sid[:], in1=t1[:], op=AL.add)
                nc.vector.tensor_tensor(out=sid[:], in0=sid[:], in1=kq[:], op=AL.add)
                # write dense classes: idxb[:, l*S + x] = sid[:, x, l]
                nc.vector.tensor_copy(
                    out=apd(idxb, 0, [list(idxb.ap[0]), [1, S], [S, 8]]),
                    in_=sid[:],
                )

                # ================= hash slot ids ==============================
                ih = bli_h[:, :, 0, :]
                jh = bli_h[:, :, 1, :]
                kh = bli_h[:, :, 2, :]
                mt1 = workp.tile([P, S, 8], DT.int32, tag="mt1", name="mt1")
                mt2 = workp.tile([P, S, 8], DT.int32, tag="mt2", name="mt2")
                mt3 = workp.tile([P, S, 8], DT.int32, tag="mt3", name="mt3")

                def ts(o, i, op, s):
                    nc.vector.tensor_scalar(out=o, in0=i, op0=op, scalar1=s, scalar2=None)

                def tt(o, a, b, op):
                    nc.vector.tensor_tensor(out=o, in0=a, in1=b, op=op)

                def mul32(dst, src, prime):
                    Hp, Lp = (prime >> 16) & 0xFFFF, prime & 0xFFFF
                    Hs = Hp - 32768 if Hp >= 32768 else Hp
                    ts(mt1[:], src, AL.mult, Lp)
                    ts(mt2[:], src, AL.mult, Hs)
                    if Hp >= 32768:
                        ts(mt3[:], src, AL.logical_shift_left, 15)
                        ts(mt3[:], mt3[:], AL.bitwise_and, 0xFFFF)
                        ts(mt2[:], mt2[:], AL.bitwise_and, 0xFFFF)
                        tt(mt2[:], mt2[:], mt3[:], AL.add)
                    ts(mt2[:], mt2[:], AL.bitwise_and, 0xFFFF)
                    ts(mt3[:], mt1[:], AL.logical_shift_right, 16)
                    tt(mt2[:], mt2[:], mt3[:], AL.add)
                    ts(mt2[:], mt2[:], AL.bitwise_and, 0xFFFF)
                    ts(mt2[:], mt2[:], AL.logical_shift_left, 16)
                    ts(mt1[:], mt1[:], AL.bitwise_and, 0xFFFF)
                    tt(dst, mt2[:], mt1[:], AL.bitwise_or)

                def add32(dst, src, const):
                    cl, ch = const & 0xFFFF, (const >> 16) & 0xFFFF
                    ts(mt1[:], src, AL.bitwise_and, 0xFFFF)
                    ts(mt1[:], mt1[:], AL.add, cl)
                    ts(mt2[:], src, AL.logical_shift_right, 16)
                    ts(mt2[:], mt2[:], AL.bitwise_and, 0xFFFF)
                    ts(mt2[:], mt2[:], AL.add, ch)
                    ts(mt3[:], mt1[:], AL.logical_shift_right, 16)
                    tt(mt2[:], mt2[:], mt3[:], AL.add)
                    ts(mt2[:], mt2[:], AL.bitwise_and, 0xFFFF)
                    ts(mt2[:], mt2[:], AL.logical_shift_left, 16)
                    ts(mt1[:], mt1[:], AL.bitwise_and, 0xFFFF)
                    tt(dst, mt2[:], mt1[:], AL.bitwise_or)

                jp0 = workp.tile([P, S, 8], DT.int32, tag="jp0", name="jp0")
                jp1 = workp.tile([P, S, 8], DT.int32, tag="jp1", name="jp1")
                kp0 = workp.tile([P, S, 8], DT.int32, tag="kp0", name="kp0")
                kp1 = workp.tile([P, S, 8], DT.int32, tag="kp1", name="kp1")
                ii1 = workp.tile([P, S, 8], DT.int32, tag="ii1", name="ii1")
                rr = workp.tile([P, S, 8], DT.int32, tag="rr", name="rr")
                rr2 = workp.tile([P, S, 8], DT.int32, tag="rr2", name="rr2")
                mul32(jp0[:], jh, PRIMES[1])
                add32(jp1[:], jp0[:], PRIMES[1])
                mul32(kp0[:], kh, PRIMES[2])
                add32(kp1[:], kp0[:], PRIMES[2])
                ts(ii1[:], ih, AL.add, 1)
                for m in range(8):
                    di, dj, dk = (m >> 2) & 1, (m >> 1) & 1, m & 1
                    tt(rr[:], ii1[:] if di else ih, (jp1 if dj else jp0)[:], AL.bitwise_xor)
                    tt(rr[:], rr[:], (kp1 if dk else kp0)[:], AL.bitwise_xor)
                    ts(rr[:], rr[:], AL.bitwise_and, T - 1)
                    # idxb[:, (8 + (l-8)*8 + m)*S + x] = rr[:, x, l] >> 4
                    ts(rr2[:], rr[:], AL.logical_shift_right, 4)
                    nc.vector.tensor_copy(
                        out=apd(idxb, (8 + m) * S, [list(idxb.ap[0]), [1, S], [8 * S, 8]]),
                        in_=rr2[:],
                    )
                    # hm[:, (l-8)*8S + m*S + x] = rr & 15
                    ts(rr2[:], rr[:], AL.bitwise_and, 15)
                    nc.vector.tensor_copy(
                        out=apd(hm, m * S, [list(hm.ap[0]), [1, S], [8 * S, 8]]),
                        in_=rr2[:],
                    )

                # ============== idx transpose to wrapped-16 + replicate =======
                nc.sync.dma_start(scr[:], idxb[:])
                # wt[q, h*nW + col] = scr[16(h+h0)+q, col]  (q<16), four chunks
                for h0 in (0, 2, 4, 6):
                    nc.sync.dma_start(
                        wt[0:16, :],
                        apd(scr, h0 * 16 * nW, [[nW, 16], [16 * nW, 2], [1, nW]]),
                    )
                    # wrp[q, col*8 + h0+h] = wt[q, h*nW + col]
                    nc.vector.tensor_copy(
                        out=apd(wrp, h0, [list(wrp.ap[0]), [8, nW], [1, 2]]),
                        in_=apd(wt, 0, [list(wt.ap[0]), [1, nW], [nW, 2]]),
                    )
                for g in range(1, 8):
                    nc.sync.dma_start(wrp[16 * g : 16 * g + 16, :], wrp[0:16, :])

                # ================= gathers ====================================
                def lerp(dst, lo, hi, w):
                    nc.vector.tensor_tensor(out=dst, in0=hi, in1=lo, op=AL.subtract)
                    nc.vector.tensor_tensor(out=dst, in0=dst, in1=w, op=AL.mult)
                    nc.vector.tensor_tensor(out=dst, in0=dst, in1=lo, op=AL.add)

                def wof(l, d, reps, minor):
                    # weight w_bf[:, x, 3l+d] broadcast: minor=True -> [S, reps]
                    # (x outer), else [reps, S]
                    wf = workp.tile([P, S], DT.float32, tag=f"wf{d}", name=f"wf{d}")
                    nc.vector.tensor_copy(out=wf[:], in_=apd(w_bf, 3 * l + d, [list(w_bf.ap[0]), [48, S]]))
                    if minor:
                        return apd(wf, 0, [list(wf.ap[0]), [1, S], [0, reps]])
                    return apd(wf, 0, [list(wf.ap[0]), [0, reps], [1, S]])

                def unpack(src_ap, n, tag):
                    f0 = workp.tile([P, n], DT.int32, tag=f"u0{tag}", name=f"u0{tag}")
                    f1 = workp.tile([P, n], DT.int32, tag=f"u1{tag}", name=f"u1{tag}")
                    nc.vector.tensor_scalar(out=f0[:], in0=src_ap, op0=AL.logical_shift_left, scalar1=16, scalar2=None)
                    nc.vector.tensor_scalar(out=f1[:], in0=src_ap, op0=AL.bitwise_and, scalar1=_i32(0xFFFF0000), scalar2=None)
                    return f0, f1

                def f32v(t, dims, off=0):
                    return apd(t, off, [list(t.ap[0])] + [list(d) for d in dims]).bitcast(DT.float32)

                # ---- dense levels ----
                def ktree(l, gd, el, ngrp, q, Q, tag):
                    # select k_off window over bits q-1..0; gd lanes
                    # [x*el + g*(Q+1) + t]; returns tile [P, S, ngrp, 2]
                    cur = None
                    for b in range(q - 1, -1, -1):
                        wnew = 2**b + 1 if b > 0 else 2
                        half = 2**b
                        nxt = workp.tile([P, S, ngrp, wnew], DT.int32, tag=f"dt{tag}{b}", name=f"dt{tag}{b}")
                        if cur is None:
                            lo = apd(gd, 0, [list(gd.ap[0]), [el, S], [Q + 1, ngrp], [1, wnew]])
                            hi = apd(gd, half, [list(gd.ap[0]), [el, S], [Q + 1, ngrp], [1, wnew]])
                        else:
                            lo = apd(cur, 0, [list(cur.ap[0]), [cur.ap[1][0], S], [cur.ap[2][0], ngrp], [1, wnew]])
                            hi = apd(cur, half, [list(cur.ap[0]), [cur.ap[1][0], S], [cur.ap[2][0], ngrp], [1, wnew]])
                        mb = workp.tile([P, S], DT.int32, tag=f"dm{b}", name=f"dm{b}")
                        nc.vector.tensor_scalar(out=mb[:], in0=koffs[:, :, l], op0=AL.logical_shift_right, scalar1=b, op1=AL.bitwise_and, scalar2=1)
                        nc.vector.tensor_scalar(out=mb[:], in0=mb[:], op0=AL.mult, scalar1=-1, scalar2=None)
                        mbb = apd(mb, 0, [list(mb.ap[0]), [1, S], [0, ngrp], [0, wnew]])
                        nc.vector.tensor_tensor(out=nxt[:], in0=lo, in1=hi, op=AL.bitwise_xor)
                        nc.vector.tensor_tensor(out=nxt[:], in0=nxt[:], in1=mbb, op=AL.bitwise_and)
                        nc.vector.tensor_tensor(out=nxt[:], in0=nxt[:], in1=lo, op=AL.bitwise_xor)
                        cur = nxt
                    return cur

                for l in DENSE_LV:
                    el = ELEM[l]
                    Q = LEVEL_Q[l]
                    q = Q.bit_length() - 1
                    gd = gdd[l % 2]
                    if l == 7:
                        baseA = tabx[SLOT_OFF[7] : SLOT_OFF[7] + N_SLOTS[7], 0:el]
                        baseB = tabx[SLOT_OFF7B : SLOT_OFF7B + N_SLOTS[7], 0:el]
                        gdA, gdB = gdd[0], gdd[1]
                        for gdX, baseX in ((gdA, baseA), (gdB, baseB)):
                            gview = apd(gdX, 0, [list(gdX.ap[0]), [el, S], [1, el]])
                            dma_gather_raw(
                                nc.gpsimd, gview, baseX,
                                wrp[:, l * 8 * S : (l + 1) * 8 * S], S * P, el,
                            )
                        curA = ktree(l, gdA, el, 2, q, Q, "2a")
                        curB = ktree(l, gdB, el, 2, q, Q, "2b")
                        comb = workp.tile([P, S, 8], DT.int32, tag="comb7", name="comb7")
                        nc.vector.tensor_copy(
                            out=apd(comb, 0, [list(comb.ap[0]), [8, S], [1, 4]]),
                            in_=apd(curA, 0, [list(curA.ap[0]), [4, S], [1, 4]]),
                        )
                        nc.vector.tensor_copy(
                            out=apd(comb, 4, [list(comb.ap[0]), [8, S], [1, 4]]),
                            in_=apd(curB, 0, [list(curB.ap[0]), [4, S], [1, 4]]),
                        )
                        csrc = apd(comb, 0, [list(comb.ap[0]), [1, 8 * S]])
                    else:
                        base = tabx[SLOT_OFF[l] : SLOT_OFF[l] + N_SLOTS[l], 0:el]
                        gview = apd(gd, 0, [list(gd.ap[0]), [el, S], [1, el]])
                        dma_gather_raw(
                            nc.gpsimd, gview, base,
                            wrp[:, l * 8 * S : (l + 1) * 8 * S], S * P, el,
                        )
                        if Q == 1:
                            csrc = apd(gd, 0, [list(gd.ap[0]), [1, 8 * S]])
                        else:
                            cur = ktree(l, gd, el, 4, q, Q, "4g")
                            csrc = apd(cur, 0, [list(cur.ap[0]), [1, 8 * S]])
                    # corners in x-major layout: lane x*8 + m
                    e0, e1 = unpack(csrc, 8 * S, "d")
                    wx = wof(l, 0, 4, minor=True)
                    xt0 = workp.tile([P, S, 4], DT.float32, tag="xt0", name="xt0")
                    xt1 = workp.tile([P, S, 4], DT.float32, tag="xt1", name="xt1")
                    lerp(xt0[:], f32v(e0, [[8, S], [1, 4]]), f32v(e0, [[8, S], [1, 4]], 4), wx)
                    lerp(xt1[:], f32v(e1, [[8, S], [1, 4]]), f32v(e1, [[8, S], [1, 4]], 4), wx)
                    wy = wof(l, 1, 2, minor=True)
                    yt0 = workp.tile([P, S, 2], DT.float32, tag="yt0", name="yt0")
                    yt1 = workp.tile([P, S, 2], DT.float32, tag="yt1", name="yt1")
                    lerp(yt0[:], apd(xt0, 0, [list(xt0.ap[0]), [4, S], [1, 2]]), apd(xt0, 2, [list(xt0.ap[0]), [4, S], [1, 2]]), wy)
                    lerp(yt1[:], apd(xt1, 0, [list(xt1.ap[0]), [4, S], [1, 2]]), apd(xt1, 2, [list(xt1.ap[0]), [4, S], [1, 2]]), wy)
                    wz = wof(l, 2, 1, minor=True)
                    o0 = apd(osb, 2 * l, [list(osb.ap[0]), [32, S]])
                    o1 = apd(osb, 2 * l + 1, [list(osb.ap[0]), [32, S]])
                    lerp(o0, apd(yt0, 0, [list(yt0.ap[0]), [2, S]]), apd(yt0, 1, [list(yt0.ap[0]), [2, S]]), wz)
                    lerp(o1, apd(yt1, 0, [list(yt1.ap[0]), [2, S]]), apd(yt1, 1, [list(yt1.ap[0]), [2, S]]), wz)

                # ---- hash levels ----
                for l in HASH_LV:
                    lh = l - 8
                    base = tabx[SLOT_OFF[l] : SLOT_OFF[l] + N_SLOTS[l], 0:16]
                    cls0 = 8 + lh * 8
                    for h2 in range(4):  # 2 classes per 8192-idx instruction
                        gd = gdh[(4 * lh + h2) % 3]
                        gview = apd(gd, 0, [list(gd.ap[0]), [16, 2 * S], [1, 16]])
                        dma_gather_raw(
                            nc.gpsimd, gview, base,
                            wrp[:, (cls0 + 2 * h2) * 8 * S : (cls0 + 2 * h2 + 2) * 8 * S],
                            2 * S * P, 16,
                        )
                        # extraction tree over 16 lanes for these 2 classes
                        hmv = apd(hm, lh * 8 * S + h2 * 2 * S, [list(hm.ap[0]), [1, 2 * S]])
                        cur = None
                        for b in range(3, -1, -1):
                            half = 2**b
                            nxt = workp.tile([P, 2 * S, half], DT.int32, tag=f"ht{b}", name=f"ht{b}")
                            if cur is None:
                                lo = apd(gd, 0, [list(gd.ap[0]), [16, 2 * S], [1, half]])
                                hi = apd(gd, half, [list(gd.ap[0]), [16, 2 * S], [1, half]])
                            else:
                                lo = cur[:, :, 0:half]
                                hi = cur[:, :, half : 2 * half]
                            mb = workp.tile([P, 2 * S], DT.int32, tag=f"hmk{b}", name=f"hmk{b}")
                            nc.vector.tensor_scalar(out=mb[:], in0=hmv, op0=AL.logical_shift_right, scalar1=b, op1=AL.bitwise_and, scalar2=1)
                            nc.vector.tensor_scalar(out=mb[:], in0=mb[:], op0=AL.mult, scalar1=-1, scalar2=None)
                            mbb = apd(mb, 0, [list(mb.ap[0]), [1, 2 * S], [0, half]])
                            nc.vector.tensor_tensor(out=nxt[:], in0=lo, in1=hi, op=AL.bitwise_xor)
                            nc.vector.tensor_tensor(out=nxt[:], in0=nxt[:], in1=mbb, op=AL.bitwise_and)
                            nc.vector.tensor_tensor(out=nxt[:], in0=nxt[:], in1=lo, op=AL.bitwise_xor)
                            cur = nxt
                        nc.vector.tensor_copy(out=hcor[:, h2 * 2 * S : (h2 + 1) * 2 * S], in_=cur[:, :, 0])
                    # hcor: [P, 8S] corner values, class-major (c*S + x)
                    e0, e1 = unpack(hcor[:, :], 8 * S, "h")
                    wx = wof(l, 0, 4, minor=False)
                    x0 = workp.tile([P, 4 * S], DT.float32, tag="hx0", name="hx0")
                    x1 = workp.tile([P, 4 * S], DT.float32, tag="hx1", name="hx1")
                    lerp(x0[:], f32v(e0, [[1, 4 * S]]), f32v(e0, [[1, 4 * S]], 4 * S), wx)
                    lerp(x1[:], f32v(e1, [[1, 4 * S]]), f32v(e1, [[1, 4 * S]], 4 * S), wx)
                    wy = wof(l, 1, 2, minor=False)
                    y0 = workp.tile([P, 2 * S], DT.float32, tag="hy0", name="hy0")
                    y1 = workp.tile([P, 2 * S], DT.float32, tag="hy1", name="hy1")
                    lerp(y0[:], x0[:, 0 : 2 * S], x0[:, 2 * S : 4 * S], wy)
                    lerp(y1[:], x1[:, 0 : 2 * S], x1[:, 2 * S : 4 * S], wy)
                    wz = wof(l, 2, 1, minor=False)
                    o0 = apd(osb, 2 * l, [list(osb.ap[0]), [32, S]])
                    o1 = apd(osb, 2 * l + 1, [list(osb.ap[0]), [32, S]])
                    lerp(o0, y0[:, 0:S], y0[:, S : 2 * S], wz)
                    lerp(o1, y1[:, 0:S], y1[:, S : 2 * S], wz)

                nc.gpsimd.dma_start(out[:, bass.ds(st, S), :], osb[:])
    nc.compile()
    return nc


# ---------------- host side ----------------

def _pack_bf16(t):
    u = np.ascontiguousarray(t.astype(np.float32)).view(np.uint32).astype(np.uint64)
    r = ((u + 0x7FFF + ((u >> 16) & 1)) >> 16).astype(np.uint32)
    return r[:, 0] | (r[:, 1] << np.uint32(16))


def build_tabx(tables):
    pk = _pack_bf16(tables)
    tabx = np.zeros((TOT_SLOTS, 64), dtype=np.uint32)
    for l in DENSE_LV:
        r, Q, M = RES[l], LEVEL_Q[l], LEVEL_M[l]
        o = SLOT_OFF[l]
        rp1 = r + 1
        seg = pk[OFFS[l] : OFFS[l + 1]]
        if Q == 1:
            ii, jj, kk = np.meshgrid(np.arange(r), np.arange(r), np.arange(r), indexing="ij")
            # reference indexes the (r+1)^3 table with strides r^2, r, 1
            n0 = ((ii * r + jj) * r + kk).ravel()
            lane = 0
            for di in (0, 1):
                for dj in (0, 1):
                    for dk in (0, 1):
                        tabx[o : o + r * r * r, lane] = seg[n0 + di * r * r + dj * r + dk]
                        lane += 1
        else:
            ii, jj, mm = np.meshgrid(np.arange(r), np.arange(r), np.arange(M), indexing="ij")
            if l == 7:
                for di in (0, 1):
                    oo = o if di == 0 else SLOT_OFF7B
                    lane = 0
                    for dj in (0, 1):
                        for t in range(Q + 1):
                            kidx = np.minimum(mm * Q + t, r)
                            row = ((ii + di) * r + (jj + dj)) * r + kidx
                            src = np.where(mm * Q + t <= r, seg[row], 0)
                            tabx[oo : oo + r * r * M, lane] = src.ravel()
                            lane += 1
            else:
                lane = 0
                for di in (0, 1):
                    for dj in (0, 1):
                        for t in range(Q + 1):
                            kidx = np.minimum(mm * Q + t, r)
                            row = ((ii + di) * r + (jj + dj)) * r + kidx
                            src = np.where(mm * Q + t <= r, seg[row], 0)
                            tabx[o : o + r * r * M, lane] = src.ravel()
                            lane += 1
    for l in HASH_LV:
        o, ns = SLOT_OFF[l], N_SLOTS[l]
        seg = pk[OFFS[l] : OFFS[l + 1]]
        tabx[o : o + ns, 0:16] = seg.reshape(ns, 16)
    return tabx.view(np.int32)


def build_cvec():
    cv = np.zeros(128, dtype=np.float32)
    for l in range(N_LEVELS):
        grid = np.float32(2.0) / np.float32(RES[l])
        cv[l] = np.float32(1.0) / grid
        cv[16 + l] = np.float32(RES[l] - 1)
        cv[32 + l] = grid
    for l in DENSE_LV:
        r, Q, M = RES[l], LEVEL_Q[l], LEVEL_M[l]
        cv[48 + l] = np.float32(r * M)
        cv[56 + l] = np.float32(M)
        cv[64 + l] = np.float32(1.0 / Q)
        cv[72 + l] = np.float32(Q)
    return cv.reshape(1, 128)


_NC_CACHE = {}
TRACE = False
LAST_NS = None


def _get_nc(slots, S):
    key = (slots, S)
    if key not in _NC_CACHE:
        _NC_CACHE[key] = build_kernel(slots, S)
    return _NC_CACHE[key]


def kernel(x: np.ndarray, tables: np.ndarray) -> np.ndarray:
    global LAST_NS
    from concourse.bass_utils import run_bass_kernel_spmd

    B = x.shape[0]
    per_core = B // N_CORES
    slots = per_core // P
    S = min(32, slots)
    nc = _get_nc(slots, S)
    tabx = build_tabx(tables)
    cv = build_cvec()
    in_maps = []
    for c in range(N_CORES):
        xs = np.ascontiguousarray(
            x[c * per_core : (c + 1) * per_core].reshape(P, slots, 3)
        ).astype(np.float32)
        in_maps.append({"x": xs, "tabx": tabx, "cvec": cv})
    kw = {"trace": True} if TRACE else {}
    res = run_bass_kernel_spmd(nc, in_maps, core_ids=list(range(N_CORES)), **kw)
    LAST_NS = res.exec_time_ns
    outs = [res.results[c]["out"].reshape(per_core, 32) for c in range(N_CORES)]
    return np.concatenate(outs, axis=0).astype(np.float32)


# revision 16
# speedup vs baseline: 1.1155x; 1.1155x over previous
"""HashEmbedder3D Trainium2 kernel v6.

Key changes vs v2 baseline:
- dma_gather with single_packet=False + indices replicated across all 8
  16-partition groups allows 8192-idx instructions (994ns fixed cost
  amortized 8x).
- Dense levels 0-3: slot-per-voxel-base block tables (one 32B descriptor
  fetches all 8 corners; no select trees). Levels 4-7: Q-packed slots
  (one descriptor + small k-offset select tree).
- Hash levels 8-15: one 64B slot read per corner (8 classes), extraction
  via 4-round select tree, gathers batched 2 classes per instruction.
- Index wrap/transpose done by DRAM-bounce DMA + DVE interleave + SBUF
  broadcast DMAs; Pool engine only runs SWDGE gathers.
"""
import math
import sys

import numpy as np

sys.path.insert(0, "/opt/trn_rl_repo")

from concourse import bacc, bass, mybir
import concourse.tile as tile

N_LEVELS = 16
F = 2
LOG2_T = 19
T = 1 << LOG2_T
BASE, FINEST = 16, 512
B_GROWTH = float(np.exp((np.log(np.float32(FINEST)) - np.log(np.float32(BASE))) / np.float32(N_LEVELS - 1)))
RES = [math.floor(BASE * B_GROWTH**i) for i in range(N_LEVELS)]
SIZES = [(r + 1) ** 3 if r**3 < T else T for r in RES]
OFFS = np.concatenate([[0], np.cumsum(SIZES)]).tolist()
TOTAL_ROWS = OFFS[-1]
PRIMES = [1, 2654435761, 805459861]
N_POINTS = 1048576
N_CORES = 8
P = 128

DT = mybir.dt
AL = mybir.AluOpType

DENSE_LV = list(range(8))
HASH_LV = list(range(8, 16))

# dense level slot geometry: levels 0-3 slot-per-base, 4-7 Q-packed
LEVEL_Q = {0: 1, 1: 1, 2: 1, 3: 1, 4: 2, 5: 4, 6: 8, 7: 16}
LEVEL_M = {}
N_SLOTS, ELEM = {}, {}
for l in DENSE_LV:
    r, Q = RES[l], LEVEL_Q[l]
    M = -(-r // Q)
    LEVEL_M[l] = M
    N_SLOTS[l] = r * r * M
    ELEM[l] = 8 if Q == 1 else 4 * (Q + 1)
ELEM[7] = 2 * (LEVEL_Q[7] + 1)  # level 7 split in two di-halves of 2x17 lanes
for l in HASH_LV:
    N_SLOTS[l], ELEM[l] = T // 16, 16
SLOT_OFF = {}
_a = 0
for l in range(N_LEVELS):
    SLOT_OFF[l] = _a
    _a += N_SLOTS[l]
SLOT_OFF7B = _a  # second (di=1) half of level 7
_a += N_SLOTS[7]
TOT_SLOTS = _a
assert all(N_SLOTS[l] <= 32768 for l in range(N_LEVELS)), N_SLOTS

# class layout: dense levels are classes 0-7; hash level l corner m is
# class 8 + (l-8)*8 + m. Each class contributes S indices per tile.
NCC = 8 + 8 * 8  # 72


def _i32(v):
    return int(np.int32(np.uint32(v)))


MAX_GIDX = 8192


def dma_gather_raw(eng, out_ap, in_ap, idxs_ap, num_idxs, elem_size, elem_step=64):
    stride_bytes = elem_step * 4
    assert stride_bytes % 256 == 0
    _in_ap = eng.lower_ap_dma(in_ap, for_custom_bir_dma=True)
    _idxs_ap = eng.lower_ap(idxs_ap)
    _out_ap = eng.lower_ap(out_ap)
    return eng.add_instruction(
        mybir.InstDMAGatherAnt(
            name=eng.bass.get_next_instruction_name(),
            ins=[*_in_ap, _idxs_ap, eng.lower_val_access(eng.to_reg(num_idxs))],
            outs=[_out_ap],
            transpose=False,
            num_idxs=num_idxs,
            elem_size=elem_size,
            stride_bytes_256=stride_bytes // 256,
            gen_mode=0,
            single_packet=num_idxs <= 1024,
            queue_num=0,
            sbuf_tokens_per_rank=0,
            sbuf_free_dim_per_rank=0,
            sbuf_free_dim_pad_per_rank=0,
            sbuf_byte_offset=0,
        )
    )


def apd(tap, off, dims):
    return bass.AP(tap.tensor, tap.offset + off, [list(d) for d in dims])


def build_kernel(slots_total, S):
    n_outer = slots_total // S
    assert n_outer * S == slots_total

    nc = bacc.Bacc(None, target_bir_lowering=False, debug=False)
    x_in = nc.dram_tensor("x", [P, slots_total, 3], DT.float32, kind="ExternalInput")
    tabx = nc.dram_tensor("tabx", [TOT_SLOTS, 64], DT.int32, kind="ExternalInput")
    cvec_in = nc.dram_tensor("cvec", [1, 128], DT.float32, kind="ExternalInput")
    out = nc.dram_tensor("out", [P, slots_total, 32], DT.float32, kind="ExternalOutput")

    nW = NCC * S  # idx ints per partition per tile

    with tile.TileContext(nc) as tc:
        with (
            tc.tile_pool(name="big", bufs=1) as bigp,
            tc.tile_pool(name="dbl", bufs=1) as dblp,
            tc.tile_pool(name="work", bufs=1) as workp,
        ):
            cv = bigp.tile([P, 128], DT.float32, tag="cv", name="cv")
            nc.sync.dma_start(cv[:], apd(cvec_in[:], 0, [[0, P], [1, 128]]))

            def cvb3(col, n, w=8):
                # [P, n, w] view of per-level const at cv[col:col+w]
                return apd(cv, col, [list(cv.ap[0]), [0, n], [1, w]])

            def cvb4(col, a, b, w=8):
                return apd(cv, col, [list(cv.ap[0]), [0, a], [0, b], [1, w]])

            x_t = bigp.tile([P, S, 3], DT.float32, tag="x_t", name="x_t")
            w_bf = None
            idxb = bigp.tile([P, nW], DT.int16, tag="idxb", name="idxb")
            hm = None
            koffs = None
            scr = bigp.tile([P, nW], DT.int16, tag="scr", name="scr", space="DRAM")
            wt = bigp.tile([P, 2 * nW], DT.int16, tag="wt", name="wt")
            wrp = None
            gdd = [
                bigp.tile([P, 68 * S], DT.int32, tag=f"gdd{i}", name=f"gdd{i}")
                for i in range(2)
            ]
            gdh = [
                bigp.tile([P, 2 * 16 * S], DT.int32, tag=f"gdh{i}", name=f"gdh{i}")
                for i in range(3)
            ]
            hcor = bigp.tile([P, 8 * S], DT.int32, tag="hcor", name="hcor")
            osb = bigp.tile([P, S, 32], DT.float32, tag="osb", name="osb")
            bli_d = bigp.tile([P, S, 3, 8], DT.int32, tag="bli_d", name="bli_d")
            bli_h = bigp.tile([P, S, 3, 8], DT.int32, tag="bli_h", name="bli_h")

            nc.vector.memset(wt[:], 0)
            _wrp0 = dblp.tile([P, 8 * nW], DT.int16, tag="wrp", name="wrp_init")
            nc.vector.memset(_wrp0[:], 0)

            with tc.For_i(
                0,
                slots_total,
                S,
                hint_engines=(mybir.EngineType.DVE, mybir.EngineType.Pool),
            ) as st:
                nc.sync.dma_start(x_t[:], x_in[:, bass.ds(st, S), :])
                w_bf = dblp.tile([P, S, 48], DT.bfloat16, tag="w_bf", name="w_bf")
                hm = dblp.tile([P, 64 * S], DT.int32, tag="hm", name="hm")
                koffs = dblp.tile([P, S, 8], DT.int32, tag="koffs", name="koffs")
                wrp = dblp.tile([P, 8 * nW], DT.int16, tag="wrp", name="wrp")

                # ================= phase 1: voxel coords + weights ============
                xc = workp.tile([P, S, 3], DT.float32, tag="xc", name="xc")
                nc.vector.tensor_scalar(out=xc[:], in0=x_t[:], op0=AL.max, scalar1=-1.0, op1=AL.min, scalar2=1.0)

                for half, lv0 in ((0, 0), (1, 8)):
                    bli = bli_d if half == 0 else bli_h
                    tf = workp.tile([P, S, 3, 8], DT.float32, tag="tf", name="tf")
                    fi = workp.tile([P, S, 3, 8], DT.int32, tag="fi", name="fi")
                    ff = workp.tile([P, S, 3, 8], DT.float32, tag="ff", name="ff")
                    blf = workp.tile([P, S, 3, 8], DT.float32, tag="blf", name="blf")
                    su = workp.tile([P, S, 3, 8], DT.float32, tag="su", name="su")
                    xb = apd(xc, 0, [list(xc.ap[0]), [3, S], [1, 3], [0, 8]])
                    xbu = apd(x_t, 0, [list(x_t.ap[0]), [3, S], [1, 3], [0, 8]])
                    nc.vector.tensor_scalar(out=tf[:], in0=xb, op0=AL.add, scalar1=1.0, scalar2=None)
                    nc.vector.tensor_tensor(out=tf[:], in0=tf[:], in1=cvb4(lv0, S, 3), op=AL.mult)
                    nc.vector.tensor_copy(out=fi[:], in_=tf[:])
                    nc.vector.tensor_copy(out=ff[:], in_=fi[:])
                    nc.vector.tensor_tensor(out=blf[:], in0=ff[:], in1=tf[:], op=AL.is_gt)
                    nc.vector.tensor_tensor(out=blf[:], in0=ff[:], in1=blf[:], op=AL.subtract)
                    nc.vector.tensor_scalar(out=blf[:], in0=blf[:], op0=AL.max, scalar1=0.0, scalar2=None)
                    nc.vector.tensor_tensor(out=blf[:], in0=blf[:], in1=cvb4(16 + lv0, S, 3), op=AL.min)
                    nc.vector.tensor_copy(out=bli[:], in_=blf[:])
                    nc.vector.tensor_tensor(out=su[:], in0=blf[:], in1=cvb4(32 + lv0, S, 3), op=AL.mult)
                    nc.vector.tensor_scalar(out=su[:], in0=su[:], op0=AL.add, scalar1=-1.0, scalar2=None)
                    nc.vector.tensor_tensor(out=su[:], in0=xbu, in1=su[:], op=AL.subtract)
                    nc.vector.tensor_tensor(out=su[:], in0=su[:], in1=cvb4(lv0, S, 3), op=AL.mult)
                    wdst = apd(w_bf, 3 * lv0, [list(w_bf.ap[0]), [48, S], [1, 3], [3, 8]])
                    nc.vector.tensor_copy(out=wdst, in_=su[:])

                # ================= dense slot ids =============================
                i_ = bli_d[:, :, 0, :]
                j_ = bli_d[:, :, 1, :]
                k_ = bli_d[:, :, 2, :]
                kq = workp.tile([P, S, 8], DT.int32, tag="kq", name="kq")
                sid = workp.tile([P, S, 8], DT.int32, tag="sid", name="sid")
                t1 = workp.tile([P, S, 8], DT.int32, tag="t1d", name="t1d")
                nc.vector.tensor_copy(out=kq[:], in_=k_)
                for l in range(4, 8):
                    q = LEVEL_Q[l].bit_length() - 1
                    nc.vector.tensor_scalar(out=kq[:, :, l], in0=k_[:, :, l], op0=AL.logical_shift_right, scalar1=q, scalar2=None)
                # koff = k - kq*Q  (only levels 4-7 used)
                nc.vector.tensor_tensor(out=koffs[:], in0=kq[:], in1=apd(cv, 72, [list(cv.ap[0]), [0, S], [1, 8]]), op=AL.mult)
                nc.vector.tensor_tensor(out=koffs[:], in0=k_, in1=koffs[:], op=AL.subtract)
                # sid = i*A + j*B + kq
                nc.vector.tensor_tensor(out=t1[:], in0=i_, in1=apd(cv, 48, [list(cv.ap[0]), [0, S], [1, 8]]), op=AL.mult)
                nc.vector.tensor_tensor(out=sid[:], in0=j_, in1=apd(cv, 56, [list(cv.ap[0]), [0, S], [1, 8]]), op=AL.mult)
                nc.vector.tensor_tensor(out=sid[:], in0=sid[:], in1=t1[:], op=AL.add)
                nc.vector.tensor_tensor(out=sid[:], in0=sid[:], in1=kq[:], op=AL.add)
                # write dense classes: idxb[:, l*S + x] = sid[:, x, l]
                nc.vector.tensor_copy(
                    out=apd(idxb, 0, [list(idxb.ap[0]), [1, S], [S, 8]]),
                    in_=sid[:],
                )

                # ================= hash slot ids ==============================
                ih = bli_h[:, :, 0, :]
                jh = bli_h[:, :, 1, :]
                kh = bli_h[:, :, 2, :]
                mt1 = workp.tile([P, S, 8], DT.int32, tag="mt1", name="mt1")
                mt2 = workp.tile([P, S, 8], DT.int32, tag="mt2", name="mt2")
                mt3 = workp.tile([P, S, 8], DT.int32, tag="mt3", name="mt3")

                def ts(o, i, op, s):
                    nc.vector.tensor_scalar(out=o, in0=i, op0=op, scalar1=s, scalar2=None)

                def tt(o, a, b, op):
                    nc.vector.tensor_tensor(out=o, in0=a, in1=b, op=op)

                def mul32(dst, src, prime):
                    Hp, Lp = (prime >> 16) & 0xFFFF, prime & 0xFFFF
                    Hs = Hp - 32768 if Hp >= 32768 else Hp
                    ts(mt1[:], src, AL.mult, Lp)
                    ts(mt2[:], src, AL.mult, Hs)
                    if Hp >= 32768:
                        ts(mt3[:], src, AL.logical_shift_left, 15)
                        ts(mt3[:], mt3[:], AL.bitwise_and, 0xFFFF)
                        ts(mt2[:], mt2[:], AL.bitwise_and, 0xFFFF)
                        tt(mt2[:], mt2[:], mt3[:], AL.add)
                    ts(mt2[:], mt2[:], AL.bitwise_and, 0xFFFF)
                    ts(mt3[:], mt1[:], AL.logical_shift_right, 16)
                    tt(mt2[:], mt2[:], mt3[:], AL.add)
                    ts(mt2[:], mt2[:], AL.bitwise_and, 0xFFFF)
                    ts(mt2[:], mt2[:], AL.logical_shift_left, 16)
                    ts(mt1[:], mt1[:], AL.bitwise_and, 0xFFFF)
                    tt(dst, mt2[:], mt1[:], AL.bitwise_or)

                def add32(dst, src, const):
                    cl, ch = const & 0xFFFF, (const >> 16) & 0xFFFF
                    ts(mt1[:], src, AL.bitwise_and, 0xFFFF)
                    ts(mt1[:], mt1[:], AL.add, cl)
                    ts(mt2[:], src, AL.logical_shift_right, 16)
                    ts(mt2[:], mt2[:], AL.bitwise_and, 0xFFFF)
                    ts(mt2[:], mt2[:], AL.add, ch)
                    ts(mt3[:], mt1[:], AL.logical_shift_right, 16)
                    tt(mt2[:], mt2[:], mt3[:], AL.add)
                    ts(mt2[:], mt2[:], AL.bitwise_and, 0xFFFF)
                    ts(mt2[:], mt2[:], AL.logical_shift_left, 16)
                    ts(mt1[:], mt1[:], AL.bitwise_and, 0xFFFF)
                    tt(dst, mt2[:], mt1[:], AL.bitwise_or)

                jp0 = workp.tile([P, S, 8], DT.int32, tag="jp0", name="jp0")
                jp1 = workp.tile([P, S, 8], DT.int32, tag="jp1", name="jp1")
                kp0 = workp.tile([P, S, 8], DT.int32, tag="kp0", name="kp0")
                kp1 = workp.tile([P, S, 8], DT.int32, tag="kp1", name="kp1")
                ii1 = workp.tile([P, S, 8], DT.int32, tag="ii1", name="ii1")
                rr = workp.tile([P, S, 8], DT.int32, tag="rr", name="rr")
                rr2 = workp.tile([P, S, 8], DT.int32, tag="rr2", name="rr2")
                mul32(jp0[:], jh, PRIMES[1])
                add32(jp1[:], jp0[:], PRIMES[1])
                mul32(kp0[:], kh, PRIMES[2])
                add32(kp1[:], kp0[:], PRIMES[2])
                ts(ii1[:], ih, AL.add, 1)
                # vectorized corner ids: xab[c] = (ih|ii1) ^ (jp0|jp1), c=di*2+dj
                xab = workp.tile([P, S, 4, 8], DT.int32, tag="xab", name="xab")
                tt(xab[:, :, 0, :], ih, jp0[:], AL.bitwise_xor)
                tt(xab[:, :, 1, :], ih, jp1[:], AL.bitwise_xor)
                tt(xab[:, :, 2, :], ii1[:], jp0[:], AL.bitwise_xor)
                tt(xab[:, :, 3, :], ii1[:], jp1[:], AL.bitwise_xor)
                # rr_all[x, m, l] with m = c*2+dk
                rr_all = workp.tile([P, S, 8, 8], DT.int32, tag="rr_all", name="rr_all")
                for dk in (0, 1):
                    tt(
                        apd(rr_all, dk * 8, [list(rr_all.ap[0]), [64, S], [16, 4], [1, 8]]),
                        apd(xab, 0, [list(xab.ap[0]), [32, S], [8, 4], [1, 8]]),
                        (kp1 if dk else kp0)[:].to_broadcast([P, S, 4, 8]) if False else apd(kp1 if dk else kp0, 0, [list(kp0.ap[0]), [8, S], [0, 4], [1, 8]]),
                        AL.bitwise_xor,
                    )
                ts(rr_all[:], rr_all[:], AL.bitwise_and, T - 1)
                sh = workp.tile([P, S, 8, 8], DT.int32, tag="rrsh", name="rrsh")
                ts(sh[:], rr_all[:], AL.logical_shift_right, 4)
                nc.vector.tensor_copy(
                    out=apd(idxb, 8 * S, [list(idxb.ap[0]), [1, S], [S, 8], [8 * S, 8]]),
                    in_=sh[:],
                )
                ts(sh[:], rr_all[:], AL.bitwise_and, 15)
                nc.vector.tensor_copy(
                    out=apd(hm, 0, [list(hm.ap[0]), [1, S], [S, 8], [8 * S, 8]]),
                    in_=sh[:],
                )

                # ============== idx transpose to wrapped-16 + replicate =======
                nc.sync.dma_start(scr[:], idxb[:])
                # Only partitions 16:32 (group 1) are read by the SWDGE
                # cores (each core g reads cols == g mod 8 there). Build the
                # wrapped matrix in group 1; mirror to group 0 for CoreSim,
                # whose interpreter consumes group 0.
                for h0 in (0, 2, 4, 6):
                    nc.sync.dma_start(
                        wt[0:16, :],
                        apd(scr, h0 * 16 * nW, [[nW, 16], [16 * nW, 2], [1, nW]]),
                    )
                    # wrp[q, col*8 + h0+h] = wt[q, h*nW + col]
                    nc.vector.tensor_copy(
                        out=apd(wrp, h0, [list(wrp.ap[0]), [8, nW], [1, 2]]),
                        in_=apd(wt, 0, [list(wt.ap[0]), [1, nW], [nW, 2]]),
                    )
                nc.sync.dma_start(wrp[16:32, :], wrp[0:16, :])

                # ================= gathers ====================================
                def lerp(dst, lo, hi, w):
                    nc.vector.tensor_tensor(out=dst, in0=hi, in1=lo, op=AL.subtract)
                    nc.vector.tensor_tensor(out=dst, in0=dst, in1=w, op=AL.mult)
                    nc.vector.tensor_tensor(out=dst, in0=dst, in1=lo, op=AL.add)

                def wof(l, d, reps, minor):
                    # weight w_bf[:, x, 3l+d] broadcast directly as bf16:
                    # minor=True -> [S, reps] (x outer), else [reps, S]
                    if minor:
                        return apd(w_bf, 3 * l + d, [list(w_bf.ap[0]), [48, S], [0, reps]])
                    return apd(w_bf, 3 * l + d, [list(w_bf.ap[0]), [0, reps], [48, S]])

                def unpack(src_ap, n, tag):
                    ef = workp.tile([P, 2, n], DT.int32, tag=f"ef{tag}", name=f"ef{tag}")
                    nc.vector.tensor_scalar(out=ef[:, 0, :], in0=src_ap, op0=AL.logical_shift_left, scalar1=16, scalar2=None)
                    nc.vector.tensor_scalar(out=ef[:, 1, :], in0=src_ap, op0=AL.bitwise_and, scalar1=_i32(0xFFFF0000), scalar2=None)
                    return ef

                def f32v(t, dims, off=0):
                    return apd(t, off, [list(t.ap[0])] + [list(d) for d in dims]).bitcast(DT.float32)

                # ---- dense levels ----
                def ktree(l, gd, el, ngrp, q, Q, tag):
                    # select k_off window over bits q-1..0; gd lanes
                    # [x*el + g*(Q+1) + t]; returns tile [P, S, ngrp, 2]
                    cur = None
                    for b in range(q - 1, -1, -1):
                        wnew = 2**b + 1 if b > 0 else 2
                        half = 2**b
                        nxt = workp.tile([P, S, ngrp, wnew], DT.int32, tag=f"dt{tag}{b}", name=f"dt{tag}{b}")
                        if cur is None:
                            lo = apd(gd, 0, [list(gd.ap[0]), [el, S], [Q + 1, ngrp], [1, wnew]])
                            hi = apd(gd, half, [list(gd.ap[0]), [el, S], [Q + 1, ngrp], [1, wnew]])
                        else:
                            lo = apd(cur, 0, [list(cur.ap[0]), [cur.ap[1][0], S], [cur.ap[2][0], ngrp], [1, wnew]])
                            hi = apd(cur, half, [list(cur.ap[0]), [cur.ap[1][0], S], [cur.ap[2][0], ngrp], [1, wnew]])
                        mb = workp.tile([P, S], DT.int32, tag=f"dm{b}", name=f"dm{b}")
                        nc.vector.tensor_scalar(out=mb[:], in0=koffs[:, :, l], op0=AL.logical_shift_right, scalar1=b, op1=AL.bitwise_and, scalar2=1)
                        nc.vector.tensor_scalar(out=mb[:], in0=mb[:], op0=AL.mult, scalar1=-1, scalar2=None)
                        mbb = apd(mb, 0, [list(mb.ap[0]), [1, S], [0, ngrp], [0, wnew]])
                        nc.vector.tensor_tensor(out=nxt[:], in0=lo, in1=hi, op=AL.bitwise_xor)
                        nc.vector.tensor_tensor(out=nxt[:], in0=nxt[:], in1=mbb, op=AL.bitwise_and)
                        nc.vector.tensor_tensor(out=nxt[:], in0=nxt[:], in1=lo, op=AL.bitwise_xor)
                        cur = nxt
                    return cur

                for l in DENSE_LV:
                    el = ELEM[l]
                    Q = LEVEL_Q[l]
                    q = Q.bit_length() - 1
                    gd = gdd[l % 2]
                    if l == 7:
                        baseA = tabx[SLOT_OFF[7] : SLOT_OFF[7] + N_SLOTS[7], 0:el]
                        baseB = tabx[SLOT_OFF7B : SLOT_OFF7B + N_SLOTS[7], 0:el]
                        gdA, gdB = gdd[0], gdd[1]
                        for gdX, baseX in ((gdA, baseA), (gdB, baseB)):
                            gview = apd(gdX, 0, [list(gdX.ap[0]), [el, S], [1, el]])
                            dma_gather_raw(
                                nc.gpsimd, gview, baseX,
                                wrp[:, l * 8 * S : (l + 1) * 8 * S], S * P, el,
                            )
                        curA = ktree(l, gdA, el, 2, q, Q, "2a")
                        curB = ktree(l, gdB, el, 2, q, Q, "2b")
                        comb = workp.tile([P, S, 8], DT.int32, tag="comb7", name="comb7")
                        nc.vector.tensor_copy(
                            out=apd(comb, 0, [list(comb.ap[0]), [8, S], [1, 4]]),
                            in_=apd(curA, 0, [list(curA.ap[0]), [4, S], [1, 4]]),
                        )
                        nc.vector.tensor_copy(
                            out=apd(comb, 4, [list(comb.ap[0]), [8, S], [1, 4]]),
                            in_=apd(curB, 0, [list(curB.ap[0]), [4, S], [1, 4]]),
                        )
                        csrc = apd(comb, 0, [list(comb.ap[0]), [1, 8 * S]])
                    else:
                        base = tabx[SLOT_OFF[l] : SLOT_OFF[l] + N_SLOTS[l], 0:el]
                        gview = apd(gd, 0, [list(gd.ap[0]), [el, S], [1, el]])
                        dma_gather_raw(
                            nc.gpsimd, gview, base,
                            wrp[:, l * 8 * S : (l + 1) * 8 * S], S * P, el,
                        )
                        if Q == 1:
                            csrc = apd(gd, 0, [list(gd.ap[0]), [1, 8 * S]])
                        else:
                            cur = ktree(l, gd, el, 4, q, Q, "4g")
                            csrc = apd(cur, 0, [list(cur.ap[0]), [1, 8 * S]])
                    # corners in x-major layout: lane x*8 + m; both features fused
                    ef = unpack(csrc, 8 * S, "d")
                    def wv(d, dims):
                        return apd(w_bf, 3 * l + d, [list(w_bf.ap[0])] + dims)
                    efv = lambda off, dims: apd(ef, off, [list(ef.ap[0])] + dims).bitcast(DT.float32)
                    xt = workp.tile([P, 2, S, 4], DT.float32, tag="xtd", name="xtd")
                    lerp(xt[:], efv(0, [[8 * S, 2], [8, S], [1, 4]]), efv(4, [[8 * S, 2], [8, S], [1, 4]]), wv(0, [[0, 2], [48, S], [0, 4]]))
                    yt = workp.tile([P, 2, S, 2], DT.float32, tag="ytd", name="ytd")
                    lerp(yt[:], apd(xt, 0, [list(xt.ap[0]), [4 * S, 2], [4, S], [1, 2]]), apd(xt, 2, [list(xt.ap[0]), [4 * S, 2], [4, S], [1, 2]]), wv(1, [[0, 2], [48, S], [0, 2]]))
                    od = apd(osb, 2 * l, [list(osb.ap[0]), [1, 2], [32, S]])
                    lerp(od, apd(yt, 0, [list(yt.ap[0]), [2 * S, 2], [2, S]]), apd(yt, 1, [list(yt.ap[0]), [2 * S, 2], [2, S]]), wv(2, [[0, 2], [48, S]]))

                # ---- hash levels ----
                for l in HASH_LV:
                    lh = l - 8
                    base = tabx[SLOT_OFF[l] : SLOT_OFF[l] + N_SLOTS[l], 0:16]
                    cls0 = 8 + lh * 8

                    hmv_l = apd(hm, lh * 8 * S, [list(hm.ap[0]), [1, 8 * S]])
                    mbl = {}
                    for b in range(4):
                        mb = workp.tile([P, 8 * S], DT.int32, tag=f"hmk{b}", name=f"hmk{b}")
                        nc.vector.tensor_scalar(out=mb[:], in0=hmv_l, op0=AL.logical_shift_right, scalar1=b, op1=AL.bitwise_and, scalar2=1)
                        nc.vector.tensor_scalar(out=mb[:], in0=mb[:], op0=AL.mult, scalar1=-1, scalar2=None)
                        mbl[b] = mb

                    def hgather(h2):
                        gd = gdh[(4 * lh + h2) % 3]
                        gview = apd(gd, 0, [list(gd.ap[0]), [16, 2 * S], [1, 16]])
                        dma_gather_raw(
                            nc.gpsimd, gview, base,
                            wrp[:, (cls0 + 2 * h2) * 8 * S : (cls0 + 2 * h2 + 2) * 8 * S],
                            2 * S * P, 16,
                        )

                    def hextract(h2):
                        gd = gdh[(4 * lh + h2) % 3]
                        cur = None
                        for b in range(3, -1, -1):
                            half = 2**b
                            nxt = workp.tile([P, 2 * S, half], DT.int32, tag=f"ht{b}", name=f"ht{b}")
                            if cur is None:
                                lo = apd(gd, 0, [list(gd.ap[0]), [16, 2 * S], [1, half]])
                                hi = apd(gd, half, [list(gd.ap[0]), [16, 2 * S], [1, half]])
                            else:
                                lo = cur[:, :, 0:half]
                                hi = cur[:, :, half : 2 * half]
                            mb = mbl[b]
                            mbb = apd(mb, h2 * 2 * S, [list(mb.ap[0]), [1, 2 * S], [0, half]])
                            nc.vector.tensor_tensor(out=nxt[:], in0=lo, in1=hi, op=AL.bitwise_xor)
                            nc.vector.tensor_tensor(out=nxt[:], in0=nxt[:], in1=mbb, op=AL.bitwise_and)
                            nc.vector.tensor_tensor(out=nxt[:], in0=nxt[:], in1=lo, op=AL.bitwise_xor)
                            cur = nxt
                        nc.vector.tensor_copy(out=hcor[:, h2 * 2 * S : (h2 + 1) * 2 * S], in_=cur[:, :, 0])

                    hgather(0)
                    hgather(1)
                    hextract(0)
                    hgather(2)
                    hextract(1)
                    hgather(3)
                    hextract(2)
                    hextract(3)
                    # hcor: [P, 8S] corner values, class-major (c*S + x); fused feats
                    ef = unpack(hcor[:, :], 8 * S, "h")
                    def wvh(d, reps):
                        return apd(w_bf, 3 * l + d, [list(w_bf.ap[0]), [0, 2], [0, reps], [48, S]])
                    efvh = lambda off, n: apd(ef, off, [list(ef.ap[0]), [8 * S, 2], [1, n]]).bitcast(DT.float32)
                    xh = workp.tile([P, 2, 4 * S], DT.float32, tag="xth", name="xth")
                    lerp(xh[:], efvh(0, 4 * S), efvh(4 * S, 4 * S), wvh(0, 4))
                    yh = workp.tile([P, 2, 2 * S], DT.float32, tag="yth", name="yth")
                    lerp(yh[:], apd(xh, 0, [list(xh.ap[0]), [4 * S, 2], [1, 2 * S]]), apd(xh, 2 * S, [list(xh.ap[0]), [4 * S, 2], [1, 2 * S]]), wvh(1, 2))
                    oh = apd(osb, 2 * l, [list(osb.ap[0]), [1, 2], [32, S]])
                    lerp(oh, apd(yh, 0, [list(yh.ap[0]), [2 * S, 2], [1, S]]), apd(yh, S, [list(yh.ap[0]), [2 * S, 2], [1, S]]), wvh(2, 1))

                nc.gpsimd.dma_start(out[:, bass.ds(st, S), :], osb[:])
    nc.compile()
    return nc


# ---------------- host side ----------------

def _pack_bf16(t):
    u = np.ascontiguousarray(t.astype(np.float32)).view(np.uint32).astype(np.uint64)
    r = ((u + 0x7FFF + ((u >> 16) & 1)) >> 16).astype(np.uint32)
    return r[:, 0] | (r[:, 1] << np.uint32(16))


def build_tabx(tables):
    pk = _pack_bf16(tables)
    tabx = np.zeros((TOT_SLOTS, 64), dtype=np.uint32)
    for l in DENSE_LV:
        r, Q, M = RES[l], LEVEL_Q[l], LEVEL_M[l]
        o = SLOT_OFF[l]
        rp1 = r + 1
        seg = pk[OFFS[l] : OFFS[l + 1]]
        if Q == 1:
            ii, jj, kk = np.meshgrid(np.arange(r), np.arange(r), np.arange(r), indexing="ij")
            # reference indexes the (r+1)^3 table with strides r^2, r, 1
            n0 = ((ii * r + jj) * r + kk).ravel()
            lane = 0
            for di in (0, 1):
                for dj in (0, 1):
                    for dk in (0, 1):
                        tabx[o : o + r * r * r, lane] = seg[n0 + di * r * r + dj * r + dk]
                        lane += 1
        else:
            ii, jj, mm = np.meshgrid(np.arange(r), np.arange(r), np.arange(M), indexing="ij")
            if l == 7:
                for di in (0, 1):
                    oo = o if di == 0 else SLOT_OFF7B
                    lane = 0
                    for dj in (0, 1):
                        for t in range(Q + 1):
                            kidx = np.minimum(mm * Q + t, r)
                            row = ((ii + di) * r + (jj + dj)) * r + kidx
                            src = np.where(mm * Q + t <= r, seg[row], 0)
                            tabx[oo : oo + r * r * M, lane] = src.ravel()
                            lane += 1
            else:
                lane = 0
                for di in (0, 1):
                    for dj in (0, 1):
                        for t in range(Q + 1):
                            kidx = np.minimum(mm * Q + t, r)
                            row = ((ii + di) * r + (jj + dj)) * r + kidx
                            src = np.where(mm * Q + t <= r, seg[row], 0)
                            tabx[o : o + r * r * M, lane] = src.ravel()
                            lane += 1
    for l in HASH_LV:
        o, ns = SLOT_OFF[l], N_SLOTS[l]
        seg = pk[OFFS[l] : OFFS[l + 1]]
        tabx[o : o + ns, 0:16] = seg.reshape(ns, 16)
    return tabx.view(np.int32)


def build_cvec():
    cv = np.zeros(128, dtype=np.float32)
    for l in range(N_LEVELS):
        grid = np.float32(2.0) / np.float32(RES[l])
        cv[l] = np.float32(1.0) / grid
        cv[16 + l] = np.float32(RES[l] - 1)
        cv[32 + l] = grid
    for l in DENSE_LV:
        r, Q, M = RES[l], LEVEL_Q[l], LEVEL_M[l]
        cv[48 + l] = np.float32(r * M)
        cv[56 + l] = np.float32(M)
        cv[64 + l] = np.float32(1.0 / Q)
        cv[72 + l] = np.float32(Q)
    return cv.reshape(1, 128)


_NC_CACHE = {}
TRACE = False
LAST_NS = None


def _get_nc(slots, S):
    key = (slots, S)
    if key not in _NC_CACHE:
        _NC_CACHE[key] = build_kernel(slots, S)
    return _NC_CACHE[key]


def kernel(x: np.ndarray, tables: np.ndarray) -> np.ndarray:
    global LAST_NS
    from concourse.bass_utils import run_bass_kernel_spmd

    B = x.shape[0]
    per_core = B // N_CORES
    slots = per_core // P
    S = min(32, slots)
    nc = _get_nc(slots, S)
    tabx = build_tabx(tables)
    cv = build_cvec()
    in_maps = []
    for c in range(N_CORES):
        xs = np.ascontiguousarray(
            x[c * per_core : (c + 1) * per_core].reshape(P, slots, 3)
        ).astype(np.float32)
        in_maps.append({"x": xs, "tabx": tabx, "cvec": cv})
    kw = {"trace": True} if TRACE else {}
    res = run_bass_kernel_spmd(nc, in_maps, core_ids=list(range(N_CORES)), **kw)
    LAST_NS = res.exec_time_ns
    outs = [res.results[c]["out"].reshape(per_core, 32) for c in range(N_CORES)]
    return np.concatenate(outs, axis=0).astype(np.float32)


# revision 23
# speedup vs baseline: 1.1444x; 1.0259x over previous
"""HashEmbedder3D Trainium2 kernel v6.

Key changes vs v2 baseline:
- dma_gather with single_packet=False + indices replicated across all 8
  16-partition groups allows 8192-idx instructions (994ns fixed cost
  amortized 8x).
- Dense levels 0-3: slot-per-voxel-base block tables (one 32B descriptor
  fetches all 8 corners; no select trees). Levels 4-7: Q-packed slots
  (one descriptor + small k-offset select tree).
- Hash levels 8-15: one 64B slot read per corner (8 classes), extraction
  via 4-round select tree, gathers batched 2 classes per instruction.
- Index wrap/transpose done by DRAM-bounce DMA + DVE interleave + SBUF
  broadcast DMAs; Pool engine only runs SWDGE gathers.
"""
import math
import sys

import numpy as np

sys.path.insert(0, "/opt/trn_rl_repo")

from concourse import bacc, bass, mybir
import concourse.tile as tile

N_LEVELS = 16
F = 2
LOG2_T = 19
T = 1 << LOG2_T
BASE, FINEST = 16, 512
B_GROWTH = float(np.exp((np.log(np.float32(FINEST)) - np.log(np.float32(BASE))) / np.float32(N_LEVELS - 1)))
RES = [math.floor(BASE * B_GROWTH**i) for i in range(N_LEVELS)]
SIZES = [(r + 1) ** 3 if r**3 < T else T for r in RES]
OFFS = np.concatenate([[0], np.cumsum(SIZES)]).tolist()
TOTAL_ROWS = OFFS[-1]
PRIMES = [1, 2654435761, 805459861]
N_POINTS = 1048576
N_CORES = 8
P = 128

DT = mybir.dt
AL = mybir.AluOpType

DENSE_LV = list(range(8))
HASH_LV = list(range(8, 16))

# dense level slot geometry: levels 0-3 slot-per-base, 4-7 Q-packed
LEVEL_Q = {0: 1, 1: 1, 2: 1, 3: 1, 4: 2, 5: 4, 6: 8, 7: 16}
LEVEL_M = {}
N_SLOTS, ELEM = {}, {}
for l in DENSE_LV:
    r, Q = RES[l], LEVEL_Q[l]
    M = -(-r // Q)
    LEVEL_M[l] = M
    N_SLOTS[l] = r * r * M
    ELEM[l] = 8 if Q == 1 else 4 * (Q + 1)
ELEM[7] = 2 * (LEVEL_Q[7] + 1)  # level 7 split in two di-halves of 2x17 lanes
for l in HASH_LV:
    N_SLOTS[l], ELEM[l] = T // 16, 16
SLOT_OFF = {}
_a = 0
for l in range(N_LEVELS):
    SLOT_OFF[l] = _a
    _a += N_SLOTS[l]
SLOT_OFF7B = _a  # second (di=1) half of level 7
_a += N_SLOTS[7]
TOT_SLOTS = _a
assert all(N_SLOTS[l] <= 32768 for l in range(N_LEVELS)), N_SLOTS

# class layout: dense levels are classes 0-7; hash level l corner m is
# class 8 + (l-8)*8 + m. Each class contributes S indices per tile.
NCC = 8 + 8 * 8  # 72


def _i32(v):
    return int(np.int32(np.uint32(v)))


MAX_GIDX = 8192


def dma_gather_raw(eng, out_ap, in_ap, idxs_ap, num_idxs, elem_size, elem_step=64):
    stride_bytes = elem_step * 4
    assert stride_bytes % 256 == 0
    _in_ap = eng.lower_ap_dma(in_ap, for_custom_bir_dma=True)
    _idxs_ap = eng.lower_ap(idxs_ap)
    _out_ap = eng.lower_ap(out_ap)
    return eng.add_instruction(
        mybir.InstDMAGatherAnt(
            name=eng.bass.get_next_instruction_name(),
            ins=[*_in_ap, _idxs_ap, eng.lower_val_access(eng.to_reg(num_idxs))],
            outs=[_out_ap],
            transpose=False,
            num_idxs=num_idxs,
            elem_size=elem_size,
            stride_bytes_256=stride_bytes // 256,
            gen_mode=0,
            single_packet=num_idxs <= 1024,
            queue_num=0,
            sbuf_tokens_per_rank=0,
            sbuf_free_dim_per_rank=0,
            sbuf_free_dim_pad_per_rank=0,
            sbuf_byte_offset=0,
        )
    )


def apd(tap, off, dims):
    return bass.AP(tap.tensor, tap.offset + off, [list(d) for d in dims])


def build_kernel(slots_total, S):
    n_outer = slots_total // S
    assert n_outer * S == slots_total

    nc = bacc.Bacc(None, target_bir_lowering=False, debug=False)
    x_in = nc.dram_tensor("x", [P, slots_total, 3], DT.float32, kind="ExternalInput")
    tabx = nc.dram_tensor("tabx", [TOT_SLOTS, 64], DT.int32, kind="ExternalInput")
    cvec_in = nc.dram_tensor("cvec", [1, 128], DT.float32, kind="ExternalInput")
    out = nc.dram_tensor("out", [P, slots_total, 32], DT.float32, kind="ExternalOutput")

    nW = NCC * S  # idx ints per partition per tile

    with tile.TileContext(nc) as tc:
        with (
            tc.tile_pool(name="big", bufs=1) as bigp,
            tc.tile_pool(name="dbl", bufs=1) as dblp,
            tc.tile_pool(name="work", bufs=1) as workp,
        ):
            cv = bigp.tile([P, 128], DT.float32, tag="cv", name="cv")
            nc.sync.dma_start(cv[:], apd(cvec_in[:], 0, [[0, P], [1, 128]]))

            def cvb3(col, n, w=8):
                # [P, n, w] view of per-level const at cv[col:col+w]
                return apd(cv, col, [list(cv.ap[0]), [0, n], [1, w]])

            def cvb4(col, a, b, w=8):
                return apd(cv, col, [list(cv.ap[0]), [0, a], [0, b], [1, w]])

            x_t = bigp.tile([P, S, 3], DT.float32, tag="x_t", name="x_t")
            w_bf = None
            idxb = bigp.tile([P, nW], DT.int16, tag="idxb", name="idxb")
            hm = None
            koffs = None
            scr = bigp.tile([P, nW], DT.int16, tag="scr", name="scr", space="DRAM")
            wt = bigp.tile([P, 2 * nW], DT.int16, tag="wt", name="wt")
            wrp = None
            gdd = [
                bigp.tile([P, 68 * S], DT.int32, tag=f"gdd{i}", name=f"gdd{i}")
                for i in range(2)
            ]
            gdh = [
                bigp.tile([P, 2 * 16 * S], DT.int32, tag=f"gdh{i}", name=f"gdh{i}")
                for i in range(3)
            ]
            hcor = bigp.tile([P, 8 * S], DT.int32, tag="hcor", name="hcor")
            osb = bigp.tile([P, S, 32], DT.float32, tag="osb", name="osb")
            bli_d = bigp.tile([P, S, 3, 8], DT.int32, tag="bli_d", name="bli_d")
            bli_h = bigp.tile([P, S, 3, 8], DT.int32, tag="bli_h", name="bli_h")

            nc.vector.memset(wt[:], 0)
            _wrp0 = dblp.tile([P, 8 * nW], DT.int16, tag="wrp", name="wrp_init")
            nc.vector.memset(_wrp0[:], 0)

            with tc.For_i(
                0,
                slots_total,
                S,
                hint_engines=(mybir.EngineType.DVE, mybir.EngineType.Pool),
            ) as st:
                nc.sync.dma_start(x_t[:], x_in[:, bass.ds(st, S), :])
                w_bf = dblp.tile([P, S, 48], DT.bfloat16, tag="w_bf", name="w_bf")
                hm = dblp.tile([P, 64 * S], DT.int32, tag="hm", name="hm")
                koffs = dblp.tile([P, S, 8], DT.int32, tag="koffs", name="koffs")
                wrp = dblp.tile([P, 8 * nW], DT.int16, tag="wrp", name="wrp")

                # ================= phase 1: voxel coords + weights ============
                xc = workp.tile([P, S, 3], DT.float32, tag="xc", name="xc")
                nc.vector.tensor_scalar(out=xc[:], in0=x_t[:], op0=AL.max, scalar1=-1.0, op1=AL.min, scalar2=1.0)

                for half, lv0 in ((0, 0), (1, 8)):
                    bli = bli_d if half == 0 else bli_h
                    tf = workp.tile([P, S, 3, 8], DT.float32, tag="tf", name="tf")
                    fi = workp.tile([P, S, 3, 8], DT.int32, tag="fi", name="fi")
                    ff = workp.tile([P, S, 3, 8], DT.float32, tag="ff", name="ff")
                    blf = workp.tile([P, S, 3, 8], DT.float32, tag="blf", name="blf")
                    su = workp.tile([P, S, 3, 8], DT.float32, tag="su", name="su")
                    xb = apd(xc, 0, [list(xc.ap[0]), [3, S], [1, 3], [0, 8]])
                    xbu = apd(x_t, 0, [list(x_t.ap[0]), [3, S], [1, 3], [0, 8]])
                    nc.vector.tensor_scalar(out=tf[:], in0=xb, op0=AL.add, scalar1=1.0, scalar2=None)
                    nc.vector.tensor_tensor(out=tf[:], in0=tf[:], in1=cvb4(lv0, S, 3), op=AL.mult)
                    nc.vector.tensor_copy(out=fi[:], in_=tf[:])
                    nc.vector.tensor_copy(out=ff[:], in_=fi[:])
                    nc.vector.tensor_tensor(out=blf[:], in0=ff[:], in1=tf[:], op=AL.is_gt)
                    nc.vector.tensor_tensor(out=blf[:], in0=ff[:], in1=blf[:], op=AL.subtract)
                    nc.vector.tensor_scalar(out=blf[:], in0=blf[:], op0=AL.max, scalar1=0.0, scalar2=None)
                    nc.vector.tensor_tensor(out=blf[:], in0=blf[:], in1=cvb4(16 + lv0, S, 3), op=AL.min)
                    nc.vector.tensor_copy(out=bli[:], in_=blf[:])
                    nc.vector.tensor_tensor(out=su[:], in0=blf[:], in1=cvb4(32 + lv0, S, 3), op=AL.mult)
                    nc.vector.tensor_scalar(out=su[:], in0=su[:], op0=AL.add, scalar1=-1.0, scalar2=None)
                    nc.vector.tensor_tensor(out=su[:], in0=xbu, in1=su[:], op=AL.subtract)
                    nc.vector.tensor_tensor(out=su[:], in0=su[:], in1=cvb4(lv0, S, 3), op=AL.mult)
                    wdst = apd(w_bf, 3 * lv0, [list(w_bf.ap[0]), [48, S], [1, 3], [3, 8]])
                    nc.vector.tensor_copy(out=wdst, in_=su[:])

                # ================= dense slot ids =============================
                i_ = bli_d[:, :, 0, :]
                j_ = bli_d[:, :, 1, :]
                k_ = bli_d[:, :, 2, :]
                kq = workp.tile([P, S, 8], DT.int32, tag="kq", name="kq")
                sid = workp.tile([P, S, 8], DT.int32, tag="sid", name="sid")
                t1 = workp.tile([P, S, 8], DT.int32, tag="t1d", name="t1d")
                nc.vector.tensor_copy(out=kq[:], in_=k_)
                for l in range(4, 8):
                    q = LEVEL_Q[l].bit_length() - 1
                    nc.vector.tensor_scalar(out=kq[:, :, l], in0=k_[:, :, l], op0=AL.logical_shift_right, scalar1=q, scalar2=None)
                # koff = k - kq*Q  (only levels 4-7 used)
                nc.vector.tensor_tensor(out=koffs[:], in0=kq[:], in1=apd(cv, 72, [list(cv.ap[0]), [0, S], [1, 8]]), op=AL.mult)
                nc.vector.tensor_tensor(out=koffs[:], in0=k_, in1=koffs[:], op=AL.subtract)
                # sid = i*A + j*B + kq
                nc.vector.tensor_tensor(out=t1[:], in0=i_, in1=apd(cv, 48, [list(cv.ap[0]), [0, S], [1, 8]]), op=AL.mult)
                nc.vector.tensor_tensor(out=sid[:], in0=j_, in1=apd(cv, 56, [list(cv.ap[0]), [0, S], [1, 8]]), op=AL.mult)
                nc.vector.tensor_tensor(out=sid[:], in0=sid[:], in1=t1[:], op=AL.add)
                nc.vector.tensor_tensor(out=sid[:], in0=sid[:], in1=kq[:], op=AL.add)
                # write dense classes: idxb[:, l*S + x] = sid[:, x, l]
                nc.vector.tensor_copy(
                    out=apd(idxb, 0, [list(idxb.ap[0]), [1, S], [S, 8]]),
                    in_=sid[:],
                )
                # dense k_off select masks, all levels at once, per bit
                dmb = []
                for b in range(4):
                    mbt = workp.tile([P, S, 8], DT.int32, tag=f"dmb{b}", name=f"dmb{b}")
                    nc.vector.tensor_scalar(out=mbt[:], in0=koffs[:], op0=AL.logical_shift_right, scalar1=b, op1=AL.bitwise_and, scalar2=1)
                    nc.vector.tensor_scalar(out=mbt[:], in0=mbt[:], op0=AL.mult, scalar1=-1, scalar2=None)
                    dmb.append(mbt)

                # ================= hash slot ids ==============================
                ih = bli_h[:, :, 0, :]
                jh = bli_h[:, :, 1, :]
                kh = bli_h[:, :, 2, :]
                mt1 = workp.tile([P, S, 8], DT.int32, tag="mt1", name="mt1")
                mt2 = workp.tile([P, S, 8], DT.int32, tag="mt2", name="mt2")
                mt3 = workp.tile([P, S, 8], DT.int32, tag="mt3", name="mt3")

                def ts(o, i, op, s):
                    nc.vector.tensor_scalar(out=o, in0=i, op0=op, scalar1=s, scalar2=None)

                def tt(o, a, b, op):
                    nc.vector.tensor_tensor(out=o, in0=a, in1=b, op=op)

                def mul32(dst, src, prime):
                    Hp, Lp = (prime >> 16) & 0xFFFF, prime & 0xFFFF
                    Hs = Hp - 32768 if Hp >= 32768 else Hp
                    ts(mt1[:], src, AL.mult, Lp)
                    ts(mt2[:], src, AL.mult, Hs)
                    if Hp >= 32768:
                        ts(mt3[:], src, AL.logical_shift_left, 15)
                        ts(mt3[:], mt3[:], AL.bitwise_and, 0xFFFF)
                        ts(mt2[:], mt2[:], AL.bitwise_and, 0xFFFF)
                        tt(mt2[:], mt2[:], mt3[:], AL.add)
                    ts(mt2[:], mt2[:], AL.bitwise_and, 0xFFFF)
                    ts(mt3[:], mt1[:], AL.logical_shift_right, 16)
                    tt(mt2[:], mt2[:], mt3[:], AL.add)
                    ts(mt2[:], mt2[:], AL.bitwise_and, 0xFFFF)
                    ts(mt2[:], mt2[:], AL.logical_shift_left, 16)
                    ts(mt1[:], mt1[:], AL.bitwise_and, 0xFFFF)
                    tt(dst, mt2[:], mt1[:], AL.bitwise_or)

                def add32(dst, src, const):
                    cl, ch = const & 0xFFFF, (const >> 16) & 0xFFFF
                    ts(mt1[:], src, AL.bitwise_and, 0xFFFF)
                    ts(mt1[:], mt1[:], AL.add, cl)
                    ts(mt2[:], src, AL.logical_shift_right, 16)
                    ts(mt2[:], mt2[:], AL.bitwise_and, 0xFFFF)
                    ts(mt2[:], mt2[:], AL.add, ch)
                    ts(mt3[:], mt1[:], AL.logical_shift_right, 16)
                    tt(mt2[:], mt2[:], mt3[:], AL.add)
                    ts(mt2[:], mt2[:], AL.bitwise_and, 0xFFFF)
                    ts(mt2[:], mt2[:], AL.logical_shift_left, 16)
                    ts(mt1[:], mt1[:], AL.bitwise_and, 0xFFFF)
                    tt(dst, mt2[:], mt1[:], AL.bitwise_or)

                jp0 = workp.tile([P, S, 8], DT.int32, tag="jp0", name="jp0")
                jp1 = workp.tile([P, S, 8], DT.int32, tag="jp1", name="jp1")
                kp0 = workp.tile([P, S, 8], DT.int32, tag="kp0", name="kp0")
                kp1 = workp.tile([P, S, 8], DT.int32, tag="kp1", name="kp1")
                ii1 = workp.tile([P, S, 8], DT.int32, tag="ii1", name="ii1")
                rr = workp.tile([P, S, 8], DT.int32, tag="rr", name="rr")
                rr2 = workp.tile([P, S, 8], DT.int32, tag="rr2", name="rr2")
                mul32(jp0[:], jh, PRIMES[1])
                add32(jp1[:], jp0[:], PRIMES[1])
                mul32(kp0[:], kh, PRIMES[2])
                add32(kp1[:], kp0[:], PRIMES[2])
                ts(ii1[:], ih, AL.add, 1)
                # vectorized corner ids: xab[c] = (ih|ii1) ^ (jp0|jp1), c=di*2+dj
                xab = workp.tile([P, S, 4, 8], DT.int32, tag="xab", name="xab")
                tt(xab[:, :, 0, :], ih, jp0[:], AL.bitwise_xor)
                tt(xab[:, :, 1, :], ih, jp1[:], AL.bitwise_xor)
                tt(xab[:, :, 2, :], ii1[:], jp0[:], AL.bitwise_xor)
                tt(xab[:, :, 3, :], ii1[:], jp1[:], AL.bitwise_xor)
                # rr_all[x, m, l] with m = c*2+dk
                rr_all = workp.tile([P, S, 8, 8], DT.int32, tag="rr_all", name="rr_all")
                for dk in (0, 1):
                    tt(
                        apd(rr_all, dk * 8, [list(rr_all.ap[0]), [64, S], [16, 4], [1, 8]]),
                        apd(xab, 0, [list(xab.ap[0]), [32, S], [8, 4], [1, 8]]),
                        (kp1 if dk else kp0)[:].to_broadcast([P, S, 4, 8]) if False else apd(kp1 if dk else kp0, 0, [list(kp0.ap[0]), [8, S], [0, 4], [1, 8]]),
                        AL.bitwise_xor,
                    )
                ts(rr_all[:], rr_all[:], AL.bitwise_and, T - 1)
                sh = workp.tile([P, S, 8, 8], DT.int32, tag="rrsh", name="rrsh")
                ts(sh[:], rr_all[:], AL.logical_shift_right, 4)
                nc.vector.tensor_copy(
                    out=apd(idxb, 8 * S, [list(idxb.ap[0]), [1, S], [S, 8], [8 * S, 8]]),
                    in_=sh[:],
                )
                ts(sh[:], rr_all[:], AL.bitwise_and, 15)
                nc.vector.tensor_copy(
                    out=apd(hm, 0, [list(hm.ap[0]), [1, S], [S, 8], [8 * S, 8]]),
                    in_=sh[:],
                )

                # ============== idx transpose to wrapped-16 + replicate =======
                nc.sync.dma_start(scr[:], idxb[:])
                # Only partitions 16:32 (group 1) are read by the SWDGE
                # cores (each core g reads cols == g mod 8 there). Build the
                # wrapped matrix in group 1; mirror to group 0 for CoreSim,
                # whose interpreter consumes group 0.
                # 8 ping-ponged chunks: DVE only stalls on the first read
                def rd(h):
                    half = wt[:, (h % 2) * nW : (h % 2 + 1) * nW]
                    nc.sync.dma_start(
                        half[0:16, :],
                        apd(scr, h * 16 * nW, [[nW, 16], [1, nW]]),
                    )

                def il(h):
                    half = wt[:, (h % 2) * nW : (h % 2 + 1) * nW]
                    nc.vector.tensor_copy(
                        out=apd(wrp, h, [list(wrp.ap[0]), [8, nW]]),
                        in_=apd(half, 0, [list(half.ap[0]), [1, nW]]),
                    )

                rd(0)
                rd(1)
                for h in range(8):
                    il(h)
                    if h + 2 < 8:
                        rd(h + 2)
                nc.sync.dma_start(wrp[16:32, :], wrp[0:16, :])

                # ================= gathers ====================================
                def lerp(dst, lo, hi, w):
                    nc.vector.tensor_tensor(out=dst, in0=hi, in1=lo, op=AL.subtract)
                    nc.vector.tensor_tensor(out=dst, in0=dst, in1=w, op=AL.mult)
                    nc.vector.tensor_tensor(out=dst, in0=dst, in1=lo, op=AL.add)

                def wof(l, d, reps, minor):
                    # weight w_bf[:, x, 3l+d] broadcast directly as bf16:
                    # minor=True -> [S, reps] (x outer), else [reps, S]
                    if minor:
                        return apd(w_bf, 3 * l + d, [list(w_bf.ap[0]), [48, S], [0, reps]])
                    return apd(w_bf, 3 * l + d, [list(w_bf.ap[0]), [0, reps], [48, S]])

                def unpack(src_ap, n, tag):
                    ef = workp.tile([P, 2, n], DT.int32, tag=f"ef{tag}", name=f"ef{tag}")
                    nc.vector.tensor_scalar(out=ef[:, 0, :], in0=src_ap, op0=AL.logical_shift_left, scalar1=16, scalar2=None)
                    nc.vector.tensor_scalar(out=ef[:, 1, :], in0=src_ap, op0=AL.bitwise_and, scalar1=_i32(0xFFFF0000), scalar2=None)
                    return ef

                def f32v(t, dims, off=0):
                    return apd(t, off, [list(t.ap[0])] + [list(d) for d in dims]).bitcast(DT.float32)

                # ---- dense levels ----
                def ktree(l, gd, el, ngrp, q, Q, tag):
                    # select k_off window over bits q-1..0; gd lanes
                    # [x*el + g*(Q+1) + t]; returns tile [P, S, ngrp, 2]
                    cur = None
                    for b in range(q - 1, -1, -1):
                        wnew = 2**b + 1 if b > 0 else 2
                        half = 2**b
                        nxt = workp.tile([P, S, ngrp, wnew], DT.int32, tag=f"dt{tag}{b}", name=f"dt{tag}{b}")
                        if cur is None:
                            lo = apd(gd, 0, [list(gd.ap[0]), [el, S], [Q + 1, ngrp], [1, wnew]])
                            hi = apd(gd, half, [list(gd.ap[0]), [el, S], [Q + 1, ngrp], [1, wnew]])
                        else:
                            lo = apd(cur, 0, [list(cur.ap[0]), [cur.ap[1][0], S], [cur.ap[2][0], ngrp], [1, wnew]])
                            hi = apd(cur, half, [list(cur.ap[0]), [cur.ap[1][0], S], [cur.ap[2][0], ngrp], [1, wnew]])
                        mbb = apd(dmb[b], l, [list(dmb[b].ap[0]), [8, S], [0, ngrp], [0, wnew]])
                        nc.vector.tensor_tensor(out=nxt[:], in0=lo, in1=hi, op=AL.bitwise_xor)
                        nc.vector.tensor_tensor(out=nxt[:], in0=nxt[:], in1=mbb, op=AL.bitwise_and)
                        nc.vector.tensor_tensor(out=nxt[:], in0=nxt[:], in1=lo, op=AL.bitwise_xor)
                        cur = nxt
                    return cur

                for l in DENSE_LV:
                    el = ELEM[l]
                    Q = LEVEL_Q[l]
                    q = Q.bit_length() - 1
                    gd = gdd[l % 2]
                    if l == 7:
                        baseA = tabx[SLOT_OFF[7] : SLOT_OFF[7] + N_SLOTS[7], 0:el]
                        baseB = tabx[SLOT_OFF7B : SLOT_OFF7B + N_SLOTS[7], 0:el]
                        gdA, gdB = gdd[0], gdd[1]
                        for gdX, baseX in ((gdA, baseA), (gdB, baseB)):
                            gview = apd(gdX, 0, [list(gdX.ap[0]), [el, S], [1, el]])
                            dma_gather_raw(
                                nc.gpsimd, gview, baseX,
                                wrp[:, l * 8 * S : (l + 1) * 8 * S], S * P, el,
                            )
                        curA = ktree(l, gdA, el, 2, q, Q, "2a")
                        curB = ktree(l, gdB, el, 2, q, Q, "2b")
                        comb = workp.tile([P, S, 8], DT.int32, tag="comb7", name="comb7")
                        nc.vector.tensor_copy(
                            out=apd(comb, 0, [list(comb.ap[0]), [8, S], [1, 4]]),
                            in_=apd(curA, 0, [list(curA.ap[0]), [4, S], [1, 4]]),
                        )
                        nc.vector.tensor_copy(
                            out=apd(comb, 4, [list(comb.ap[0]), [8, S], [1, 4]]),
                            in_=apd(curB, 0, [list(curB.ap[0]), [4, S], [1, 4]]),
                        )
                        csrc = apd(comb, 0, [list(comb.ap[0]), [1, 8 * S]])
                    else:
                        base = tabx[SLOT_OFF[l] : SLOT_OFF[l] + N_SLOTS[l], 0:el]
                        gview = apd(gd, 0, [list(gd.ap[0]), [el, S], [1, el]])
                        dma_gather_raw(
                            nc.gpsimd, gview, base,
                            wrp[:, l * 8 * S : (l + 1) * 8 * S], S * P, el,
                        )
                        if Q == 1:
                            csrc = apd(gd, 0, [list(gd.ap[0]), [1, 8 * S]])
                        else:
                            cur = ktree(l, gd, el, 4, q, Q, "4g")
                            csrc = apd(cur, 0, [list(cur.ap[0]), [1, 8 * S]])
                    # corners in x-major layout: lane x*8 + m; both features fused
                    ef = unpack(csrc, 8 * S, "d")
                    def wv(d, dims):
                        return apd(w_bf, 3 * l + d, [list(w_bf.ap[0])] + dims)
                    efv = lambda off, dims: apd(ef, off, [list(ef.ap[0])] + dims).bitcast(DT.float32)
                    xt = workp.tile([P, 2, S, 4], DT.float32, tag="xtd", name="xtd")
                    lerp(xt[:], efv(0, [[8 * S, 2], [8, S], [1, 4]]), efv(4, [[8 * S, 2], [8, S], [1, 4]]), wv(0, [[0, 2], [48, S], [0, 4]]))
                    yt = workp.tile([P, 2, S, 2], DT.float32, tag="ytd", name="ytd")
                    lerp(yt[:], apd(xt, 0, [list(xt.ap[0]), [4 * S, 2], [4, S], [1, 2]]), apd(xt, 2, [list(xt.ap[0]), [4 * S, 2], [4, S], [1, 2]]), wv(1, [[0, 2], [48, S], [0, 2]]))
                    od = apd(osb, 2 * l, [list(osb.ap[0]), [1, 2], [32, S]])
                    lerp(od, apd(yt, 0, [list(yt.ap[0]), [2 * S, 2], [2, S]]), apd(yt, 1, [list(yt.ap[0]), [2 * S, 2], [2, S]]), wv(2, [[0, 2], [48, S]]))

                # ---- hash levels ----
                for l in HASH_LV:
                    lh = l - 8
                    base = tabx[SLOT_OFF[l] : SLOT_OFF[l] + N_SLOTS[l], 0:16]
                    cls0 = 8 + lh * 8

                    hmv_l = apd(hm, lh * 8 * S, [list(hm.ap[0]), [1, 8 * S]])
                    mbl = {}
                    for b in range(4):
                        mb = workp.tile([P, 8 * S], DT.int32, tag=f"hmk{b}", name=f"hmk{b}")
                        nc.vector.tensor_scalar(out=mb[:], in0=hmv_l, op0=AL.logical_shift_right, scalar1=b, op1=AL.bitwise_and, scalar2=1)
                        nc.vector.tensor_scalar(out=mb[:], in0=mb[:], op0=AL.mult, scalar1=-1, scalar2=None)
                        mbl[b] = mb

                    def hgather(h2):
                        gd = gdh[(4 * lh + h2) % 3]
                        gview = apd(gd, 0, [list(gd.ap[0]), [16, 2 * S], [1, 16]])
                        dma_gather_raw(
                            nc.gpsimd, gview, base,
                            wrp[:, (cls0 + 2 * h2) * 8 * S : (cls0 + 2 * h2 + 2) * 8 * S],
                            2 * S * P, 16,
                        )

                    def hextract(h2):
                        gd = gdh[(4 * lh + h2) % 3]
                        cur = None
                        for b in range(3, -1, -1):
                            half = 2**b
                            if b == 0:
                                hc = hcor[:, h2 * 2 * S : (h2 + 1) * 2 * S]
                                nxt = None
                                ov = apd(hc, 0, [list(hc.ap[0]), [1, 2 * S], [0, 1]])
                            else:
                                nxt = workp.tile([P, 2 * S, half], DT.int32, tag=f"ht{b}", name=f"ht{b}")
                                ov = nxt[:]
                            if cur is None:
                                lo = apd(gd, 0, [list(gd.ap[0]), [16, 2 * S], [1, half]])
                                hi = apd(gd, half, [list(gd.ap[0]), [16, 2 * S], [1, half]])
                            else:
                                lo = cur[:, :, 0:half]
                                hi = cur[:, :, half : 2 * half]
                            mb = mbl[b]
                            mbb = apd(mb, h2 * 2 * S, [list(mb.ap[0]), [1, 2 * S], [0, half]])
                            nc.vector.tensor_tensor(out=ov, in0=lo, in1=hi, op=AL.bitwise_xor)
                            nc.vector.tensor_tensor(out=ov, in0=ov, in1=mbb, op=AL.bitwise_and)
                            nc.vector.tensor_tensor(out=ov, in0=ov, in1=lo, op=AL.bitwise_xor)
                            cur = nxt

                    hgather(0)
                    hgather(1)
                    hextract(0)
                    hgather(2)
                    hextract(1)
                    hgather(3)
                    hextract(2)
                    hextract(3)
                    # hcor: [P, 8S] corner values, class-major (c*S + x); fused feats
                    ef = unpack(hcor[:, :], 8 * S, "h")
                    def wvh(d, reps):
                        return apd(w_bf, 3 * l + d, [list(w_bf.ap[0]), [0, 2], [0, reps], [48, S]])
                    efvh = lambda off, n: apd(ef, off, [list(ef.ap[0]), [8 * S, 2], [1, n]]).bitcast(DT.float32)
                    xh = workp.tile([P, 2, 4 * S], DT.float32, tag="xth", name="xth")
                    lerp(xh[:], efvh(0, 4 * S), efvh(4 * S, 4 * S), wvh(0, 4))
                    yh = workp.tile([P, 2, 2 * S], DT.float32, tag="yth", name="yth")
                    lerp(yh[:], apd(xh, 0, [list(xh.ap[0]), [4 * S, 2], [1, 2 * S]]), apd(xh, 2 * S, [list(xh.ap[0]), [4 * S, 2], [1, 2 * S]]), wvh(1, 2))
                    oh = apd(osb, 2 * l, [list(osb.ap[0]), [1, 2], [32, S]])
                    lerp(oh, apd(yh, 0, [list(yh.ap[0]), [2 * S, 2], [1, S]]), apd(yh, S, [list(yh.ap[0]), [2 * S, 2], [1, S]]), wvh(2, 1))

                nc.gpsimd.dma_start(out[:, bass.ds(st, S), :], osb[:])
    nc.compile()
    return nc


# ---------------- host side ----------------

def _pack_bf16(t):
    u = np.ascontiguousarray(t.astype(np.float32)).view(np.uint32).astype(np.uint64)
    r = ((u + 0x7FFF + ((u >> 16) & 1)) >> 16).astype(np.uint32)
    return r[:, 0] | (r[:, 1] << np.uint32(16))


def build_tabx(tables):
    pk = _pack_bf16(tables)
    tabx = np.zeros((TOT_SLOTS, 64), dtype=np.uint32)
    for l in DENSE_LV:
        r, Q, M = RES[l], LEVEL_Q[l], LEVEL_M[l]
        o = SLOT_OFF[l]
        rp1 = r + 1
        seg = pk[OFFS[l] : OFFS[l + 1]]
        if Q == 1:
            ii, jj, kk = np.meshgrid(np.arange(r), np.arange(r), np.arange(r), indexing="ij")
            # reference indexes the (r+1)^3 table with strides r^2, r, 1
            n0 = ((ii * r + jj) * r + kk).ravel()
            lane = 0
            for di in (0, 1):
                for dj in (0, 1):
                    for dk in (0, 1):
                        tabx[o : o + r * r * r, lane] = seg[n0 + di * r * r + dj * r + dk]
                        lane += 1
        else:
            ii, jj, mm = np.meshgrid(np.arange(r), np.arange(r), np.arange(M), indexing="ij")
            if l == 7:
                for di in (0, 1):
                    oo = o if di == 0 else SLOT_OFF7B
                    lane = 0
                    for dj in (0, 1):
                        for t in range(Q + 1):
                            kidx = np.minimum(mm * Q + t, r)
                            row = ((ii + di) * r + (jj + dj)) * r + kidx
                            src = np.where(mm * Q + t <= r, seg[row], 0)
                            tabx[oo : oo + r * r * M, lane] = src.ravel()
                            lane += 1
            else:
                lane = 0
                for di in (0, 1):
                    for dj in (0, 1):
                        for t in range(Q + 1):
                            kidx = np.minimum(mm * Q + t, r)
                            row = ((ii + di) * r + (jj + dj)) * r + kidx
                            src = np.where(mm * Q + t <= r, seg[row], 0)
                            tabx[o : o + r * r * M, lane] = src.ravel()
                            lane += 1
    for l in HASH_LV:
        o, ns = SLOT_OFF[l], N_SLOTS[l]
        seg = pk[OFFS[l] : OFFS[l + 1]]
        tabx[o : o + ns, 0:16] = seg.reshape(ns, 16)
    return tabx.view(np.int32)


def build_cvec():
    cv = np.zeros(128, dtype=np.float32)
    for l in range(N_LEVELS):
        grid = np.float32(2.0) / np.float32(RES[l])
        cv[l] = np.float32(1.0) / grid
        cv[16 + l] = np.float32(RES[l] - 1)
        cv[32 + l] = grid
    for l in DENSE_LV:
        r, Q, M = RES[l], LEVEL_Q[l], LEVEL_M[l]
        cv[48 + l] = np.float32(r * M)
        cv[56 + l] = np.float32(M)
        cv[64 + l] = np.float32(1.0 / Q)
        cv[72 + l] = np.float32(Q)
    return cv.reshape(1, 128)


_NC_CACHE = {}
TRACE = False
LAST_NS = None


def _get_nc(slots, S):
    key = (slots, S)
    if key not in _NC_CACHE:
        _NC_CACHE[key] = build_kernel(slots, S)
    return _NC_CACHE[key]


def kernel(x: np.ndarray, tables: np.ndarray) -> np.ndarray:
    global LAST_NS
    from concourse.bass_utils import run_bass_kernel_spmd

    B = x.shape[0]
    per_core = B // N_CORES
    slots = per_core // P
    S = min(32, slots)
    nc = _get_nc(slots, S)
    tabx = build_tabx(tables)
    cv = build_cvec()
    in_maps = []
    for c in range(N_CORES):
        xs = np.ascontiguousarray(
            x[c * per_core : (c + 1) * per_core].reshape(P, slots, 3)
        ).astype(np.float32)
        in_maps.append({"x": xs, "tabx": tabx, "cvec": cv})
    kw = {"trace": True} if TRACE else {}
    res = run_bass_kernel_spmd(nc, in_maps, core_ids=list(range(N_CORES)), **kw)
    LAST_NS = res.exec_time_ns
    outs = [res.results[c]["out"].reshape(per_core, 32) for c in range(N_CORES)]
    return np.concatenate(outs, axis=0).astype(np.float32)


# revision 24
# speedup vs baseline: 1.1628x; 1.0161x over previous
"""HashEmbedder3D Trainium2 kernel v6.

Key changes vs v2 baseline:
- dma_gather with single_packet=False + indices replicated across all 8
  16-partition groups allows 8192-idx instructions (994ns fixed cost
  amortized 8x).
- Dense levels 0-3: slot-per-voxel-base block tables (one 32B descriptor
  fetches all 8 corners; no select trees). Levels 4-7: Q-packed slots
  (one descriptor + small k-offset select tree).
- Hash levels 8-15: one 64B slot read per corner (8 classes), extraction
  via 4-round select tree, gathers batched 2 classes per instruction.
- Index wrap/transpose done by DRAM-bounce DMA + DVE interleave + SBUF
  broadcast DMAs; Pool engine only runs SWDGE gathers.
"""
import math
import sys

import numpy as np

sys.path.insert(0, "/opt/trn_rl_repo")

from concourse import bacc, bass, mybir
import concourse.tile as tile

N_LEVELS = 16
F = 2
LOG2_T = 19
T = 1 << LOG2_T
BASE, FINEST = 16, 512
B_GROWTH = float(np.exp((np.log(np.float32(FINEST)) - np.log(np.float32(BASE))) / np.float32(N_LEVELS - 1)))
RES = [math.floor(BASE * B_GROWTH**i) for i in range(N_LEVELS)]
SIZES = [(r + 1) ** 3 if r**3 < T else T for r in RES]
OFFS = np.concatenate([[0], np.cumsum(SIZES)]).tolist()
TOTAL_ROWS = OFFS[-1]
PRIMES = [1, 2654435761, 805459861]
N_POINTS = 1048576
N_CORES = 8
P = 128

DT = mybir.dt
AL = mybir.AluOpType

DENSE_LV = list(range(8))
HASH_LV = list(range(8, 16))

# dense level slot geometry: levels 0-3 slot-per-base, 4-7 Q-packed
LEVEL_Q = {0: 1, 1: 1, 2: 1, 3: 1, 4: 2, 5: 4, 6: 8, 7: 16}
LEVEL_M = {}
N_SLOTS, ELEM = {}, {}
for l in DENSE_LV:
    r, Q = RES[l], LEVEL_Q[l]
    M = -(-r // Q)
    LEVEL_M[l] = M
    N_SLOTS[l] = r * r * M
    ELEM[l] = 8 if Q == 1 else 4 * (Q + 1)
ELEM[7] = 2 * (LEVEL_Q[7] + 1)  # level 7 split in two di-halves of 2x17 lanes
for l in HASH_LV:
    N_SLOTS[l], ELEM[l] = T // 16, 16
SLOT_OFF = {}
_a = 0
for l in range(N_LEVELS):
    SLOT_OFF[l] = _a
    _a += N_SLOTS[l]
SLOT_OFF7B = _a  # second (di=1) half of level 7
_a += N_SLOTS[7]
TOT_SLOTS = _a
assert all(N_SLOTS[l] <= 32768 for l in range(N_LEVELS)), N_SLOTS

# class layout: dense levels are classes 0-7; hash level l corner m is
# class 8 + (l-8)*8 + m. Each class contributes S indices per tile.
NCC = 8 + 8 * 8  # 72


def _i32(v):
    return int(np.int32(np.uint32(v)))


MAX_GIDX = 8192


def dma_gather_raw(eng, out_ap, in_ap, idxs_ap, num_idxs, elem_size, elem_step=64):
    stride_bytes = elem_step * 4
    assert stride_bytes % 256 == 0
    _in_ap = eng.lower_ap_dma(in_ap, for_custom_bir_dma=True)
    _idxs_ap = eng.lower_ap(idxs_ap)
    _out_ap = eng.lower_ap(out_ap)
    return eng.add_instruction(
        mybir.InstDMAGatherAnt(
            name=eng.bass.get_next_instruction_name(),
            ins=[*_in_ap, _idxs_ap, eng.lower_val_access(eng.to_reg(num_idxs))],
            outs=[_out_ap],
            transpose=False,
            num_idxs=num_idxs,
            elem_size=elem_size,
            stride_bytes_256=stride_bytes // 256,
            gen_mode=0,
            single_packet=num_idxs <= 1024,
            queue_num=0,
            sbuf_tokens_per_rank=0,
            sbuf_free_dim_per_rank=0,
            sbuf_free_dim_pad_per_rank=0,
            sbuf_byte_offset=0,
        )
    )


def apd(tap, off, dims):
    return bass.AP(tap.tensor, tap.offset + off, [list(d) for d in dims])


def build_kernel(slots_total, S):
    n_outer = slots_total // S
    assert n_outer * S == slots_total

    nc = bacc.Bacc(None, target_bir_lowering=False, debug=False)
    x_in = nc.dram_tensor("x", [P, slots_total, 3], DT.float32, kind="ExternalInput")
    tabx = nc.dram_tensor("tabx", [TOT_SLOTS, 64], DT.int32, kind="ExternalInput")
    cvec_in = nc.dram_tensor("cvec", [1, 128], DT.float32, kind="ExternalInput")
    out = nc.dram_tensor("out", [P, slots_total, 32], DT.float32, kind="ExternalOutput")

    nW = NCC * S  # idx ints per partition per tile

    with tile.TileContext(nc) as tc:
        with (
            tc.tile_pool(name="big", bufs=1) as bigp,
            tc.tile_pool(name="dbl", bufs=1) as dblp,
            tc.tile_pool(name="work", bufs=1) as workp,
        ):
            cv = bigp.tile([P, 128], DT.float32, tag="cv", name="cv")
            nc.sync.dma_start(cv[:], apd(cvec_in[:], 0, [[0, P], [1, 128]]))

            def cvb3(col, n, w=8):
                # [P, n, w] view of per-level const at cv[col:col+w]
                return apd(cv, col, [list(cv.ap[0]), [0, n], [1, w]])

            def cvb4(col, a, b, w=8):
                return apd(cv, col, [list(cv.ap[0]), [0, a], [0, b], [1, w]])

            x_t = bigp.tile([P, S, 3], DT.float32, tag="x_t", name="x_t")
            w_bf = None
            idxb = bigp.tile([P, nW], DT.int16, tag="idxb", name="idxb")
            hm = None
            koffs = None
            scr = bigp.tile([P, nW], DT.int16, tag="scr", name="scr", space="DRAM")
            wt = bigp.tile([P, 2 * nW], DT.int16, tag="wt", name="wt")
            wrp = None
            gdd = [
                bigp.tile([P, 68 * S], DT.int32, tag=f"gdd{i}", name=f"gdd{i}")
                for i in range(2)
            ]
            gdh = [
                bigp.tile([P, 2 * 16 * S], DT.int32, tag=f"gdh{i}", name=f"gdh{i}")
                for i in range(3)
            ]
            hcor = bigp.tile([P, 8 * S], DT.int32, tag="hcor", name="hcor")
            osb = bigp.tile([P, S, 32], DT.float32, tag="osb", name="osb")
            bli_d = bigp.tile([P, S, 3, 8], DT.int32, tag="bli_d", name="bli_d")
            bli_h = bigp.tile([P, S, 3, 8], DT.int32, tag="bli_h", name="bli_h")

            nc.vector.memset(wt[:], 0)
            _wrp0 = dblp.tile([P, 8 * nW], DT.int16, tag="wrp", name="wrp_init")
            nc.vector.memset(_wrp0[:], 0)

            with tc.For_i(
                0,
                slots_total,
                S,
                hint_engines=(mybir.EngineType.DVE, mybir.EngineType.Pool),
            ) as st:
                nc.sync.dma_start(x_t[:], x_in[:, bass.ds(st, S), :])
                w_bf = dblp.tile([P, S, 48], DT.bfloat16, tag="w_bf", name="w_bf")
                hm = dblp.tile([P, 64 * S], DT.int32, tag="hm", name="hm")
                koffs = dblp.tile([P, S, 8], DT.int32, tag="koffs", name="koffs")
                wrp = dblp.tile([P, 8 * nW], DT.int16, tag="wrp", name="wrp")

                # ================= phase 1: voxel coords + weights ============
                xc = workp.tile([P, S, 3], DT.float32, tag="xc", name="xc")
                nc.vector.tensor_scalar(out=xc[:], in0=x_t[:], op0=AL.max, scalar1=-1.0, op1=AL.min, scalar2=1.0)

                for half, lv0 in ((0, 0), (1, 8)):
                    bli = bli_d if half == 0 else bli_h
                    tf = workp.tile([P, S, 3, 8], DT.float32, tag="tf", name="tf")
                    fi = workp.tile([P, S, 3, 8], DT.int32, tag="fi", name="fi")
                    ff = workp.tile([P, S, 3, 8], DT.float32, tag="ff", name="ff")
                    blf = workp.tile([P, S, 3, 8], DT.float32, tag="blf", name="blf")
                    su = workp.tile([P, S, 3, 8], DT.float32, tag="su", name="su")
                    xb = apd(xc, 0, [list(xc.ap[0]), [3, S], [1, 3], [0, 8]])
                    xbu = apd(x_t, 0, [list(x_t.ap[0]), [3, S], [1, 3], [0, 8]])
                    nc.vector.tensor_scalar(out=tf[:], in0=xb, op0=AL.add, scalar1=1.0, scalar2=None)
                    nc.vector.tensor_tensor(out=tf[:], in0=tf[:], in1=cvb4(lv0, S, 3), op=AL.mult)
                    nc.vector.tensor_copy(out=fi[:], in_=tf[:])
                    nc.vector.tensor_copy(out=ff[:], in_=fi[:])
                    nc.vector.tensor_tensor(out=blf[:], in0=ff[:], in1=tf[:], op=AL.is_gt)
                    nc.vector.tensor_tensor(out=blf[:], in0=ff[:], in1=blf[:], op=AL.subtract)
                    nc.vector.tensor_scalar(out=blf[:], in0=blf[:], op0=AL.max, scalar1=0.0, scalar2=None)
                    nc.vector.tensor_tensor(out=blf[:], in0=blf[:], in1=cvb4(16 + lv0, S, 3), op=AL.min)
                    nc.vector.tensor_copy(out=bli[:], in_=blf[:])
                    # x in [-1,1) => w = (x+1)/grid - bl = tf - blf exactly
                    nc.vector.tensor_tensor(out=su[:], in0=tf[:], in1=blf[:], op=AL.subtract)
                    wdst = apd(w_bf, 3 * lv0, [list(w_bf.ap[0]), [48, S], [1, 3], [3, 8]])
                    nc.vector.tensor_copy(out=wdst, in_=su[:])

                # ================= dense slot ids =============================
                i_ = bli_d[:, :, 0, :]
                j_ = bli_d[:, :, 1, :]
                k_ = bli_d[:, :, 2, :]
                kq = workp.tile([P, S, 8], DT.int32, tag="kq", name="kq")
                sid = workp.tile([P, S, 8], DT.int32, tag="sid", name="sid")
                t1 = workp.tile([P, S, 8], DT.int32, tag="t1d", name="t1d")
                nc.vector.tensor_copy(out=kq[:], in_=k_)
                for l in range(4, 8):
                    q = LEVEL_Q[l].bit_length() - 1
                    nc.vector.tensor_scalar(out=kq[:, :, l], in0=k_[:, :, l], op0=AL.logical_shift_right, scalar1=q, scalar2=None)
                # koff = k - kq*Q  (only levels 4-7 used)
                nc.vector.tensor_tensor(out=koffs[:], in0=kq[:], in1=apd(cv, 72, [list(cv.ap[0]), [0, S], [1, 8]]), op=AL.mult)
                nc.vector.tensor_tensor(out=koffs[:], in0=k_, in1=koffs[:], op=AL.subtract)
                # sid = i*A + j*B + kq
                nc.vector.tensor_tensor(out=t1[:], in0=i_, in1=apd(cv, 48, [list(cv.ap[0]), [0, S], [1, 8]]), op=AL.mult)
                nc.vector.tensor_tensor(out=sid[:], in0=j_, in1=apd(cv, 56, [list(cv.ap[0]), [0, S], [1, 8]]), op=AL.mult)
                nc.vector.tensor_tensor(out=sid[:], in0=sid[:], in1=t1[:], op=AL.add)
                nc.vector.tensor_tensor(out=sid[:], in0=sid[:], in1=kq[:], op=AL.add)
                # write dense classes: idxb[:, l*S + x] = sid[:, x, l]
                nc.vector.tensor_copy(
                    out=apd(idxb, 0, [list(idxb.ap[0]), [1, S], [S, 8]]),
                    in_=sid[:],
                )
                # dense k_off select masks, all levels at once, per bit
                dmb = []
                for b in range(4):
                    mbt = workp.tile([P, S, 8], DT.int32, tag=f"dmb{b}", name=f"dmb{b}")
                    nc.vector.tensor_scalar(out=mbt[:], in0=koffs[:], op0=AL.logical_shift_right, scalar1=b, op1=AL.bitwise_and, scalar2=1)
                    nc.vector.tensor_scalar(out=mbt[:], in0=mbt[:], op0=AL.mult, scalar1=-1, scalar2=None)
                    dmb.append(mbt)

                # ================= hash slot ids ==============================
                ih = bli_h[:, :, 0, :]
                jh = bli_h[:, :, 1, :]
                kh = bli_h[:, :, 2, :]
                mt1 = workp.tile([P, S, 8], DT.int32, tag="mt1", name="mt1")
                mt2 = workp.tile([P, S, 8], DT.int32, tag="mt2", name="mt2")
                mt3 = workp.tile([P, S, 8], DT.int32, tag="mt3", name="mt3")

                def ts(o, i, op, s):
                    nc.vector.tensor_scalar(out=o, in0=i, op0=op, scalar1=s, scalar2=None)

                def tt(o, a, b, op):
                    nc.vector.tensor_tensor(out=o, in0=a, in1=b, op=op)

                def mul32(dst, src, prime):
                    Hp, Lp = (prime >> 16) & 0xFFFF, prime & 0xFFFF
                    Hs = Hp - 32768 if Hp >= 32768 else Hp
                    ts(mt1[:], src, AL.mult, Lp)
                    ts(mt2[:], src, AL.mult, Hs)
                    if Hp >= 32768:
                        ts(mt3[:], src, AL.logical_shift_left, 15)
                        ts(mt3[:], mt3[:], AL.bitwise_and, 0xFFFF)
                        ts(mt2[:], mt2[:], AL.bitwise_and, 0xFFFF)
                        tt(mt2[:], mt2[:], mt3[:], AL.add)
                    ts(mt2[:], mt2[:], AL.bitwise_and, 0xFFFF)
                    ts(mt3[:], mt1[:], AL.logical_shift_right, 16)
                    tt(mt2[:], mt2[:], mt3[:], AL.add)
                    ts(mt2[:], mt2[:], AL.bitwise_and, 0xFFFF)
                    ts(mt2[:], mt2[:], AL.logical_shift_left, 16)
                    ts(mt1[:], mt1[:], AL.bitwise_and, 0xFFFF)
                    tt(dst, mt2[:], mt1[:], AL.bitwise_or)

                def add32(dst, src, const):
                    cl, ch = const & 0xFFFF, (const >> 16) & 0xFFFF
                    ts(mt1[:], src, AL.bitwise_and, 0xFFFF)
                    ts(mt1[:], mt1[:], AL.add, cl)
                    ts(mt2[:], src, AL.logical_shift_right, 16)
                    ts(mt2[:], mt2[:], AL.bitwise_and, 0xFFFF)
                    ts(mt2[:], mt2[:], AL.add, ch)
                    ts(mt3[:], mt1[:], AL.logical_shift_right, 16)
                    tt(mt2[:], mt2[:], mt3[:], AL.add)
                    ts(mt2[:], mt2[:], AL.bitwise_and, 0xFFFF)
                    ts(mt2[:], mt2[:], AL.logical_shift_left, 16)
                    ts(mt1[:], mt1[:], AL.bitwise_and, 0xFFFF)
                    tt(dst, mt2[:], mt1[:], AL.bitwise_or)

                jp0 = workp.tile([P, S, 8], DT.int32, tag="jp0", name="jp0")
                jp1 = workp.tile([P, S, 8], DT.int32, tag="jp1", name="jp1")
                kp0 = workp.tile([P, S, 8], DT.int32, tag="kp0", name="kp0")
                kp1 = workp.tile([P, S, 8], DT.int32, tag="kp1", name="kp1")
                ii1 = workp.tile([P, S, 8], DT.int32, tag="ii1", name="ii1")
                rr = workp.tile([P, S, 8], DT.int32, tag="rr", name="rr")
                rr2 = workp.tile([P, S, 8], DT.int32, tag="rr2", name="rr2")
                mul32(jp0[:], jh, PRIMES[1])
                add32(jp1[:], jp0[:], PRIMES[1])
                mul32(kp0[:], kh, PRIMES[2])
                add32(kp1[:], kp0[:], PRIMES[2])
                ts(ii1[:], ih, AL.add, 1)
                # vectorized corner ids: xab[c] = (ih|ii1) ^ (jp0|jp1), c=di*2+dj
                xab = workp.tile([P, S, 4, 8], DT.int32, tag="xab", name="xab")
                tt(xab[:, :, 0, :], ih, jp0[:], AL.bitwise_xor)
                tt(xab[:, :, 1, :], ih, jp1[:], AL.bitwise_xor)
                tt(xab[:, :, 2, :], ii1[:], jp0[:], AL.bitwise_xor)
                tt(xab[:, :, 3, :], ii1[:], jp1[:], AL.bitwise_xor)
                # rr_all[x, m, l] with m = c*2+dk
                rr_all = workp.tile([P, S, 8, 8], DT.int32, tag="rr_all", name="rr_all")
                for dk in (0, 1):
                    tt(
                        apd(rr_all, dk * 8, [list(rr_all.ap[0]), [64, S], [16, 4], [1, 8]]),
                        apd(xab, 0, [list(xab.ap[0]), [32, S], [8, 4], [1, 8]]),
                        (kp1 if dk else kp0)[:].to_broadcast([P, S, 4, 8]) if False else apd(kp1 if dk else kp0, 0, [list(kp0.ap[0]), [8, S], [0, 4], [1, 8]]),
                        AL.bitwise_xor,
                    )
                ts(rr_all[:], rr_all[:], AL.bitwise_and, T - 1)
                sh = workp.tile([P, S, 8, 8], DT.int32, tag="rrsh", name="rrsh")
                ts(sh[:], rr_all[:], AL.logical_shift_right, 4)
                nc.vector.tensor_copy(
                    out=apd(idxb, 8 * S, [list(idxb.ap[0]), [1, S], [S, 8], [8 * S, 8]]),
                    in_=sh[:],
                )
                ts(sh[:], rr_all[:], AL.bitwise_and, 15)
                nc.vector.tensor_copy(
                    out=apd(hm, 0, [list(hm.ap[0]), [1, S], [S, 8], [8 * S, 8]]),
                    in_=sh[:],
                )

                # ============== idx transpose to wrapped-16 + replicate =======
                nc.sync.dma_start(scr[:], idxb[:])
                # Only partitions 16:32 (group 1) are read by the SWDGE
                # cores (each core g reads cols == g mod 8 there). Build the
                # wrapped matrix in group 1; mirror to group 0 for CoreSim,
                # whose interpreter consumes group 0.
                # 8 ping-ponged chunks: DVE only stalls on the first read
                def rd(h):
                    half = wt[:, (h % 2) * nW : (h % 2 + 1) * nW]
                    nc.sync.dma_start(
                        half[0:16, :],
                        apd(scr, h * 16 * nW, [[nW, 16], [1, nW]]),
                    )

                def il(h):
                    half = wt[:, (h % 2) * nW : (h % 2 + 1) * nW]
                    nc.vector.tensor_copy(
                        out=apd(wrp, h, [list(wrp.ap[0]), [8, nW]]),
                        in_=apd(half, 0, [list(half.ap[0]), [1, nW]]),
                    )

                rd(0)
                rd(1)
                for h in range(8):
                    il(h)
                    if h + 2 < 8:
                        rd(h + 2)
                nc.sync.dma_start(wrp[16:32, :], wrp[0:16, :])

                # ================= gathers ====================================
                def lerp(dst, lo, hi, w):
                    nc.vector.tensor_tensor(out=dst, in0=hi, in1=lo, op=AL.subtract)
                    nc.vector.tensor_tensor(out=dst, in0=dst, in1=w, op=AL.mult)
                    nc.vector.tensor_tensor(out=dst, in0=dst, in1=lo, op=AL.add)

                def wof(l, d, reps, minor):
                    # weight w_bf[:, x, 3l+d] broadcast directly as bf16:
                    # minor=True -> [S, reps] (x outer), else [reps, S]
                    if minor:
                        return apd(w_bf, 3 * l + d, [list(w_bf.ap[0]), [48, S], [0, reps]])
                    return apd(w_bf, 3 * l + d, [list(w_bf.ap[0]), [0, reps], [48, S]])

                def unpack(src_ap, n, tag):
                    ef = workp.tile([P, 2, n], DT.int32, tag=f"ef{tag}", name=f"ef{tag}")
                    nc.vector.tensor_scalar(out=ef[:, 0, :], in0=src_ap, op0=AL.logical_shift_left, scalar1=16, scalar2=None)
                    nc.vector.tensor_scalar(out=ef[:, 1, :], in0=src_ap, op0=AL.bitwise_and, scalar1=_i32(0xFFFF0000), scalar2=None)
                    return ef

                def f32v(t, dims, off=0):
                    return apd(t, off, [list(t.ap[0])] + [list(d) for d in dims]).bitcast(DT.float32)

                # ---- dense levels ----
                def ktree(l, gd, el, ngrp, q, Q, tag):
                    # select k_off window over bits q-1..0; gd lanes
                    # [x*el + g*(Q+1) + t]; returns tile [P, S, ngrp, 2]
                    cur = None
                    for b in range(q - 1, -1, -1):
                        wnew = 2**b + 1 if b > 0 else 2
                        half = 2**b
                        nxt = workp.tile([P, S, ngrp, wnew], DT.int32, tag=f"dt{tag}{b}", name=f"dt{tag}{b}")
                        if cur is None:
                            lo = apd(gd, 0, [list(gd.ap[0]), [el, S], [Q + 1, ngrp], [1, wnew]])
                            hi = apd(gd, half, [list(gd.ap[0]), [el, S], [Q + 1, ngrp], [1, wnew]])
                        else:
                            lo = apd(cur, 0, [list(cur.ap[0]), [cur.ap[1][0], S], [cur.ap[2][0], ngrp], [1, wnew]])
                            hi = apd(cur, half, [list(cur.ap[0]), [cur.ap[1][0], S], [cur.ap[2][0], ngrp], [1, wnew]])
                        mbb = apd(dmb[b], l, [list(dmb[b].ap[0]), [8, S], [0, ngrp], [0, wnew]])
                        nc.vector.tensor_tensor(out=nxt[:], in0=lo, in1=hi, op=AL.bitwise_xor)
                        nc.vector.tensor_tensor(out=nxt[:], in0=nxt[:], in1=mbb, op=AL.bitwise_and)
                        nc.vector.tensor_tensor(out=nxt[:], in0=nxt[:], in1=lo, op=AL.bitwise_xor)
                        cur = nxt
                    return cur

                for l in DENSE_LV:
                    el = ELEM[l]
                    Q = LEVEL_Q[l]
                    q = Q.bit_length() - 1
                    gd = gdd[l % 2]
                    if l == 7:
                        baseA = tabx[SLOT_OFF[7] : SLOT_OFF[7] + N_SLOTS[7], 0:el]
                        baseB = tabx[SLOT_OFF7B : SLOT_OFF7B + N_SLOTS[7], 0:el]
                        gdA, gdB = gdd[0], gdd[1]
                        for gdX, baseX in ((gdA, baseA), (gdB, baseB)):
                            gview = apd(gdX, 0, [list(gdX.ap[0]), [el, S], [1, el]])
                            dma_gather_raw(
                                nc.gpsimd, gview, baseX,
                                wrp[:, l * 8 * S : (l + 1) * 8 * S], S * P, el,
                            )
                        curA = ktree(l, gdA, el, 2, q, Q, "2a")
                        curB = ktree(l, gdB, el, 2, q, Q, "2b")
                        comb = workp.tile([P, S, 8], DT.int32, tag="comb7", name="comb7")
                        nc.vector.tensor_copy(
                            out=apd(comb, 0, [list(comb.ap[0]), [8, S], [1, 4]]),
                            in_=apd(curA, 0, [list(curA.ap[0]), [4, S], [1, 4]]),
                        )
                        nc.vector.tensor_copy(
                            out=apd(comb, 4, [list(comb.ap[0]), [8, S], [1, 4]]),
                            in_=apd(curB, 0, [list(curB.ap[0]), [4, S], [1, 4]]),
                        )
                        csrc = apd(comb, 0, [list(comb.ap[0]), [1, 8 * S]])
                    else:
                        base = tabx[SLOT_OFF[l] : SLOT_OFF[l] + N_SLOTS[l], 0:el]
                        gview = apd(gd, 0, [list(gd.ap[0]), [el, S], [1, el]])
                        dma_gather_raw(
                            nc.gpsimd, gview, base,
                            wrp[:, l * 8 * S : (l + 1) * 8 * S], S * P, el,
                        )
                        if Q == 1:
                            csrc = apd(gd, 0, [list(gd.ap[0]), [1, 8 * S]])
                        else:
                            cur = ktree(l, gd, el, 4, q, Q, "4g")
                            csrc = apd(cur, 0, [list(cur.ap[0]), [1, 8 * S]])
                    # corners in x-major layout: lane x*8 + m; both features fused
                    ef = unpack(csrc, 8 * S, "d")
                    def wv(d, dims):
                        return apd(w_bf, 3 * l + d, [list(w_bf.ap[0])] + dims)
                    efv = lambda off, dims: apd(ef, off, [list(ef.ap[0])] + dims).bitcast(DT.float32)
                    xt = workp.tile([P, 2, S, 4], DT.float32, tag="xtd", name="xtd")
                    lerp(xt[:], efv(0, [[8 * S, 2], [8, S], [1, 4]]), efv(4, [[8 * S, 2], [8, S], [1, 4]]), wv(0, [[0, 2], [48, S], [0, 4]]))
                    yt = workp.tile([P, 2, S, 2], DT.float32, tag="ytd", name="ytd")
                    lerp(yt[:], apd(xt, 0, [list(xt.ap[0]), [4 * S, 2], [4, S], [1, 2]]), apd(xt, 2, [list(xt.ap[0]), [4 * S, 2], [4, S], [1, 2]]), wv(1, [[0, 2], [48, S], [0, 2]]))
                    od = apd(osb, 2 * l, [list(osb.ap[0]), [1, 2], [32, S]])
                    lerp(od, apd(yt, 0, [list(yt.ap[0]), [2 * S, 2], [2, S]]), apd(yt, 1, [list(yt.ap[0]), [2 * S, 2], [2, S]]), wv(2, [[0, 2], [48, S]]))

                # ---- hash levels ----
                for l in HASH_LV:
                    lh = l - 8
                    base = tabx[SLOT_OFF[l] : SLOT_OFF[l] + N_SLOTS[l], 0:16]
                    cls0 = 8 + lh * 8

                    hmv_l = apd(hm, lh * 8 * S, [list(hm.ap[0]), [1, 8 * S]])
                    mbl = {}
                    for b in range(4):
                        mb = workp.tile([P, 8 * S], DT.int32, tag=f"hmk{b}", name=f"hmk{b}")
                        nc.vector.tensor_scalar(out=mb[:], in0=hmv_l, op0=AL.logical_shift_right, scalar1=b, op1=AL.bitwise_and, scalar2=1)
                        nc.vector.tensor_scalar(out=mb[:], in0=mb[:], op0=AL.mult, scalar1=-1, scalar2=None)
                        mbl[b] = mb

                    def hgather(h2):
                        gd = gdh[(4 * lh + h2) % 3]
                        gview = apd(gd, 0, [list(gd.ap[0]), [16, 2 * S], [1, 16]])
                        dma_gather_raw(
                            nc.gpsimd, gview, base,
                            wrp[:, (cls0 + 2 * h2) * 8 * S : (cls0 + 2 * h2 + 2) * 8 * S],
                            2 * S * P, 16,
                        )

                    def hextract(h2):
                        gd = gdh[(4 * lh + h2) % 3]
                        cur = None
                        for b in range(3, -1, -1):
                            half = 2**b
                            if b == 0:
                                hc = hcor[:, h2 * 2 * S : (h2 + 1) * 2 * S]
                                nxt = None
                                ov = apd(hc, 0, [list(hc.ap[0]), [1, 2 * S], [0, 1]])
                            else:
                                nxt = workp.tile([P, 2 * S, half], DT.int32, tag=f"ht{b}", name=f"ht{b}")
                                ov = nxt[:]
                            if cur is None:
                                lo = apd(gd, 0, [list(gd.ap[0]), [16, 2 * S], [1, half]])
                                hi = apd(gd, half, [list(gd.ap[0]), [16, 2 * S], [1, half]])
                            else:
                                lo = cur[:, :, 0:half]
                                hi = cur[:, :, half : 2 * half]
                            mb = mbl[b]
                            mbb = apd(mb, h2 * 2 * S, [list(mb.ap[0]), [1, 2 * S], [0, half]])
                            nc.vector.tensor_tensor(out=ov, in0=lo, in1=hi, op=AL.bitwise_xor)
                            nc.vector.tensor_tensor(out=ov, in0=ov, in1=mbb, op=AL.bitwise_and)
                            nc.vector.tensor_tensor(out=ov, in0=ov, in1=lo, op=AL.bitwise_xor)
                            cur = nxt

                    hgather(0)
                    hgather(1)
                    hextract(0)
                    hgather(2)
                    hextract(1)
                    hgather(3)
                    hextract(2)
                    hextract(3)
                    # hcor: [P, 8S] corner values, class-major (c*S + x); fused feats
                    ef = unpack(hcor[:, :], 8 * S, "h")
                    def wvh(d, reps):
                        return apd(w_bf, 3 * l + d, [list(w_bf.ap[0]), [0, 2], [0, reps], [48, S]])
                    efvh = lambda off, n: apd(ef, off, [list(ef.ap[0]), [8 * S, 2], [1, n]]).bitcast(DT.float32)
                    xh = workp.tile([P, 2, 4 * S], DT.float32, tag="xth", name="xth")
                    lerp(xh[:], efvh(0, 4 * S), efvh(4 * S, 4 * S), wvh(0, 4))
                    yh = workp.tile([P, 2, 2 * S], DT.float32, tag="yth", name="yth")
                    lerp(yh[:], apd(xh, 0, [list(xh.ap[0]), [4 * S, 2], [1, 2 * S]]), apd(xh, 2 * S, [list(xh.ap[0]), [4 * S, 2], [1, 2 * S]]), wvh(1, 2))
                    oh = apd(osb, 2 * l, [list(osb.ap[0]), [1, 2], [32, S]])
                    lerp(oh, apd(yh, 0, [list(yh.ap[0]), [2 * S, 2], [1, S]]), apd(yh, S, [list(yh.ap[0]), [2 * S, 2], [1, S]]), wvh(2, 1))

                nc.gpsimd.dma_start(out[:, bass.ds(st, S), :], osb[:])
    nc.compile()
    return nc


# ---------------- host side ----------------

def _pack_bf16(t):
    u = np.ascontiguousarray(t.astype(np.float32)).view(np.uint32).astype(np.uint64)
    r = ((u + 0x7FFF + ((u >> 16) & 1)) >> 16).astype(np.uint32)
    return r[:, 0] | (r[:, 1] << np.uint32(16))


def build_tabx(tables):
    pk = _pack_bf16(tables)
    tabx = np.zeros((TOT_SLOTS, 64), dtype=np.uint32)
    for l in DENSE_LV:
        r, Q, M = RES[l], LEVEL_Q[l], LEVEL_M[l]
        o = SLOT_OFF[l]
        rp1 = r + 1
        seg = pk[OFFS[l] : OFFS[l + 1]]
        if Q == 1:
            ii, jj, kk = np.meshgrid(np.arange(r), np.arange(r), np.arange(r), indexing="ij")
            # reference indexes the (r+1)^3 table with strides r^2, r, 1
            n0 = ((ii * r + jj) * r + kk).ravel()
            lane = 0
            for di in (0, 1):
                for dj in (0, 1):
                    for dk in (0, 1):
                        tabx[o : o + r * r * r, lane] = seg[n0 + di * r * r + dj * r + dk]
                        lane += 1
        else:
            ii, jj, mm = np.meshgrid(np.arange(r), np.arange(r), np.arange(M), indexing="ij")
            if l == 7:
                for di in (0, 1):
                    oo = o if di == 0 else SLOT_OFF7B
                    lane = 0
                    for dj in (0, 1):
                        for t in range(Q + 1):
                            kidx = np.minimum(mm * Q + t, r)
                            row = ((ii + di) * r + (jj + dj)) * r + kidx
                            src = np.where(mm * Q + t <= r, seg[row], 0)
                            tabx[oo : oo + r * r * M, lane] = src.ravel()
                            lane += 1
            else:
                lane = 0
                for di in (0, 1):
                    for dj in (0, 1):
                        for t in range(Q + 1):
                            kidx = np.minimum(mm * Q + t, r)
                            row = ((ii + di) * r + (jj + dj)) * r + kidx
                            src = np.where(mm * Q + t <= r, seg[row], 0)
                            tabx[o : o + r * r * M, lane] = src.ravel()
                            lane += 1
    for l in HASH_LV:
        o, ns = SLOT_OFF[l], N_SLOTS[l]
        seg = pk[OFFS[l] : OFFS[l + 1]]
        tabx[o : o + ns, 0:16] = seg.reshape(ns, 16)
    return tabx.view(np.int32)


def build_cvec():
    cv = np.zeros(128, dtype=np.float32)
    for l in range(N_LEVELS):
        grid = np.float32(2.0) / np.float32(RES[l])
        cv[l] = np.float32(1.0) / grid
        cv[16 + l] = np.float32(RES[l] - 1)
        cv[32 + l] = grid
    for l in DENSE_LV:
        r, Q, M = RES[l], LEVEL_Q[l], LEVEL_M[l]
        cv[48 + l] = np.float32(r * M)
        cv[56 + l] = np.float32(M)
        cv[64 + l] = np.float32(1.0 / Q)
        cv[72 + l] = np.float32(Q)
    return cv.reshape(1, 128)


_NC_CACHE = {}
TRACE = False
LAST_NS = None


def _get_nc(slots, S):
    key = (slots, S)
    if key not in _NC_CACHE:
        _NC_CACHE[key] = build_kernel(slots, S)
    return _NC_CACHE[key]


def kernel(x: np.ndarray, tables: np.ndarray) -> np.ndarray:
    global LAST_NS
    from concourse.bass_utils import run_bass_kernel_spmd

    B = x.shape[0]
    per_core = B // N_CORES
    slots = per_core // P
    S = min(32, slots)
    nc = _get_nc(slots, S)
    tabx = build_tabx(tables)
    cv = build_cvec()
    in_maps = []
    for c in range(N_CORES):
        xs = np.ascontiguousarray(
            x[c * per_core : (c + 1) * per_core].reshape(P, slots, 3)
        ).astype(np.float32)
        in_maps.append({"x": xs, "tabx": tabx, "cvec": cv})
    kw = {"trace": True} if TRACE else {}
    res = run_bass_kernel_spmd(nc, in_maps, core_ids=list(range(N_CORES)), **kw)
    LAST_NS = res.exec_time_ns
    outs = [res.results[c]["out"].reshape(per_core, 32) for c in range(N_CORES)]
    return np.concatenate(outs, axis=0).astype(np.float32)


# revision 25
# speedup vs baseline: 1.1684x; 1.0048x over previous
"""HashEmbedder3D Trainium2 kernel v6.

Key changes vs v2 baseline:
- dma_gather with single_packet=False + indices replicated across all 8
  16-partition groups allows 8192-idx instructions (994ns fixed cost
  amortized 8x).
- Dense levels 0-3: slot-per-voxel-base block tables (one 32B descriptor
  fetches all 8 corners; no select trees). Levels 4-7: Q-packed slots
  (one descriptor + small k-offset select tree).
- Hash levels 8-15: one 64B slot read per corner (8 classes), extraction
  via 4-round select tree, gathers batched 2 classes per instruction.
- Index wrap/transpose done by DRAM-bounce DMA + DVE interleave + SBUF
  broadcast DMAs; Pool engine only runs SWDGE gathers.
"""
import math
import sys

import numpy as np

sys.path.insert(0, "/opt/trn_rl_repo")

from concourse import bacc, bass, mybir
import concourse.tile as tile

N_LEVELS = 16
F = 2
LOG2_T = 19
T = 1 << LOG2_T
BASE, FINEST = 16, 512
B_GROWTH = float(np.exp((np.log(np.float32(FINEST)) - np.log(np.float32(BASE))) / np.float32(N_LEVELS - 1)))
RES = [math.floor(BASE * B_GROWTH**i) for i in range(N_LEVELS)]
SIZES = [(r + 1) ** 3 if r**3 < T else T for r in RES]
OFFS = np.concatenate([[0], np.cumsum(SIZES)]).tolist()
TOTAL_ROWS = OFFS[-1]
PRIMES = [1, 2654435761, 805459861]
N_POINTS = 1048576
N_CORES = 8
P = 128

DT = mybir.dt
AL = mybir.AluOpType

DENSE_LV = list(range(8))
HASH_LV = list(range(8, 16))

# dense level slot geometry: levels 0-3 slot-per-base, 4-7 Q-packed
LEVEL_Q = {0: 1, 1: 1, 2: 1, 3: 1, 4: 2, 5: 4, 6: 8, 7: 16}
LEVEL_M = {}
N_SLOTS, ELEM = {}, {}
for l in DENSE_LV:
    r, Q = RES[l], LEVEL_Q[l]
    M = -(-r // Q)
    LEVEL_M[l] = M
    N_SLOTS[l] = r * r * M
    ELEM[l] = 8 if Q == 1 else 4 * (Q + 1)
ELEM[7] = 2 * (LEVEL_Q[7] + 1)  # level 7 split in two di-halves of 2x17 lanes
for l in HASH_LV:
    N_SLOTS[l], ELEM[l] = T // 16, 16
SLOT_OFF = {}
_a = 0
for l in range(N_LEVELS):
    SLOT_OFF[l] = _a
    _a += N_SLOTS[l]
SLOT_OFF7B = _a  # second (di=1) half of level 7
_a += N_SLOTS[7]
TOT_SLOTS = _a
assert all(N_SLOTS[l] <= 32768 for l in range(N_LEVELS)), N_SLOTS

# class layout: dense levels are classes 0-7; hash level l corner m is
# class 8 + (l-8)*8 + m. Each class contributes S indices per tile.
NCC = 8 + 8 * 8  # 72


def _i32(v):
    return int(np.int32(np.uint32(v)))


MAX_GIDX = 8192


def dma_gather_raw(eng, out_ap, in_ap, idxs_ap, num_idxs, elem_size, elem_step=64):
    stride_bytes = elem_step * 4
    assert stride_bytes % 256 == 0
    _in_ap = eng.lower_ap_dma(in_ap, for_custom_bir_dma=True)
    _idxs_ap = eng.lower_ap(idxs_ap)
    _out_ap = eng.lower_ap(out_ap)
    return eng.add_instruction(
        mybir.InstDMAGatherAnt(
            name=eng.bass.get_next_instruction_name(),
            ins=[*_in_ap, _idxs_ap, eng.lower_val_access(eng.to_reg(num_idxs))],
            outs=[_out_ap],
            transpose=False,
            num_idxs=num_idxs,
            elem_size=elem_size,
            stride_bytes_256=stride_bytes // 256,
            gen_mode=0,
            single_packet=num_idxs <= 1024,
            queue_num=0,
            sbuf_tokens_per_rank=0,
            sbuf_free_dim_per_rank=0,
            sbuf_free_dim_pad_per_rank=0,
            sbuf_byte_offset=0,
        )
    )


def apd(tap, off, dims):
    return bass.AP(tap.tensor, tap.offset + off, [list(d) for d in dims])


def build_kernel(slots_total, S):
    n_outer = slots_total // S
    assert n_outer * S == slots_total

    nc = bacc.Bacc(None, target_bir_lowering=False, debug=False)
    x_in = nc.dram_tensor("x", [P, slots_total, 3], DT.float32, kind="ExternalInput")
    tabx = nc.dram_tensor("tabx", [TOT_SLOTS, 64], DT.int32, kind="ExternalInput")
    cvec_in = nc.dram_tensor("cvec", [1, 128], DT.float32, kind="ExternalInput")
    out = nc.dram_tensor("out", [P, slots_total, 32], DT.float32, kind="ExternalOutput")

    nW = NCC * S  # idx ints per partition per tile

    with tile.TileContext(nc) as tc:
        with (
            tc.tile_pool(name="big", bufs=1) as bigp,
            tc.tile_pool(name="dbl", bufs=1) as dblp,
            tc.tile_pool(name="work", bufs=1) as workp,
        ):
            cv = bigp.tile([P, 128], DT.float32, tag="cv", name="cv")
            nc.sync.dma_start(cv[:], apd(cvec_in[:], 0, [[0, P], [1, 128]]))

            def cvb3(col, n, w=8):
                # [P, n, w] view of per-level const at cv[col:col+w]
                return apd(cv, col, [list(cv.ap[0]), [0, n], [1, w]])

            def cvb4(col, a, b, w=8):
                return apd(cv, col, [list(cv.ap[0]), [0, a], [0, b], [1, w]])

            x_t = bigp.tile([P, S, 3], DT.float32, tag="x_t", name="x_t")
            w_bf = None
            idxb = bigp.tile([P, nW], DT.int16, tag="idxb", name="idxb")
            hm = None
            koffs = None
            scr = bigp.tile([P, nW], DT.int16, tag="scr", name="scr", space="DRAM")
            wt = bigp.tile([P, 2 * nW], DT.int16, tag="wt", name="wt")
            wrp = None
            gdd = [
                bigp.tile([P, 68 * S], DT.int32, tag=f"gdd{i}", name=f"gdd{i}")
                for i in range(2)
            ]
            gdh = [
                bigp.tile([P, 2 * 16 * S], DT.int32, tag=f"gdh{i}", name=f"gdh{i}")
                for i in range(3)
            ]
            hcor = bigp.tile([P, 8 * S], DT.int32, tag="hcor", name="hcor")
            osb = bigp.tile([P, S, 32], DT.float32, tag="osb", name="osb")
            bli_d = bigp.tile([P, S, 3, 8], DT.int32, tag="bli_d", name="bli_d")
            bli_h = bigp.tile([P, S, 3, 8], DT.int32, tag="bli_h", name="bli_h")

            nc.vector.memset(wt[:], 0)
            _wrp0 = dblp.tile([P, 8 * nW], DT.int16, tag="wrp", name="wrp_init")
            nc.vector.memset(_wrp0[:], 0)

            with tc.For_i(
                0,
                slots_total,
                S,
                hint_engines=(mybir.EngineType.DVE, mybir.EngineType.Pool),
            ) as st:
                nc.sync.dma_start(x_t[:], x_in[:, bass.ds(st, S), :])
                w_bf = dblp.tile([P, S, 48], DT.bfloat16, tag="w_bf", name="w_bf")
                hm = dblp.tile([P, 64 * S], DT.int32, tag="hm", name="hm")
                koffs = dblp.tile([P, S, 8], DT.int32, tag="koffs", name="koffs")
                wrp = dblp.tile([P, 8 * nW], DT.int16, tag="wrp", name="wrp")

                # ================= phase 1: voxel coords + weights ============
                xc = x_t

                for half, lv0 in ((0, 0), (1, 8)):
                    bli = bli_d if half == 0 else bli_h
                    tf = workp.tile([P, S, 3, 8], DT.float32, tag="tf", name="tf")
                    fi = workp.tile([P, S, 3, 8], DT.int32, tag="fi", name="fi")
                    ff = workp.tile([P, S, 3, 8], DT.float32, tag="ff", name="ff")
                    blf = workp.tile([P, S, 3, 8], DT.float32, tag="blf", name="blf")
                    su = workp.tile([P, S, 3, 8], DT.float32, tag="su", name="su")
                    xb = apd(xc, 0, [list(xc.ap[0]), [3, S], [1, 3], [0, 8]])
                    xbu = apd(x_t, 0, [list(x_t.ap[0]), [3, S], [1, 3], [0, 8]])
                    nc.vector.tensor_scalar(out=tf[:], in0=xb, op0=AL.add, scalar1=1.0, scalar2=None)
                    nc.vector.tensor_tensor(out=tf[:], in0=tf[:], in1=cvb4(lv0, S, 3), op=AL.mult)
                    nc.vector.tensor_copy(out=fi[:], in_=tf[:])
                    nc.vector.tensor_copy(out=ff[:], in_=fi[:])
                    nc.vector.tensor_tensor(out=blf[:], in0=ff[:], in1=tf[:], op=AL.is_gt)
                    nc.vector.tensor_tensor(out=blf[:], in0=ff[:], in1=blf[:], op=AL.subtract)
                    nc.vector.tensor_tensor(out=blf[:], in0=blf[:], in1=cvb4(16 + lv0, S, 3), op=AL.min)
                    nc.vector.tensor_copy(out=bli[:], in_=blf[:])
                    # x in [-1,1) => w = (x+1)/grid - bl = tf - blf exactly
                    nc.vector.tensor_tensor(out=su[:], in0=tf[:], in1=blf[:], op=AL.subtract)
                    wdst = apd(w_bf, 3 * lv0, [list(w_bf.ap[0]), [48, S], [1, 3], [3, 8]])
                    nc.vector.tensor_copy(out=wdst, in_=su[:])

                # ================= dense slot ids =============================
                i_ = bli_d[:, :, 0, :]
                j_ = bli_d[:, :, 1, :]
                k_ = bli_d[:, :, 2, :]
                kq = workp.tile([P, S, 8], DT.int32, tag="kq", name="kq")
                sid = workp.tile([P, S, 8], DT.int32, tag="sid", name="sid")
                t1 = workp.tile([P, S, 8], DT.int32, tag="t1d", name="t1d")
                nc.vector.tensor_copy(out=kq[:], in_=k_)
                for l in range(4, 8):
                    q = LEVEL_Q[l].bit_length() - 1
                    nc.vector.tensor_scalar(out=kq[:, :, l], in0=k_[:, :, l], op0=AL.logical_shift_right, scalar1=q, scalar2=None)
                # koff = k - kq*Q  (only levels 4-7 used)
                nc.vector.tensor_tensor(out=koffs[:], in0=kq[:], in1=apd(cv, 72, [list(cv.ap[0]), [0, S], [1, 8]]), op=AL.mult)
                nc.vector.tensor_tensor(out=koffs[:], in0=k_, in1=koffs[:], op=AL.subtract)
                # sid = i*A + j*B + kq
                nc.vector.tensor_tensor(out=t1[:], in0=i_, in1=apd(cv, 48, [list(cv.ap[0]), [0, S], [1, 8]]), op=AL.mult)
                nc.vector.tensor_tensor(out=sid[:], in0=j_, in1=apd(cv, 56, [list(cv.ap[0]), [0, S], [1, 8]]), op=AL.mult)
                nc.vector.tensor_tensor(out=sid[:], in0=sid[:], in1=t1[:], op=AL.add)
                nc.vector.tensor_tensor(out=sid[:], in0=sid[:], in1=kq[:], op=AL.add)
                # write dense classes: idxb[:, l*S + x] = sid[:, x, l]
                nc.vector.tensor_copy(
                    out=apd(idxb, 0, [list(idxb.ap[0]), [1, S], [S, 8]]),
                    in_=sid[:],
                )
                # dense k_off select masks, all levels at once, per bit
                dmb = []
                for b in range(4):
                    mbt = workp.tile([P, S, 8], DT.int32, tag=f"dmb{b}", name=f"dmb{b}")
                    nc.vector.tensor_scalar(out=mbt[:], in0=koffs[:], op0=AL.logical_shift_right, scalar1=b, op1=AL.bitwise_and, scalar2=1)
                    nc.vector.tensor_scalar(out=mbt[:], in0=mbt[:], op0=AL.mult, scalar1=-1, scalar2=None)
                    dmb.append(mbt)

                # ================= hash slot ids ==============================
                ih = bli_h[:, :, 0, :]
                jh = bli_h[:, :, 1, :]
                kh = bli_h[:, :, 2, :]
                mt1 = workp.tile([P, S, 8], DT.int32, tag="mt1", name="mt1")
                mt2 = workp.tile([P, S, 8], DT.int32, tag="mt2", name="mt2")
                mt3 = workp.tile([P, S, 8], DT.int32, tag="mt3", name="mt3")

                def ts(o, i, op, s):
                    nc.vector.tensor_scalar(out=o, in0=i, op0=op, scalar1=s, scalar2=None)

                def tt(o, a, b, op):
                    nc.vector.tensor_tensor(out=o, in0=a, in1=b, op=op)

                def mul32(dst, src, prime):
                    Hp, Lp = (prime >> 16) & 0xFFFF, prime & 0xFFFF
                    Hs = Hp - 32768 if Hp >= 32768 else Hp
                    ts(mt1[:], src, AL.mult, Lp)
                    ts(mt2[:], src, AL.mult, Hs)
                    if Hp >= 32768:
                        ts(mt3[:], src, AL.logical_shift_left, 15)
                        ts(mt3[:], mt3[:], AL.bitwise_and, 0xFFFF)
                        ts(mt2[:], mt2[:], AL.bitwise_and, 0xFFFF)
                        tt(mt2[:], mt2[:], mt3[:], AL.add)
                    ts(mt2[:], mt2[:], AL.bitwise_and, 0xFFFF)
                    ts(mt3[:], mt1[:], AL.logical_shift_right, 16)
                    tt(mt2[:], mt2[:], mt3[:], AL.add)
                    ts(mt2[:], mt2[:], AL.bitwise_and, 0xFFFF)
                    ts(mt2[:], mt2[:], AL.logical_shift_left, 16)
                    ts(mt1[:], mt1[:], AL.bitwise_and, 0xFFFF)
                    tt(dst, mt2[:], mt1[:], AL.bitwise_or)

                def add32(dst, src, const):
                    cl, ch = const & 0xFFFF, (const >> 16) & 0xFFFF
                    ts(mt1[:], src, AL.bitwise_and, 0xFFFF)
                    ts(mt1[:], mt1[:], AL.add, cl)
                    ts(mt2[:], src, AL.logical_shift_right, 16)
                    ts(mt2[:], mt2[:], AL.bitwise_and, 0xFFFF)
                    ts(mt2[:], mt2[:], AL.add, ch)
                    ts(mt3[:], mt1[:], AL.logical_shift_right, 16)
                    tt(mt2[:], mt2[:], mt3[:], AL.add)
                    ts(mt2[:], mt2[:], AL.bitwise_and, 0xFFFF)
                    ts(mt2[:], mt2[:], AL.logical_shift_left, 16)
                    ts(mt1[:], mt1[:], AL.bitwise_and, 0xFFFF)
                    tt(dst, mt2[:], mt1[:], AL.bitwise_or)

                jp0 = workp.tile([P, S, 8], DT.int32, tag="jp0", name="jp0")
                jp1 = workp.tile([P, S, 8], DT.int32, tag="jp1", name="jp1")
                kp0 = workp.tile([P, S, 8], DT.int32, tag="kp0", name="kp0")
                kp1 = workp.tile([P, S, 8], DT.int32, tag="kp1", name="kp1")
                ii1 = workp.tile([P, S, 8], DT.int32, tag="ii1", name="ii1")
                rr = workp.tile([P, S, 8], DT.int32, tag="rr", name="rr")
                rr2 = workp.tile([P, S, 8], DT.int32, tag="rr2", name="rr2")
                mul32(jp0[:], jh, PRIMES[1])
                add32(jp1[:], jp0[:], PRIMES[1])
                mul32(kp0[:], kh, PRIMES[2])
                add32(kp1[:], kp0[:], PRIMES[2])
                ts(ii1[:], ih, AL.add, 1)
                # vectorized corner ids: xab[c] = (ih|ii1) ^ (jp0|jp1), c=di*2+dj
                xab = workp.tile([P, S, 4, 8], DT.int32, tag="xab", name="xab")
                tt(xab[:, :, 0, :], ih, jp0[:], AL.bitwise_xor)
                tt(xab[:, :, 1, :], ih, jp1[:], AL.bitwise_xor)
                tt(xab[:, :, 2, :], ii1[:], jp0[:], AL.bitwise_xor)
                tt(xab[:, :, 3, :], ii1[:], jp1[:], AL.bitwise_xor)
                # rr_all[x, m, l] with m = c*2+dk
                rr_all = workp.tile([P, S, 8, 8], DT.int32, tag="rr_all", name="rr_all")
                for dk in (0, 1):
                    tt(
                        apd(rr_all, dk * 8, [list(rr_all.ap[0]), [64, S], [16, 4], [1, 8]]),
                        apd(xab, 0, [list(xab.ap[0]), [32, S], [8, 4], [1, 8]]),
                        (kp1 if dk else kp0)[:].to_broadcast([P, S, 4, 8]) if False else apd(kp1 if dk else kp0, 0, [list(kp0.ap[0]), [8, S], [0, 4], [1, 8]]),
                        AL.bitwise_xor,
                    )
                ts(rr_all[:], rr_all[:], AL.bitwise_and, T - 1)
                sh = workp.tile([P, S, 8, 8], DT.int32, tag="rrsh", name="rrsh")
                ts(sh[:], rr_all[:], AL.logical_shift_right, 4)
                nc.vector.tensor_copy(
                    out=apd(idxb, 8 * S, [list(idxb.ap[0]), [1, S], [S, 8], [8 * S, 8]]),
                    in_=sh[:],
                )
                ts(sh[:], rr_all[:], AL.bitwise_and, 15)
                nc.vector.tensor_copy(
                    out=apd(hm, 0, [list(hm.ap[0]), [1, S], [S, 8], [8 * S, 8]]),
                    in_=sh[:],
                )

                # ============== idx transpose to wrapped-16 + replicate =======
                nc.sync.dma_start(scr[:], idxb[:])
                # Only partitions 16:32 (group 1) are read by the SWDGE
                # cores (each core g reads cols == g mod 8 there). Build the
                # wrapped matrix in group 1; mirror to group 0 for CoreSim,
                # whose interpreter consumes group 0.
                # 8 ping-ponged chunks: DVE only stalls on the first read
                def rd(h):
                    half = wt[:, (h % 2) * nW : (h % 2 + 1) * nW]
                    nc.sync.dma_start(
                        half[0:16, :],
                        apd(scr, h * 16 * nW, [[nW, 16], [1, nW]]),
                    )

                def il(h):
                    half = wt[:, (h % 2) * nW : (h % 2 + 1) * nW]
                    nc.vector.tensor_copy(
                        out=apd(wrp, h, [list(wrp.ap[0]), [8, nW]]),
                        in_=apd(half, 0, [list(half.ap[0]), [1, nW]]),
                    )

                rd(0)
                rd(1)
                for h in range(8):
                    il(h)
                    if h + 2 < 8:
                        rd(h + 2)
                nc.sync.dma_start(wrp[16:32, :], wrp[0:16, :])

                # ================= gathers ====================================
                def lerp(dst, lo, hi, w):
                    nc.vector.tensor_tensor(out=dst, in0=hi, in1=lo, op=AL.subtract)
                    nc.vector.tensor_tensor(out=dst, in0=dst, in1=w, op=AL.mult)
                    nc.vector.tensor_tensor(out=dst, in0=dst, in1=lo, op=AL.add)

                def wof(l, d, reps, minor):
                    # weight w_bf[:, x, 3l+d] broadcast directly as bf16:
                    # minor=True -> [S, reps] (x outer), else [reps, S]
                    if minor:
                        return apd(w_bf, 3 * l + d, [list(w_bf.ap[0]), [48, S], [0, reps]])
                    return apd(w_bf, 3 * l + d, [list(w_bf.ap[0]), [0, reps], [48, S]])

                def unpack(src_ap, n, tag):
                    ef = workp.tile([P, 2, n], DT.int32, tag=f"ef{tag}", name=f"ef{tag}")
                    nc.vector.tensor_scalar(out=ef[:, 0, :], in0=src_ap, op0=AL.logical_shift_left, scalar1=16, scalar2=None)
                    nc.vector.tensor_scalar(out=ef[:, 1, :], in0=src_ap, op0=AL.bitwise_and, scalar1=_i32(0xFFFF0000), scalar2=None)
                    return ef

                def f32v(t, dims, off=0):
                    return apd(t, off, [list(t.ap[0])] + [list(d) for d in dims]).bitcast(DT.float32)

                # ---- dense levels ----
                def ktree(l, gd, el, ngrp, q, Q, tag):
                    # select k_off window over bits q-1..0; gd lanes
                    # [x*el + g*(Q+1) + t]; returns tile [P, S, ngrp, 2]
                    cur = None
                    for b in range(q - 1, -1, -1):
                        wnew = 2**b + 1 if b > 0 else 2
                        half = 2**b
                        nxt = workp.tile([P, S, ngrp, wnew], DT.int32, tag=f"dt{tag}{b}", name=f"dt{tag}{b}")
                        if cur is None:
                            lo = apd(gd, 0, [list(gd.ap[0]), [el, S], [Q + 1, ngrp], [1, wnew]])
                            hi = apd(gd, half, [list(gd.ap[0]), [el, S], [Q + 1, ngrp], [1, wnew]])
                        else:
                            lo = apd(cur, 0, [list(cur.ap[0]), [cur.ap[1][0], S], [cur.ap[2][0], ngrp], [1, wnew]])
                            hi = apd(cur, half, [list(cur.ap[0]), [cur.ap[1][0], S], [cur.ap[2][0], ngrp], [1, wnew]])
                        mbb = apd(dmb[b], l, [list(dmb[b].ap[0]), [8, S], [0, ngrp], [0, wnew]])
                        nc.vector.tensor_tensor(out=nxt[:], in0=lo, in1=hi, op=AL.bitwise_xor)
                        nc.vector.tensor_tensor(out=nxt[:], in0=nxt[:], in1=mbb, op=AL.bitwise_and)
                        nc.vector.tensor_tensor(out=nxt[:], in0=nxt[:], in1=lo, op=AL.bitwise_xor)
                        cur = nxt
                    return cur

                for l in DENSE_LV:
                    el = ELEM[l]
                    Q = LEVEL_Q[l]
                    q = Q.bit_length() - 1
                    gd = gdd[l % 2]
                    if l == 7:
                        baseA = tabx[SLOT_OFF[7] : SLOT_OFF[7] + N_SLOTS[7], 0:el]
                        baseB = tabx[SLOT_OFF7B : SLOT_OFF7B + N_SLOTS[7], 0:el]
                        gdA, gdB = gdd[0], gdd[1]
                        for gdX, baseX in ((gdA, baseA), (gdB, baseB)):
                            gview = apd(gdX, 0, [list(gdX.ap[0]), [el, S], [1, el]])
                            dma_gather_raw(
                                nc.gpsimd, gview, baseX,
                                wrp[:, l * 8 * S : (l + 1) * 8 * S], S * P, el,
                            )
                        curA = ktree(l, gdA, el, 2, q, Q, "2a")
                        curB = ktree(l, gdB, el, 2, q, Q, "2b")
                        comb = workp.tile([P, S, 8], DT.int32, tag="comb7", name="comb7")
                        nc.vector.tensor_copy(
                            out=apd(comb, 0, [list(comb.ap[0]), [8, S], [1, 4]]),
                            in_=apd(curA, 0, [list(curA.ap[0]), [4, S], [1, 4]]),
                        )
                        nc.vector.tensor_copy(
                            out=apd(comb, 4, [list(comb.ap[0]), [8, S], [1, 4]]),
                            in_=apd(curB, 0, [list(curB.ap[0]), [4, S], [1, 4]]),
                        )
                        csrc = apd(comb, 0, [list(comb.ap[0]), [1, 8 * S]])
                    else:
                        base = tabx[SLOT_OFF[l] : SLOT_OFF[l] + N_SLOTS[l], 0:el]
                        gview = apd(gd, 0, [list(gd.ap[0]), [el, S], [1, el]])
                        dma_gather_raw(
                            nc.gpsimd, gview, base,
                            wrp[:, l * 8 * S : (l + 1) * 8 * S], S * P, el,
                        )
                        if Q == 1:
                            csrc = apd(gd, 0, [list(gd.ap[0]), [1, 8 * S]])
                        else:
                            cur = ktree(l, gd, el, 4, q, Q, "4g")
                            csrc = apd(cur, 0, [list(cur.ap[0]), [1, 8 * S]])
                    # corners in x-major layout: lane x*8 + m; both features fused
                    ef = unpack(csrc, 8 * S, "d")
                    def wv(d, dims):
                        return apd(w_bf, 3 * l + d, [list(w_bf.ap[0])] + dims)
                    efv = lambda off, dims: apd(ef, off, [list(ef.ap[0])] + dims).bitcast(DT.float32)
                    xt = workp.tile([P, 2, S, 4], DT.float32, tag="xtd", name="xtd")
                    lerp(xt[:], efv(0, [[8 * S, 2], [8, S], [1, 4]]), efv(4, [[8 * S, 2], [8, S], [1, 4]]), wv(0, [[0, 2], [48, S], [0, 4]]))
                    yt = workp.tile([P, 2, S, 2], DT.float32, tag="ytd", name="ytd")
                    lerp(yt[:], apd(xt, 0, [list(xt.ap[0]), [4 * S, 2], [4, S], [1, 2]]), apd(xt, 2, [list(xt.ap[0]), [4 * S, 2], [4, S], [1, 2]]), wv(1, [[0, 2], [48, S], [0, 2]]))
                    od = apd(osb, 2 * l, [list(osb.ap[0]), [1, 2], [32, S]])
                    lerp(od, apd(yt, 0, [list(yt.ap[0]), [2 * S, 2], [2, S]]), apd(yt, 1, [list(yt.ap[0]), [2 * S, 2], [2, S]]), wv(2, [[0, 2], [48, S]]))

                # ---- hash levels ----
                for l in HASH_LV:
                    lh = l - 8
                    base = tabx[SLOT_OFF[l] : SLOT_OFF[l] + N_SLOTS[l], 0:16]
                    cls0 = 8 + lh * 8

                    hmv_l = apd(hm, lh * 8 * S, [list(hm.ap[0]), [1, 8 * S]])
                    mbl = {}
                    for b in range(4):
                        mb = workp.tile([P, 8 * S], DT.int32, tag=f"hmk{b}", name=f"hmk{b}")
                        nc.vector.tensor_scalar(out=mb[:], in0=hmv_l, op0=AL.logical_shift_right, scalar1=b, op1=AL.bitwise_and, scalar2=1)
                        nc.vector.tensor_scalar(out=mb[:], in0=mb[:], op0=AL.mult, scalar1=-1, scalar2=None)
                        mbl[b] = mb

                    def hgather(h2):
                        gd = gdh[(4 * lh + h2) % 3]
                        gview = apd(gd, 0, [list(gd.ap[0]), [16, 2 * S], [1, 16]])
                        dma_gather_raw(
                            nc.gpsimd, gview, base,
                            wrp[:, (cls0 + 2 * h2) * 8 * S : (cls0 + 2 * h2 + 2) * 8 * S],
                            2 * S * P, 16,
                        )

                    def hextract(h2):
                        gd = gdh[(4 * lh + h2) % 3]
                        cur = None
                        for b in range(3, -1, -1):
                            half = 2**b
                            if b == 0:
                                hc = hcor[:, h2 * 2 * S : (h2 + 1) * 2 * S]
                                nxt = None
                                ov = apd(hc, 0, [list(hc.ap[0]), [1, 2 * S], [0, 1]])
                            else:
                                nxt = workp.tile([P, 2 * S, half], DT.int32, tag=f"ht{b}", name=f"ht{b}")
                                ov = nxt[:]
                            if cur is None:
                                lo = apd(gd, 0, [list(gd.ap[0]), [16, 2 * S], [1, half]])
                                hi = apd(gd, half, [list(gd.ap[0]), [16, 2 * S], [1, half]])
                            else:
                                lo = cur[:, :, 0:half]
                                hi = cur[:, :, half : 2 * half]
                            mb = mbl[b]
                            mbb = apd(mb, h2 * 2 * S, [list(mb.ap[0]), [1, 2 * S], [0, half]])
                            nc.vector.tensor_tensor(out=ov, in0=lo, in1=hi, op=AL.bitwise_xor)
                            nc.vector.tensor_tensor(out=ov, in0=ov, in1=mbb, op=AL.bitwise_and)
                            nc.vector.tensor_tensor(out=ov, in0=ov, in1=lo, op=AL.bitwise_xor)
                            cur = nxt

                    hgather(0)
                    hgather(1)
                    hextract(0)
                    hgather(2)
                    hextract(1)
                    hgather(3)
                    hextract(2)
                    hextract(3)
                    # hcor: [P, 8S] corner values, class-major (c*S + x); fused feats
                    ef = unpack(hcor[:, :], 8 * S, "h")
                    def wvh(d, reps):
                        return apd(w_bf, 3 * l + d, [list(w_bf.ap[0]), [0, 2], [0, reps], [48, S]])
                    efvh = lambda off, n: apd(ef, off, [list(ef.ap[0]), [8 * S, 2], [1, n]]).bitcast(DT.float32)
                    xh = workp.tile([P, 2, 4 * S], DT.float32, tag="xth", name="xth")
                    lerp(xh[:], efvh(0, 4 * S), efvh(4 * S, 4 * S), wvh(0, 4))
                    yh = workp.tile([P, 2, 2 * S], DT.float32, tag="yth", name="yth")
                    lerp(yh[:], apd(xh, 0, [list(xh.ap[0]), [4 * S, 2], [1, 2 * S]]), apd(xh, 2 * S, [list(xh.ap[0]), [4 * S, 2], [1, 2 * S]]), wvh(1, 2))
                    oh = apd(osb, 2 * l, [list(osb.ap[0]), [1, 2], [32, S]])
                    lerp(oh, apd(yh, 0, [list(yh.ap[0]), [2 * S, 2], [1, S]]), apd(yh, S, [list(yh.ap[0]), [2 * S, 2], [1, S]]), wvh(2, 1))

                nc.gpsimd.dma_start(out[:, bass.ds(st, S), :], osb[:])
    nc.compile()
    return nc


# ---------------- host side ----------------

def _pack_bf16(t):
    u = np.ascontiguousarray(t.astype(np.float32)).view(np.uint32).astype(np.uint64)
    r = ((u + 0x7FFF + ((u >> 16) & 1)) >> 16).astype(np.uint32)
    return r[:, 0] | (r[:, 1] << np.uint32(16))


def build_tabx(tables):
    pk = _pack_bf16(tables)
    tabx = np.zeros((TOT_SLOTS, 64), dtype=np.uint32)
    for l in DENSE_LV:
        r, Q, M = RES[l], LEVEL_Q[l], LEVEL_M[l]
        o = SLOT_OFF[l]
        rp1 = r + 1
        seg = pk[OFFS[l] : OFFS[l + 1]]
        if Q == 1:
            ii, jj, kk = np.meshgrid(np.arange(r), np.arange(r), np.arange(r), indexing="ij")
            # reference indexes the (r+1)^3 table with strides r^2, r, 1
            n0 = ((ii * r + jj) * r + kk).ravel()
            lane = 0
            for di in (0, 1):
                for dj in (0, 1):
                    for dk in (0, 1):
                        tabx[o : o + r * r * r, lane] = seg[n0 + di * r * r + dj * r + dk]
                        lane += 1
        else:
            ii, jj, mm = np.meshgrid(np.arange(r), np.arange(r), np.arange(M), indexing="ij")
            if l == 7:
                for di in (0, 1):
                    oo = o if di == 0 else SLOT_OFF7B
                    lane = 0
                    for dj in (0, 1):
                        for t in range(Q + 1):
                            kidx = np.minimum(mm * Q + t, r)
                            row = ((ii + di) * r + (jj + dj)) * r + kidx
                            src = np.where(mm * Q + t <= r, seg[row], 0)
                            tabx[oo : oo + r * r * M, lane] = src.ravel()
                            lane += 1
            else:
                lane = 0
                for di in (0, 1):
                    for dj in (0, 1):
                        for t in range(Q + 1):
                            kidx = np.minimum(mm * Q + t, r)
                            row = ((ii + di) * r + (jj + dj)) * r + kidx
                            src = np.where(mm * Q + t <= r, seg[row], 0)
                            tabx[o : o + r * r * M, lane] = src.ravel()
                            lane += 1
    for l in HASH_LV:
        o, ns = SLOT_OFF[l], N_SLOTS[l]
        seg = pk[OFFS[l] : OFFS[l + 1]]
        tabx[o : o + ns, 0:16] = seg.reshape(ns, 16)
    return tabx.view(np.int32)


def build_cvec():
    cv = np.zeros(128, dtype=np.float32)
    for l in range(N_LEVELS):
        grid = np.float32(2.0) / np.float32(RES[l])
        cv[l] = np.float32(1.0) / grid
        cv[16 + l] = np.float32(RES[l] - 1)
        cv[32 + l] = grid
    for l in DENSE_LV:
        r, Q, M = RES[l], LEVEL_Q[l], LEVEL_M[l]
        cv[48 + l] = np.float32(r * M)
        cv[56 + l] = np.float32(M)
        cv[64 + l] = np.float32(1.0 / Q)
        cv[72 + l] = np.float32(Q)
    return cv.reshape(1, 128)


_NC_CACHE = {}
TRACE = False
LAST_NS = None


def _get_nc(slots, S):
    key = (slots, S)
    if key not in _NC_CACHE:
        _NC_CACHE[key] = build_kernel(slots, S)
    return _NC_CACHE[key]


def kernel(x: np.ndarray, tables: np.ndarray) -> np.ndarray:
    global LAST_NS
    from concourse.bass_utils import run_bass_kernel_spmd

    B = x.shape[0]
    per_core = B // N_CORES
    slots = per_core // P
    S = min(32, slots)
    nc = _get_nc(slots, S)
    tabx = build_tabx(tables)
    cv = build_cvec()
    in_maps = []
    for c in range(N_CORES):
        xs = np.ascontiguousarray(
            x[c * per_core : (c + 1) * per_core].reshape(P, slots, 3)
        ).astype(np.float32)
        in_maps.append({"x": xs, "tabx": tabx, "cvec": cv})
    kw = {"trace": True} if TRACE else {}
    res = run_bass_kernel_spmd(nc, in_maps, core_ids=list(range(N_CORES)), **kw)
    LAST_NS = res.exec_time_ns
    outs = [res.results[c]["out"].reshape(per_core, 32) for c in range(N_CORES)]
    return np.concatenate(outs, axis=0).astype(np.float32)


# revision 26
# speedup vs baseline: 1.2424x; 1.0633x over previous
"""HashEmbedder3D Trainium2 kernel v6.

Key changes vs v2 baseline:
- dma_gather with single_packet=False + indices replicated across all 8
  16-partition groups allows 8192-idx instructions (994ns fixed cost
  amortized 8x).
- Dense levels 0-3: slot-per-voxel-base block tables (one 32B descriptor
  fetches all 8 corners; no select trees). Levels 4-7: Q-packed slots
  (one descriptor + small k-offset select tree).
- Hash levels 8-15: one 64B slot read per corner (8 classes), extraction
  via 4-round select tree, gathers batched 2 classes per instruction.
- Index wrap/transpose done by DRAM-bounce DMA + DVE interleave + SBUF
  broadcast DMAs; Pool engine only runs SWDGE gathers.
"""
import math
import sys

import numpy as np

sys.path.insert(0, "/opt/trn_rl_repo")

from concourse import bacc, bass, mybir
import concourse.tile as tile
from concourse import library_config

N_LEVELS = 16
F = 2
LOG2_T = 19
T = 1 << LOG2_T
BASE, FINEST = 16, 512
B_GROWTH = float(np.exp((np.log(np.float32(FINEST)) - np.log(np.float32(BASE))) / np.float32(N_LEVELS - 1)))
RES = [math.floor(BASE * B_GROWTH**i) for i in range(N_LEVELS)]
SIZES = [(r + 1) ** 3 if r**3 < T else T for r in RES]
OFFS = np.concatenate([[0], np.cumsum(SIZES)]).tolist()
TOTAL_ROWS = OFFS[-1]
PRIMES = [1, 2654435761, 805459861]
N_POINTS = 1048576
N_CORES = 8
P = 128

DT = mybir.dt
AL = mybir.AluOpType

DENSE_LV = list(range(8))
HASH_LV = list(range(8, 16))

# dense level slot geometry: levels 0-3 slot-per-base, 4-7 Q-packed
LEVEL_Q = {0: 1, 1: 1, 2: 1, 3: 1, 4: 2, 5: 4, 6: 8, 7: 16}
LEVEL_M = {}
N_SLOTS, ELEM = {}, {}
for l in DENSE_LV:
    r, Q = RES[l], LEVEL_Q[l]
    M = -(-r // Q)
    LEVEL_M[l] = M
    N_SLOTS[l] = r * r * M
    ELEM[l] = 8 if Q == 1 else 4 * (Q + 1)
ELEM[7] = 2 * (LEVEL_Q[7] + 1)  # level 7 split in two di-halves of 2x17 lanes
for l in HASH_LV:
    N_SLOTS[l], ELEM[l] = T // 16, 16
SLOT_OFF = {}
_a = 0
for l in range(N_LEVELS):
    SLOT_OFF[l] = _a
    _a += N_SLOTS[l]
SLOT_OFF7B = _a  # second (di=1) half of level 7
_a += N_SLOTS[7]
TOT_SLOTS = _a
assert all(N_SLOTS[l] <= 32768 for l in range(N_LEVELS)), N_SLOTS

# class layout: dense levels are classes 0-7; hash level l corner m is
# class 8 + (l-8)*8 + m. Each class contributes S indices per tile.
NCC = 8 + 8 * 8  # 72


def _i32(v):
    return int(np.int32(np.uint32(v)))


MAX_GIDX = 8192


def dma_gather_raw(eng, out_ap, in_ap, idxs_ap, num_idxs, elem_size, elem_step=64):
    stride_bytes = elem_step * 4
    assert stride_bytes % 256 == 0
    _in_ap = eng.lower_ap_dma(in_ap, for_custom_bir_dma=True)
    _idxs_ap = eng.lower_ap(idxs_ap)
    _out_ap = eng.lower_ap(out_ap)
    return eng.add_instruction(
        mybir.InstDMAGatherAnt(
            name=eng.bass.get_next_instruction_name(),
            ins=[*_in_ap, _idxs_ap, eng.lower_val_access(eng.to_reg(num_idxs))],
            outs=[_out_ap],
            transpose=False,
            num_idxs=num_idxs,
            elem_size=elem_size,
            stride_bytes_256=stride_bytes // 256,
            gen_mode=0,
            single_packet=num_idxs <= 1024,
            queue_num=0,
            sbuf_tokens_per_rank=0,
            sbuf_free_dim_per_rank=0,
            sbuf_free_dim_pad_per_rank=0,
            sbuf_byte_offset=0,
        )
    )


def apd(tap, off, dims):
    return bass.AP(tap.tensor, tap.offset + off, [list(d) for d in dims])


def build_kernel(slots_total, S):
    n_outer = slots_total // S
    assert n_outer * S == slots_total

    nc = bacc.Bacc(None, target_bir_lowering=False, debug=False)
    x_in = nc.dram_tensor("x", [P, slots_total, 3], DT.float32, kind="ExternalInput")
    tabx = nc.dram_tensor("tabx", [TOT_SLOTS, 64], DT.int32, kind="ExternalInput")
    cvec_in = nc.dram_tensor("cvec", [1, 128], DT.float32, kind="ExternalInput")
    out = nc.dram_tensor("out", [P, slots_total, 32], DT.float32, kind="ExternalOutput")

    nW = NCC * S  # idx ints per partition per tile

    with tile.TileContext(nc) as tc:
        with (
            tc.tile_pool(name="big", bufs=1) as bigp,
            tc.tile_pool(name="dbl", bufs=1) as dblp,
            tc.tile_pool(name="work", bufs=1) as workp,
        ):
            cv = bigp.tile([P, 128], DT.float32, tag="cv", name="cv")
            nc.sync.dma_start(cv[:], apd(cvec_in[:], 0, [[0, P], [1, 128]]))

            def cvb3(col, n, w=8):
                # [P, n, w] view of per-level const at cv[col:col+w]
                return apd(cv, col, [list(cv.ap[0]), [0, n], [1, w]])

            def cvb4(col, a, b, w=8):
                return apd(cv, col, [list(cv.ap[0]), [0, a], [0, b], [1, w]])

            x_t = bigp.tile([P, S, 3], DT.float32, tag="x_t", name="x_t")
            w_bf = None
            idxb = bigp.tile([P, nW], DT.int16, tag="idxb", name="idxb")
            hm = None
            koffs = None
            scr = bigp.tile([P, nW], DT.int16, tag="scr", name="scr", space="DRAM")
            wt = bigp.tile([P, 2 * nW], DT.int16, tag="wt", name="wt")
            wrp = None
            gdd = [
                bigp.tile([P, 68 * S], DT.int32, tag=f"gdd{i}", name=f"gdd{i}")
                for i in range(2)
            ]
            gdh = [
                bigp.tile([P, 2 * 16 * S], DT.int32, tag=f"gdh{i}", name=f"gdh{i}")
                for i in range(3)
            ]
            hcor = bigp.tile([P, 8 * S], DT.int32, tag="hcor", name="hcor")
            osb = bigp.tile([P, S, 32], DT.float32, tag="osb", name="osb")
            bli_d = bigp.tile([P, S, 3, 8], DT.int32, tag="bli_d", name="bli_d")
            bli_h = bigp.tile([P, S, 3, 8], DT.int32, tag="bli_h", name="bli_h")

            nc.vector.memset(wt[:], 0)
            _wrp0 = dblp.tile([P, 8 * nW], DT.int16, tag="wrp", name="wrp_init")
            nc.vector.memset(_wrp0[:], 0)

            with tc.For_i(
                0,
                slots_total,
                S,
                hint_engines=(mybir.EngineType.DVE, mybir.EngineType.Pool),
            ) as st:
                nc.sync.dma_start(x_t[:], x_in[:, bass.ds(st, S), :])
                w_bf = dblp.tile([P, S, 48], DT.bfloat16, tag="w_bf", name="w_bf")
                hm = dblp.tile([P, 64 * S], DT.int32, tag="hm", name="hm")
                koffs = dblp.tile([P, S, 8], DT.int32, tag="koffs", name="koffs")
                wrp = dblp.tile([P, 8 * nW], DT.int16, tag="wrp", name="wrp")

                # ================= phase 1: voxel coords + weights ============
                xc = x_t

                for half, lv0 in ((0, 0), (1, 8)):
                    bli = bli_d if half == 0 else bli_h
                    tf = workp.tile([P, S, 3, 8], DT.float32, tag="tf", name="tf")
                    fi = workp.tile([P, S, 3, 8], DT.int32, tag="fi", name="fi")
                    ff = workp.tile([P, S, 3, 8], DT.float32, tag="ff", name="ff")
                    blf = workp.tile([P, S, 3, 8], DT.float32, tag="blf", name="blf")
                    su = workp.tile([P, S, 3, 8], DT.float32, tag="su", name="su")
                    xb = apd(xc, 0, [list(xc.ap[0]), [3, S], [1, 3], [0, 8]])
                    xbu = apd(x_t, 0, [list(x_t.ap[0]), [3, S], [1, 3], [0, 8]])
                    nc.vector.tensor_scalar(out=tf[:], in0=xb, op0=AL.add, scalar1=1.0, scalar2=None)
                    nc.vector.tensor_tensor(out=tf[:], in0=tf[:], in1=cvb4(lv0, S, 3), op=AL.mult)
                    nc.vector.tensor_copy(out=fi[:], in_=tf[:])
                    nc.vector.tensor_copy(out=ff[:], in_=fi[:])
                    nc.vector.tensor_tensor(out=blf[:], in0=ff[:], in1=tf[:], op=AL.is_gt)
                    nc.vector.tensor_tensor(out=blf[:], in0=ff[:], in1=blf[:], op=AL.subtract)
                    nc.vector.tensor_tensor(out=blf[:], in0=blf[:], in1=cvb4(16 + lv0, S, 3), op=AL.min)
                    nc.vector.tensor_copy(out=bli[:], in_=blf[:])
                    # x in [-1,1) => w = (x+1)/grid - bl = tf - blf exactly
                    nc.vector.tensor_tensor(out=su[:], in0=tf[:], in1=blf[:], op=AL.subtract)
                    wdst = apd(w_bf, 3 * lv0, [list(w_bf.ap[0]), [48, S], [1, 3], [3, 8]])
                    nc.vector.tensor_copy(out=wdst, in_=su[:])

                # ================= dense slot ids =============================
                i_ = bli_d[:, :, 0, :]
                j_ = bli_d[:, :, 1, :]
                k_ = bli_d[:, :, 2, :]
                kq = workp.tile([P, S, 8], DT.int32, tag="kq", name="kq")
                sid = workp.tile([P, S, 8], DT.int32, tag="sid", name="sid")
                t1 = workp.tile([P, S, 8], DT.int32, tag="t1d", name="t1d")
                nc.vector.tensor_copy(out=kq[:], in_=k_)
                for l in range(4, 8):
                    q = LEVEL_Q[l].bit_length() - 1
                    nc.vector.tensor_scalar(out=kq[:, :, l], in0=k_[:, :, l], op0=AL.logical_shift_right, scalar1=q, scalar2=None)
                # koff = k - kq*Q  (only levels 4-7 used)
                nc.vector.tensor_tensor(out=koffs[:], in0=kq[:], in1=apd(cv, 72, [list(cv.ap[0]), [0, S], [1, 8]]), op=AL.mult)
                nc.vector.tensor_tensor(out=koffs[:], in0=k_, in1=koffs[:], op=AL.subtract)
                # sid = i*A + j*B + kq
                nc.vector.tensor_tensor(out=t1[:], in0=i_, in1=apd(cv, 48, [list(cv.ap[0]), [0, S], [1, 8]]), op=AL.mult)
                nc.vector.tensor_tensor(out=sid[:], in0=j_, in1=apd(cv, 56, [list(cv.ap[0]), [0, S], [1, 8]]), op=AL.mult)
                nc.vector.tensor_tensor(out=sid[:], in0=sid[:], in1=t1[:], op=AL.add)
                nc.vector.tensor_tensor(out=sid[:], in0=sid[:], in1=kq[:], op=AL.add)
                # write dense classes: idxb[:, l*S + x] = sid[:, x, l]
                nc.vector.tensor_copy(
                    out=apd(idxb, 0, [list(idxb.ap[0]), [1, S], [S, 8]]),
                    in_=sid[:],
                )
                # dense k_off select masks, all levels at once, per bit
                dmb = []
                for b in range(4):
                    mbt = workp.tile([P, S, 8], DT.int32, tag=f"dmb{b}", name=f"dmb{b}")
                    nc.vector.tensor_scalar(out=mbt[:], in0=koffs[:], op0=AL.logical_shift_right, scalar1=b, op1=AL.bitwise_and, scalar2=1)
                    nc.vector.tensor_scalar(out=mbt[:], in0=mbt[:], op0=AL.mult, scalar1=-1, scalar2=None)
                    dmb.append(mbt)

                # ================= hash slot ids ==============================
                ih = bli_h[:, :, 0, :]
                jh = bli_h[:, :, 1, :]
                kh = bli_h[:, :, 2, :]
                mt1 = workp.tile([P, S, 8], DT.int32, tag="mt1", name="mt1")
                mt2 = workp.tile([P, S, 8], DT.int32, tag="mt2", name="mt2")
                mt3 = workp.tile([P, S, 8], DT.int32, tag="mt3", name="mt3")

                def ts(o, i, op, s):
                    nc.vector.tensor_scalar(out=o, in0=i, op0=op, scalar1=s, scalar2=None)

                def tt(o, a, b, op):
                    nc.vector.tensor_tensor(out=o, in0=a, in1=b, op=op)

                def mul32(dst, src, prime):
                    Hp, Lp = (prime >> 16) & 0xFFFF, prime & 0xFFFF
                    Hs = Hp - 32768 if Hp >= 32768 else Hp
                    ts(mt1[:], src, AL.mult, Lp)
                    ts(mt2[:], src, AL.mult, Hs)
                    if Hp >= 32768:
                        ts(mt3[:], src, AL.logical_shift_left, 15)
                        ts(mt3[:], mt3[:], AL.bitwise_and, 0xFFFF)
                        ts(mt2[:], mt2[:], AL.bitwise_and, 0xFFFF)
                        tt(mt2[:], mt2[:], mt3[:], AL.add)
                    ts(mt2[:], mt2[:], AL.bitwise_and, 0xFFFF)
                    ts(mt3[:], mt1[:], AL.logical_shift_right, 16)
                    tt(mt2[:], mt2[:], mt3[:], AL.add)
                    ts(mt2[:], mt2[:], AL.bitwise_and, 0xFFFF)
                    ts(mt2[:], mt2[:], AL.logical_shift_left, 16)
                    ts(mt1[:], mt1[:], AL.bitwise_and, 0xFFFF)
                    tt(dst, mt2[:], mt1[:], AL.bitwise_or)

                def add32(dst, src, const):
                    cl, ch = const & 0xFFFF, (const >> 16) & 0xFFFF
                    ts(mt1[:], src, AL.bitwise_and, 0xFFFF)
                    ts(mt1[:], mt1[:], AL.add, cl)
                    ts(mt2[:], src, AL.logical_shift_right, 16)
                    ts(mt2[:], mt2[:], AL.bitwise_and, 0xFFFF)
                    ts(mt2[:], mt2[:], AL.add, ch)
                    ts(mt3[:], mt1[:], AL.logical_shift_right, 16)
                    tt(mt2[:], mt2[:], mt3[:], AL.add)
                    ts(mt2[:], mt2[:], AL.bitwise_and, 0xFFFF)
                    ts(mt2[:], mt2[:], AL.logical_shift_left, 16)
                    ts(mt1[:], mt1[:], AL.bitwise_and, 0xFFFF)
                    tt(dst, mt2[:], mt1[:], AL.bitwise_or)

                jp0 = workp.tile([P, S, 8], DT.int32, tag="jp0", name="jp0")
                jp1 = workp.tile([P, S, 8], DT.int32, tag="jp1", name="jp1")
                kp0 = workp.tile([P, S, 8], DT.int32, tag="kp0", name="kp0")
                kp1 = workp.tile([P, S, 8], DT.int32, tag="kp1", name="kp1")
                ii1 = workp.tile([P, S, 8], DT.int32, tag="ii1", name="ii1")
                rr = workp.tile([P, S, 8], DT.int32, tag="rr", name="rr")
                rr2 = workp.tile([P, S, 8], DT.int32, tag="rr2", name="rr2")
                mul32(jp0[:], jh, PRIMES[1])
                add32(jp1[:], jp0[:], PRIMES[1])
                mul32(kp0[:], kh, PRIMES[2])
                add32(kp1[:], kp0[:], PRIMES[2])
                ts(ii1[:], ih, AL.add, 1)
                # vectorized corner ids: xab[c] = (ih|ii1) ^ (jp0|jp1), c=di*2+dj
                xab = workp.tile([P, S, 4, 8], DT.int32, tag="xab", name="xab")
                tt(xab[:, :, 0, :], ih, jp0[:], AL.bitwise_xor)
                tt(xab[:, :, 1, :], ih, jp1[:], AL.bitwise_xor)
                tt(xab[:, :, 2, :], ii1[:], jp0[:], AL.bitwise_xor)
                tt(xab[:, :, 3, :], ii1[:], jp1[:], AL.bitwise_xor)
                # rr_all[x, m, l] with m = c*2+dk
                rr_all = workp.tile([P, S, 8, 8], DT.int32, tag="rr_all", name="rr_all")
                for dk in (0, 1):
                    tt(
                        apd(rr_all, dk * 8, [list(rr_all.ap[0]), [64, S], [16, 4], [1, 8]]),
                        apd(xab, 0, [list(xab.ap[0]), [32, S], [8, 4], [1, 8]]),
                        (kp1 if dk else kp0)[:].to_broadcast([P, S, 4, 8]) if False else apd(kp1 if dk else kp0, 0, [list(kp0.ap[0]), [8, S], [0, 4], [1, 8]]),
                        AL.bitwise_xor,
                    )
                ts(rr_all[:], rr_all[:], AL.bitwise_and, T - 1)
                sh = workp.tile([P, S, 8, 8], DT.int32, tag="rrsh", name="rrsh")
                ts(sh[:], rr_all[:], AL.logical_shift_right, 4)
                nc.vector.tensor_copy(
                    out=apd(idxb, 8 * S, [list(idxb.ap[0]), [1, S], [S, 8], [8 * S, 8]]),
                    in_=sh[:],
                )
                ts(sh[:], rr_all[:], AL.bitwise_and, 15)
                nc.vector.tensor_copy(
                    out=apd(hm, 0, [list(hm.ap[0]), [1, S], [S, 8], [8 * S, 8]]),
                    in_=sh[:],
                )

                # ============== idx transpose to wrapped-16 + replicate =======
                nc.sync.dma_start(scr[:], idxb[:])
                # Only partitions 16:32 (group 1) are read by the SWDGE
                # cores (each core g reads cols == g mod 8 there). Build the
                # wrapped matrix in group 1; mirror to group 0 for CoreSim,
                # whose interpreter consumes group 0.
                # 8 ping-ponged chunks: DVE only stalls on the first read
                def rd(h):
                    half = wt[:, (h % 2) * nW : (h % 2 + 1) * nW]
                    nc.sync.dma_start(
                        half[0:16, :],
                        apd(scr, h * 16 * nW, [[nW, 16], [1, nW]]),
                    )

                def il(h):
                    half = wt[:, (h % 2) * nW : (h % 2 + 1) * nW]
                    nc.gpsimd.tensor_copy(
                        out=apd(wrp, h, [list(wrp.ap[0]), [8, nW]]),
                        in_=apd(half, 0, [list(half.ap[0]), [1, nW]]),
                    )

                rd(0)
                rd(1)
                nc.gpsimd.load_library(library_config.standard)
                for h in range(8):
                    il(h)
                    if h + 2 < 8:
                        rd(h + 2)
                nc.gpsimd.load_library(library_config.mlp)
                nc.sync.dma_start(wrp[16:32, :], wrp[0:16, :])

                # ================= gathers ====================================
                def lerp(dst, lo, hi, w):
                    nc.vector.tensor_tensor(out=dst, in0=hi, in1=lo, op=AL.subtract)
                    nc.vector.tensor_tensor(out=dst, in0=dst, in1=w, op=AL.mult)
                    nc.vector.tensor_tensor(out=dst, in0=dst, in1=lo, op=AL.add)

                def wof(l, d, reps, minor):
                    # weight w_bf[:, x, 3l+d] broadcast directly as bf16:
                    # minor=True -> [S, reps] (x outer), else [reps, S]
                    if minor:
                        return apd(w_bf, 3 * l + d, [list(w_bf.ap[0]), [48, S], [0, reps]])
                    return apd(w_bf, 3 * l + d, [list(w_bf.ap[0]), [0, reps], [48, S]])

                def unpack(src_ap, n, tag):
                    ef = workp.tile([P, 2, n], DT.int32, tag=f"ef{tag}", name=f"ef{tag}")
                    nc.vector.tensor_scalar(out=ef[:, 0, :], in0=src_ap, op0=AL.logical_shift_left, scalar1=16, scalar2=None)
                    nc.vector.tensor_scalar(out=ef[:, 1, :], in0=src_ap, op0=AL.bitwise_and, scalar1=_i32(0xFFFF0000), scalar2=None)
                    return ef

                def f32v(t, dims, off=0):
                    return apd(t, off, [list(t.ap[0])] + [list(d) for d in dims]).bitcast(DT.float32)

                # ---- dense levels ----
                def ktree(l, gd, el, ngrp, q, Q, tag):
                    # select k_off window over bits q-1..0; gd lanes
                    # [x*el + g*(Q+1) + t]; returns tile [P, S, ngrp, 2]
                    cur = None
                    for b in range(q - 1, -1, -1):
                        wnew = 2**b + 1 if b > 0 else 2
                        half = 2**b
                        nxt = workp.tile([P, S, ngrp, wnew], DT.int32, tag=f"dt{tag}{b}", name=f"dt{tag}{b}")
                        if cur is None:
                            lo = apd(gd, 0, [list(gd.ap[0]), [el, S], [Q + 1, ngrp], [1, wnew]])
                            hi = apd(gd, half, [list(gd.ap[0]), [el, S], [Q + 1, ngrp], [1, wnew]])
                        else:
                            lo = apd(cur, 0, [list(cur.ap[0]), [cur.ap[1][0], S], [cur.ap[2][0], ngrp], [1, wnew]])
                            hi = apd(cur, half, [list(cur.ap[0]), [cur.ap[1][0], S], [cur.ap[2][0], ngrp], [1, wnew]])
                        mbb = apd(dmb[b], l, [list(dmb[b].ap[0]), [8, S], [0, ngrp], [0, wnew]])
                        nc.vector.tensor_tensor(out=nxt[:], in0=lo, in1=hi, op=AL.bitwise_xor)
                        nc.vector.tensor_tensor(out=nxt[:], in0=nxt[:], in1=mbb, op=AL.bitwise_and)
                        nc.vector.tensor_tensor(out=nxt[:], in0=nxt[:], in1=lo, op=AL.bitwise_xor)
                        cur = nxt
                    return cur

                for l in DENSE_LV:
                    el = ELEM[l]
                    Q = LEVEL_Q[l]
                    q = Q.bit_length() - 1
                    gd = gdd[l % 2]
                    if l == 7:
                        baseA = tabx[SLOT_OFF[7] : SLOT_OFF[7] + N_SLOTS[7], 0:el]
                        baseB = tabx[SLOT_OFF7B : SLOT_OFF7B + N_SLOTS[7], 0:el]
                        gdA, gdB = gdd[0], gdd[1]
                        for gdX, baseX in ((gdA, baseA), (gdB, baseB)):
                            gview = apd(gdX, 0, [list(gdX.ap[0]), [el, S], [1, el]])
                            dma_gather_raw(
                                nc.gpsimd, gview, baseX,
                                wrp[:, l * 8 * S : (l + 1) * 8 * S], S * P, el,
                            )
                        curA = ktree(l, gdA, el, 2, q, Q, "2a")
                        curB = ktree(l, gdB, el, 2, q, Q, "2b")
                        comb = workp.tile([P, S, 8], DT.int32, tag="comb7", name="comb7")
                        nc.vector.tensor_copy(
                            out=apd(comb, 0, [list(comb.ap[0]), [8, S], [1, 4]]),
                            in_=apd(curA, 0, [list(curA.ap[0]), [4, S], [1, 4]]),
                        )
                        nc.vector.tensor_copy(
                            out=apd(comb, 4, [list(comb.ap[0]), [8, S], [1, 4]]),
                            in_=apd(curB, 0, [list(curB.ap[0]), [4, S], [1, 4]]),
                        )
                        csrc = apd(comb, 0, [list(comb.ap[0]), [1, 8 * S]])
                    else:
                        base = tabx[SLOT_OFF[l] : SLOT_OFF[l] + N_SLOTS[l], 0:el]
                        gview = apd(gd, 0, [list(gd.ap[0]), [el, S], [1, el]])
                        dma_gather_raw(
                            nc.gpsimd, gview, base,
                            wrp[:, l * 8 * S : (l + 1) * 8 * S], S * P, el,
                        )
                        if Q == 1:
                            csrc = apd(gd, 0, [list(gd.ap[0]), [1, 8 * S]])
                        else:
                            cur = ktree(l, gd, el, 4, q, Q, "4g")
                            csrc = apd(cur, 0, [list(cur.ap[0]), [1, 8 * S]])
                    # corners in x-major layout: lane x*8 + m; both features fused
                    ef = unpack(csrc, 8 * S, "d")
                    def wv(d, dims):
                        return apd(w_bf, 3 * l + d, [list(w_bf.ap[0])] + dims)
                    efv = lambda off, dims: apd(ef, off, [list(ef.ap[0])] + dims).bitcast(DT.float32)
                    xt = workp.tile([P, 2, S, 4], DT.float32, tag="xtd", name="xtd")
                    lerp(xt[:], efv(0, [[8 * S, 2], [8, S], [1, 4]]), efv(4, [[8 * S, 2], [8, S], [1, 4]]), wv(0, [[0, 2], [48, S], [0, 4]]))
                    yt = workp.tile([P, 2, S, 2], DT.float32, tag="ytd", name="ytd")
                    lerp(yt[:], apd(xt, 0, [list(xt.ap[0]), [4 * S, 2], [4, S], [1, 2]]), apd(xt, 2, [list(xt.ap[0]), [4 * S, 2], [4, S], [1, 2]]), wv(1, [[0, 2], [48, S], [0, 2]]))
                    od = apd(osb, 2 * l, [list(osb.ap[0]), [1, 2], [32, S]])
                    lerp(od, apd(yt, 0, [list(yt.ap[0]), [2 * S, 2], [2, S]]), apd(yt, 1, [list(yt.ap[0]), [2 * S, 2], [2, S]]), wv(2, [[0, 2], [48, S]]))

                # ---- hash levels ----
                for l in HASH_LV:
                    lh = l - 8
                    base = tabx[SLOT_OFF[l] : SLOT_OFF[l] + N_SLOTS[l], 0:16]
                    cls0 = 8 + lh * 8

                    hmv_l = apd(hm, lh * 8 * S, [list(hm.ap[0]), [1, 8 * S]])
                    mbl = {}
                    for b in range(4):
                        mb = workp.tile([P, 8 * S], DT.int32, tag=f"hmk{b}", name=f"hmk{b}")
                        nc.vector.tensor_scalar(out=mb[:], in0=hmv_l, op0=AL.logical_shift_right, scalar1=b, op1=AL.bitwise_and, scalar2=1)
                        nc.vector.tensor_scalar(out=mb[:], in0=mb[:], op0=AL.mult, scalar1=-1, scalar2=None)
                        mbl[b] = mb

                    def hgather(h2):
                        gd = gdh[(4 * lh + h2) % 3]
                        gview = apd(gd, 0, [list(gd.ap[0]), [16, 2 * S], [1, 16]])
                        dma_gather_raw(
                            nc.gpsimd, gview, base,
                            wrp[:, (cls0 + 2 * h2) * 8 * S : (cls0 + 2 * h2 + 2) * 8 * S],
                            2 * S * P, 16,
                        )

                    def hextract(h2):
                        gd = gdh[(4 * lh + h2) % 3]
                        cur = None
                        for b in range(3, -1, -1):
                            half = 2**b
                            if b == 0:
                                hc = hcor[:, h2 * 2 * S : (h2 + 1) * 2 * S]
                                nxt = None
                                ov = apd(hc, 0, [list(hc.ap[0]), [1, 2 * S], [0, 1]])
                            else:
                                nxt = workp.tile([P, 2 * S, half], DT.int32, tag=f"ht{b}", name=f"ht{b}")
                                ov = nxt[:]
                            if cur is None:
                                lo = apd(gd, 0, [list(gd.ap[0]), [16, 2 * S], [1, half]])
                                hi = apd(gd, half, [list(gd.ap[0]), [16, 2 * S], [1, half]])
                            else:
                                lo = cur[:, :, 0:half]
                                hi = cur[:, :, half : 2 * half]
                            mb = mbl[b]
                            mbb = apd(mb, h2 * 2 * S, [list(mb.ap[0]), [1, 2 * S], [0, half]])
                            nc.vector.tensor_tensor(out=ov, in0=lo, in1=hi, op=AL.bitwise_xor)
                            nc.vector.tensor_tensor(out=ov, in0=ov, in1=mbb, op=AL.bitwise_and)
                            nc.vector.tensor_tensor(out=ov, in0=ov, in1=lo, op=AL.bitwise_xor)
                            cur = nxt

                    hgather(0)
                    hgather(1)
                    hextract(0)
                    hgather(2)
                    hextract(1)
                    hgather(3)
                    hextract(2)
                    hextract(3)
                    # hcor: [P, 8S] corner values, class-major (c*S + x); fused feats
                    ef = unpack(hcor[:, :], 8 * S, "h")
                    def wvh(d, reps):
                        return apd(w_bf, 3 * l + d, [list(w_bf.ap[0]), [0, 2], [0, reps], [48, S]])
                    efvh = lambda off, n: apd(ef, off, [list(ef.ap[0]), [8 * S, 2], [1, n]]).bitcast(DT.float32)
                    xh = workp.tile([P, 2, 4 * S], DT.float32, tag="xth", name="xth")
                    lerp(xh[:], efvh(0, 4 * S), efvh(4 * S, 4 * S), wvh(0, 4))
                    yh = workp.tile([P, 2, 2 * S], DT.float32, tag="yth", name="yth")
                    lerp(yh[:], apd(xh, 0, [list(xh.ap[0]), [4 * S, 2], [1, 2 * S]]), apd(xh, 2 * S, [list(xh.ap[0]), [4 * S, 2], [1, 2 * S]]), wvh(1, 2))
                    oh = apd(osb, 2 * l, [list(osb.ap[0]), [1, 2], [32, S]])
                    lerp(oh, apd(yh, 0, [list(yh.ap[0]), [2 * S, 2], [1, S]]), apd(yh, S, [list(yh.ap[0]), [2 * S, 2], [1, S]]), wvh(2, 1))

                nc.gpsimd.dma_start(out[:, bass.ds(st, S), :], osb[:])
    nc.compile()
    return nc


# ---------------- host side ----------------

def _pack_bf16(t):
    u = np.ascontiguousarray(t.astype(np.float32)).view(np.uint32).astype(np.uint64)
    r = ((u + 0x7FFF + ((u >> 16) & 1)) >> 16).astype(np.uint32)
    return r[:, 0] | (r[:, 1] << np.uint32(16))


def build_tabx(tables):
    pk = _pack_bf16(tables)
    tabx = np.zeros((TOT_SLOTS, 64), dtype=np.uint32)
    for l in DENSE_LV:
        r, Q, M = RES[l], LEVEL_Q[l], LEVEL_M[l]
        o = SLOT_OFF[l]
        rp1 = r + 1
        seg = pk[OFFS[l] : OFFS[l + 1]]
        if Q == 1:
            ii, jj, kk = np.meshgrid(np.arange(r), np.arange(r), np.arange(r), indexing="ij")
            # reference indexes the (r+1)^3 table with strides r^2, r, 1
            n0 = ((ii * r + jj) * r + kk).ravel()
            lane = 0
            for di in (0, 1):
                for dj in (0, 1):
                    for dk in (0, 1):
                        tabx[o : o + r * r * r, lane] = seg[n0 + di * r * r + dj * r + dk]
                        lane += 1
        else:
            ii, jj, mm = np.meshgrid(np.arange(r), np.arange(r), np.arange(M), indexing="ij")
            if l == 7:
                for di in (0, 1):
                    oo = o if di == 0 else SLOT_OFF7B
                    lane = 0
                    for dj in (0, 1):
                        for t in range(Q + 1):
                            kidx = np.minimum(mm * Q + t, r)
                            row = ((ii + di) * r + (jj + dj)) * r + kidx
                            src = np.where(mm * Q + t <= r, seg[row], 0)
                            tabx[oo : oo + r * r * M, lane] = src.ravel()
                            lane += 1
            else:
                lane = 0
                for di in (0, 1):
                    for dj in (0, 1):
                        for t in range(Q + 1):
                            kidx = np.minimum(mm * Q + t, r)
                            row = ((ii + di) * r + (jj + dj)) * r + kidx
                            src = np.where(mm * Q + t <= r, seg[row], 0)
                            tabx[o : o + r * r * M, lane] = src.ravel()
                            lane += 1
    for l in HASH_LV:
        o, ns = SLOT_OFF[l], N_SLOTS[l]
        seg = pk[OFFS[l] : OFFS[l + 1]]
        tabx[o : o + ns, 0:16] = seg.reshape(ns, 16)
    return tabx.view(np.int32)


def build_cvec():
    cv = np.zeros(128, dtype=np.float32)
    for l in range(N_LEVELS):
        grid = np.float32(2.0) / np.float32(RES[l])
        cv[l] = np.float32(1.0) / grid
        cv[16 + l] = np.float32(RES[l] - 1)
        cv[32 + l] = grid
    for l in DENSE_LV:
        r, Q, M = RES[l], LEVEL_Q[l], LEVEL_M[l]
        cv[48 + l] = np.float32(r * M)
        cv[56 + l] = np.float32(M)
        cv[64 + l] = np.float32(1.0 / Q)
        cv[72 + l] = np.float32(Q)
    return cv.reshape(1, 128)


_NC_CACHE = {}
TRACE = False
LAST_NS = None


def _get_nc(slots, S):
    key = (slots, S)
    if key not in _NC_CACHE:
        _NC_CACHE[key] = build_kernel(slots, S)
    return _NC_CACHE[key]


def kernel(x: np.ndarray, tables: np.ndarray) -> np.ndarray:
    global LAST_NS
    from concourse.bass_utils import run_bass_kernel_spmd

    B = x.shape[0]
    per_core = B // N_CORES
    slots = per_core // P
    S = min(32, slots)
    nc = _get_nc(slots, S)
    tabx = build_tabx(tables)
    cv = build_cvec()
    in_maps = []
    for c in range(N_CORES):
        xs = np.ascontiguousarray(
            x[c * per_core : (c + 1) * per_core].reshape(P, slots, 3)
        ).astype(np.float32)
        in_maps.append({"x": xs, "tabx": tabx, "cvec": cv})
    kw = {"trace": True} if TRACE else {}
    res = run_bass_kernel_spmd(nc, in_maps, core_ids=list(range(N_CORES)), **kw)
    LAST_NS = res.exec_time_ns
    outs = [res.results[c]["out"].reshape(per_core, 32) for c in range(N_CORES)]
    return np.concatenate(outs, axis=0).astype(np.float32)
